# revision 2
# baseline (speedup 1.0000x reference)
"""Trainium2 Bass kernel for nn_LLMCC_74414603370526 (loss_fn) — v2.

Data-parallel over batch: 16 sequences -> 8 cores x 2 sequences each; host
combines the scalar partial losses (the sanctioned all-reduce).

v2 design (instruction-count-minimized vs v1):
  - activations flow feature-major end-to-end; the only PE transposes are
    x (48) and the emb token-major copy needed by the quadruplet gather (48)
  - weights are pre-transposed on the HOST into DMA-friendly layouts; no
    on-device weight transposes
  - biases/residual/softmax-rowsum fold into matmuls (K=1 outer products,
    identity lhsT accumulate, ones column appended to v)
  - LayerNorm runs feature-major: per-token stats via ones/w-bar matmul
    column reductions; scale/shift applied via PE rank-1 outer products
    (g x rstd) and two vector ops per tile
  - CE in [9, T] label-major layout with host-built one-hot labels;
    context-loss label mask is host-built
  - SBUF slots reused via byte-equal tag chains:
    XTOK: x_tok -> emb_tok | XT: xT -> featT | QT: qT -> h1T
    KT: kT -> h2T | WQKT: wqkT -> w1T | WOT: woT -> w2T
"""

import numpy as np
import ml_dtypes

import concourse.bass as bass
import concourse.mybir as mybir
import concourse.tile as tile
from concourse import bacc
from concourse.bass_utils import run_bass_kernel_spmd
from concourse.masks import make_identity

FP32 = mybir.dt.float32
BF16 = mybir.dt.bfloat16
AF = mybir.ActivationFunctionType
ALU = mybir.AluOpType
AX = mybir.AxisListType

B, S, H = 16, 512, 768
NH, HD = 8, 96
NUM_LABELS = 9
MARGIN1, MARGIN2 = 1.0, 0.5
ALPHA, BETA = 0.2, 0.1
EPS = 1e-5

NCORES = 8
BL = B // NCORES          # 2 sequences per core
T = BL * S                # 1024 tokens per core
NT = T // 128             # 8 token tiles
KH = H // 128             # 6 feature tiles
D1, D2, D3 = 1024, 512, 256
KD1, KD2, KD3 = D1 // 128, D2 // 128, D3 // 128
ISQ = 1.0 / float(np.sqrt(HD))
BFD = ml_dtypes.bfloat16

# rowpack offsets (bf16 row: biases / row-vectors)
OF_BQK = 0                       # 1536
OF_BV = OF_BQK + 2 * H           # 776
OF_BO = OF_BV + 776              # 768
OF_B1 = OF_BO + H                # 1024
OF_B2 = OF_B1 + D1               # 512
OF_B3 = OF_B2 + D2               # 256
OF_BCLS = OF_B3 + D3             # 9
OF_S1B = OF_BCLS + NUM_LABELS    # 3
OF_G1 = OF_S1B + 3               # 1024
OF_G2 = OF_G1 + D1               # 512
OF_G3 = OF_G2 + D2               # 256
ROWN = OF_G3 + D3

# ppack (fp32 per-partition pack)
PP_BE1 = 0          # 8 cols
PP_BE2 = 8          # 4
PP_BE3 = 12         # 2
PP_MARG = 14        # margins in rows 0:4 of col 14
PPC = 15

# wbcol (bf16 per-partition colsum-weight pack)
WB1, WB2, WB3 = 0, 6, 14
WBC = 18

GELU_AS_COPY = False   # sim-debug only (CoreSim executor lacks Gelu numerics)

_CACHED = None


def _build(stop_after=None):
    nc = bacc.Bacc(None, target_bir_lowering=False)
    dd = {}

    def di(name, shape, dt=BF16):
        dd[name] = nc.dram_tensor(name, shape, dt, kind="ExternalInput")

    di("x", [T, H])
    di("wqk", [H, 2 * H])
    di("wv97", [H, 776])
    di("wopk", [HD * NH, H])       # rows = (d-within-head, head), cols f
    di("w1t", [H, D1])
    di("w2t", [D1, D2])
    di("w3t", [D2, D3])
    di("wct", [D3, NUM_LABELS])
    di("wrt", [H, NUM_LABELS])     # pre-scaled by 0.2 on host
    di("rowpack", [1, ROWN])
    di("ppack", [128, PPC], FP32)
    di("wbcol", [128, WBC])
    di("onehot", [NUM_LABELS, T])
    di("ctxmask", [1, T - 1])
    di("sel", [T, 6])
    di("combo", [6, 4])
    out_d = nc.dram_tensor("out", [1, 8], FP32, kind="ExternalOutput")

    with tile.TileContext(nc) as tc:
        with nc.allow_low_precision(reason="bf16 PE-transpose PSUM tiles"):
            _body(nc, tc, dd, out_d, stop_after)
    nc.finalize()
    return nc


def _body(nc, tc, dd, out_d, stop_after=None):
    const = tc.alloc_tile_pool(name="const", bufs=1)
    acts = tc.alloc_tile_pool(name="acts", bufs=1)

    def _stop(phase):
        return stop_after == phase

    def fin():
        nc.sync.dma_start(out=out_d[:, :], in_=partials)
        acts.release()
        const.release()


    # ---- constants / inputs ----
    ident = const.tile([128, 128], BF16)
    make_identity(nc, ident)
    ones_row = const.tile([1, 1024], BF16)
    nc.vector.memset(ones_row, 1.0)
    onescol = const.tile([128, 1], BF16)
    nc.vector.memset(onescol, 1.0)
    ones9 = const.tile([NUM_LABELS, 1], BF16)
    nc.vector.memset(ones9, 1.0)
    eps_t = const.tile([1, 1], FP32)
    nc.vector.memset(eps_t, EPS)
    partials = const.tile([1, 8], FP32)
    nc.vector.memset(partials, 0.0)

    x_tok = acts.tile([128, NT, H], BF16, tag="XTOK")
    nc.sync.dma_start(out=x_tok, in_=dd["x"].rearrange("(n p) h -> p n h", p=128))
    wqkT = acts.tile([128, KH, 1536], BF16, tag="WQKT")
    nc.sync.dma_start(out=wqkT, in_=dd["wqk"].rearrange("(f p) c -> p f c", p=128))
    rowpack = const.tile([1, ROWN], BF16)
    nc.sync.dma_start(out=rowpack, in_=dd["rowpack"][:, :])
    wv97T = const.tile([128, KH, 776], BF16)
    nc.gpsimd.dma_start(out=wv97T, in_=dd["wv97"].rearrange("(f p) c -> p f c", p=128))
    woT = acts.tile([HD, NH, H], BF16, tag="WOT")
    nc.gpsimd.dma_start(out=woT, in_=dd["wopk"].rearrange("(p h) f -> p h f", p=HD))
    w3T = const.tile([128, KD2, D3], BF16)
    nc.gpsimd.dma_start(out=w3T, in_=dd["w3t"].rearrange("(f p) c -> p f c", p=128))
    wcT = const.tile([128, KD3, NUM_LABELS], BF16)
    nc.gpsimd.dma_start(out=wcT, in_=dd["wct"].rearrange("(f p) c -> p f c", p=128))
    wrT = const.tile([128, KH, NUM_LABELS], BF16)
    nc.gpsimd.dma_start(out=wrT, in_=dd["wrt"].rearrange("(f p) c -> p f c", p=128))
    ppack = const.tile([128, PPC], FP32)
    nc.gpsimd.dma_start(out=ppack, in_=dd["ppack"][:, :])
    wbcol = const.tile([128, WBC], BF16)
    nc.gpsimd.dma_start(out=wbcol, in_=dd["wbcol"][:, :])
    onehot = const.tile([NUM_LABELS, T], BF16)
    nc.gpsimd.dma_start(out=onehot, in_=dd["onehot"][:, :])
    ctxmask = const.tile([1, T - 1], BF16)
    nc.gpsimd.dma_start(out=ctxmask, in_=dd["ctxmask"][:, :])
    sel = const.tile([128, NT, 6], BF16)
    nc.gpsimd.dma_start(out=sel, in_=dd["sel"].rearrange("(n p) c -> p n c", p=128))
    combo = const.tile([6, 4], BF16)
    nc.gpsimd.dma_start(out=combo, in_=dd["combo"][:, :])

    if _stop("load"):
        fin()
        return

    # ---- P1: x -> xT (feature-major) ----
    xT = acts.tile([128, KH, T], BF16, tag="XT")
    with tc.tile_pool(name="ptp", bufs=6, space="PSUM") as ptp:
        i = 0
        for t in range(NT):
            for f in range(KH):
                ps = ptp.tile([128, 128], BF16, tag="tr", name="tr")
                nc.tensor.transpose(ps, x_tok[:, t, 128 * f:128 * (f + 1)], ident)
                if i % 2 == 0:
                    nc.scalar.activation(out=xT[:, f, 128 * t:128 * (t + 1)],
                                         in_=ps, func=AF.Copy)
                else:
                    nc.vector.tensor_copy(out=xT[:, f, 128 * t:128 * (t + 1)],
                                          in_=ps)
                i += 1

    if _stop("p1"):
        fin()
        return

    # ---- P2+P3: q, k, v ----
    qT = acts.tile([HD, NH, T], BF16, tag="QT")
    kT = acts.tile([HD, NH, T], BF16, tag="KT")
    v97 = acts.tile([128, NT, 776], BF16, tag="V97")
    with tc.tile_pool(name="pq", bufs=4, space="PSUM") as pq, \
         tc.tile_pool(name="pv", bufs=2, space="PSUM") as pv:
        i = 0
        for h in range(NH):
            for w in range(2):      # 0=q, 1=k
                dst = qT if w == 0 else kT
                off = H * w + HD * h
                for c in range(2):
                    ps = pq.tile([HD, 512], FP32, tag="pqk", name="pqk")
                    for f in range(KH):
                        nc.tensor.matmul(ps, wqkT[:, f, off:off + HD],
                                         xT[:, f, 512 * c:512 * (c + 1)],
                                         start=(f == 0), stop=False)
                    nc.tensor.matmul(ps,
                                     rowpack[:, OF_BQK + off:OF_BQK + off + HD],
                                     ones_row[:, 0:512], start=False, stop=True)
                    if i % 2 == 0:
                        nc.scalar.activation(out=dst[:, h, 512 * c:512 * (c + 1)],
                                             in_=ps, func=AF.Copy)
                    else:
                        nc.vector.tensor_copy(out=dst[:, h, 512 * c:512 * (c + 1)],
                                              in_=ps)
                    i += 1
        for t in range(NT):
            for g in range(2):
                ps = pv.tile([128, 388], FP32, tag="pv", name="pv")
                for f in range(KH):
                    nc.tensor.matmul(ps, xT[:, f, 128 * t:128 * (t + 1)],
                                     wv97T[:, f, 388 * g:388 * (g + 1)],
                                     start=(f == 0), stop=False)
                nc.tensor.matmul(ps, ones_row[0:1, 0:128],
                                 rowpack[:, OF_BV + 388 * g:OF_BV + 388 * (g + 1)],
                                 start=False, stop=True)
                if t % 2 == 0:
                    nc.scalar.activation(out=v97[:, t, 388 * g:388 * (g + 1)],
                                         in_=ps, func=AF.Copy)
                else:
                    nc.vector.tensor_copy(out=v97[:, t, 388 * g:388 * (g + 1)],
                                          in_=ps)

    if _stop("qkv"):
        fin()
        return

    # ---- P4: attention (exp without max-shift; rowsum via v ones column) ----
    aoT = acts.tile([HD, BL, NH, S], BF16, tag="AOT")
    with tc.tile_pool(name="pa", bufs=4, space="PSUM") as pa, \
         tc.tile_pool(name="pa2", bufs=2, space="PSUM") as pa2, \
         tc.tile_pool(name="pa3", bufs=2, space="PSUM") as pa3, \
         tc.tile_pool(name="wet", bufs=6) as wet, \
         tc.tile_pool(name="wra", bufs=2) as wra:
        def tail(sh):
            s, h, pao = sh
            rec = wra.tile([1, S], BF16, tag="rec", name="rec")
            nc.vector.reciprocal(out=rec, in_=pao[HD:HD + 1, :])
            prec = pa3.tile([HD, S], FP32, tag="prec", name="prec")
            nc.tensor.matmul(prec, ones_row[0:1, 0:HD], rec, start=True,
                             stop=True)
            aou = wra.tile([HD, S], BF16, tag="aou", name="aou")
            nc.scalar.activation(out=aou, in_=pao[0:HD, :], func=AF.Copy)
            nc.vector.tensor_mul(out=aoT[:, s, h, :], in0=aou, in1=prec)

        pending = None
        for s in range(BL):
            for h in range(NH):
                et = []
                for kt in range(4):
                    psc = pa.tile([128, S], FP32, tag="psc", name="psc")
                    nc.tensor.matmul(
                        psc, kT[:, h, S * s + 128 * kt:S * s + 128 * (kt + 1)],
                        qT[:, h, S * s:S * (s + 1)], start=True, stop=True)
                    e = wet.tile([128, S], BF16, tag="et", name="et")
                    nc.scalar.activation(out=e, in_=psc, func=AF.Exp, scale=ISQ)
                    et.append(e)
                pao = pa2.tile([HD + 1, S], FP32, tag="pao", name="pao")
                g, hh = h // 4, h % 4
                voff = 388 * g + 97 * hh
                for kt in range(4):
                    nc.tensor.matmul(pao, v97[:, 4 * s + kt, voff:voff + 97],
                                     et[kt], start=(kt == 0), stop=(kt == 3))
                if pending is not None:
                    tail(pending)
                pending = (s, h, pao)
        tail(pending)

    if _stop("attn"):
        fin()
        return

    # ---- P5: w_o + bias + residual -> embT ----
    embT = acts.tile([128, KH, T], BF16, tag="EMBT")
    with tc.tile_pool(name="pw", bufs=3, space="PSUM") as pw:
        for f in range(KH):
            for s in range(BL):
                ps = pw.tile([128, S], FP32, tag="pwo", name="pwo")
                for h in range(NH):
                    nc.tensor.matmul(ps, woT[:, h, 128 * f:128 * (f + 1)],
                                     aoT[:, s, h, :], start=(h == 0), stop=False)
                nc.tensor.matmul(ps,
                                 rowpack[:, OF_BO + 128 * f:OF_BO + 128 * (f + 1)],
                                 ones_row[:, 0:S], start=False, stop=False)
                nc.tensor.matmul(ps, ident, xT[:, f, S * s:S * (s + 1)],
                                 start=False, stop=True)
                if (2 * f + s) % 2 == 0:
                    nc.scalar.activation(out=embT[:, f, S * s:S * (s + 1)],
                                         in_=ps, func=AF.Copy)
                else:
                    nc.vector.tensor_copy(out=embT[:, f, S * s:S * (s + 1)],
                                          in_=ps)

    # late weight loads into freed slots (scalar DMA queue, off the hot path)
    w1T = acts.tile([128, KH, 1536], BF16, tag="WQKT")
    nc.scalar.dma_start(out=w1T[:, :, 0:D1],
                        in_=dd["w1t"].rearrange("(f p) c -> p f c", p=128))
    w2T = acts.tile([128, NH, H], BF16, tag="WOT")
    nc.scalar.dma_start(out=w2T[:, :, 0:D2],
                        in_=dd["w2t"].rearrange("(f p) c -> p f c", p=128))

    if _stop("wo"):
        fin()
        return

    # ---- P6: context loss (chunked) ----
    with tc.tile_pool(name="pctx", bufs=1, space="PSUM") as pcx, \
         tc.tile_pool(name="wctx", bufs=2) as wctx:
        chunks = [(0, 512), (512, T - 1)]
        for ci, (lo, hi) in enumerate(chunks):
            n = hi - lo
            pc = pcx.tile([1, 512], FP32, tag=f"pc{ci}", name="pc")
            for f in range(KH):
                d_ = wctx.tile([128, 512], BF16, tag="ctxd", name="ctxd")
                nc.vector.tensor_sub(out=d_[:, 0:n], in0=embT[:, f, lo:hi],
                                     in1=embT[:, f, lo + 1:hi + 1])
                dsq = wctx.tile([128, 512], BF16, tag="ctxsq", name="ctxsq")
                nc.scalar.activation(out=dsq[:, 0:n], in_=d_[:, 0:n],
                                     func=AF.Square)
                nc.tensor.matmul(pc[:, 0:n], onescol, dsq[:, 0:n],
                                 start=(f == 0), stop=(f == KH - 1))
            nrm = wctx.tile([1, 512], FP32, tag="nrm", name="nrm")
            nc.scalar.activation(out=nrm[:, 0:n], in_=pc[:, 0:n], func=AF.Sqrt)
            msk = wctx.tile([1, 512], FP32, tag="msk", name="msk")
            nc.vector.tensor_mul(out=msk[:, 0:n], in0=nrm[:, 0:n],
                                 in1=ctxmask[:, lo:hi])
            nc.vector.reduce_sum(out=partials[0:1, 1 + ci:2 + ci],
                                 in_=msk[:, 0:n], axis=AX.X)

    if _stop("ctx"):
        fin()
        return

    # ---- P8: MLP layers (feature-major LayerNorm) ----
    def mlp_layer(li, inT, kd, wT, kdo, b_off, g_off, be_off, wb_off, s1_i,
                  gelu, outT):
        dout = 128 * kdo
        inv_d = 1.0 / float(dout)
        with tc.tile_pool(name=f"pm{li}", bufs=2, space="PSUM") as pm, \
             tc.tile_pool(name=f"pp1_{li}", bufs=1, space="PSUM") as pp1, \
             tc.tile_pool(name=f"pp2_{li}", bufs=1, space="PSUM") as pp2, \
             tc.tile_pool(name=f"pb{li}", bufs=2, space="PSUM") as pb, \
             tc.tile_pool(name=f"wk{li}", bufs=1) as wk, \
             tc.tile_pool(name=f"ap{li}", bufs=2) as ap, \
             tc.tile_pool(name=f"sq{li}", bufs=3) as sqp, \
             tc.tile_pool(name=f"zb{li}", bufs=kdo + 1) as zbp:
            for c in range(2):
                cs = slice(512 * c, 512 * (c + 1))
                ps1 = pp1.tile([1, 512], FP32, tag="s1", name="s1")
                for f in range(kd):
                    nc.tensor.matmul(ps1, wbcol[:, wb_off + f:wb_off + f + 1],
                                     inT[:, f, cs], start=(f == 0), stop=False)
                nc.tensor.matmul(ps1, rowpack[:, OF_S1B + s1_i:OF_S1B + s1_i + 1],
                                 ones_row[:, 0:512], start=False, stop=True)
                ps2 = pp2.tile([1, 512], FP32, tag="s2", name="s2")
                zs, sqs = [], []
                for po in range(kdo):
                    ps = pm.tile([128, 512], FP32, tag="z", name="z")
                    for f in range(kd):
                        nc.tensor.matmul(ps, wT[:, f, 128 * po:128 * (po + 1)],
                                         inT[:, f, cs], start=(f == 0),
                                         stop=False)
                    nc.tensor.matmul(
                        ps, rowpack[:, b_off + 128 * po:b_off + 128 * (po + 1)],
                        ones_row[:, 0:512], start=False, stop=True)
                    zb = zbp.tile([128, 512], BF16, tag="zb", name="zb")
                    nc.vector.tensor_copy(out=zb, in_=ps)
                    sq = sqp.tile([128, 512], BF16, tag="sq", name="sq")
                    nc.scalar.activation(out=sq, in_=ps, func=AF.Square)
                    zs.append(zb)
                    sqs.append(sq)
                    if po >= 1:
                        nc.tensor.matmul(ps2, onescol, sqs[po - 1],
                                         start=(po == 1), stop=False)
                nc.tensor.matmul(ps2, onescol, sqs[kdo - 1],
                                 start=(kdo == 1), stop=True)
                mu = wk.tile([1, 512], FP32, tag="mu", name="mu")
                nc.vector.tensor_scalar(out=mu, in0=ps1, scalar1=inv_d,
                                        scalar2=None, op0=ALU.mult)
                m2 = wk.tile([1, 512], FP32, tag="m2", name="m2")
                nc.vector.tensor_scalar(out=m2, in0=ps2, scalar1=inv_d,
                                        scalar2=None, op0=ALU.mult)
                musq = wk.tile([1, 512], FP32, tag="musq", name="musq")
                nc.vector.tensor_mul(out=musq, in0=mu, in1=mu)
                var = wk.tile([1, 512], FP32, tag="var", name="var")
                nc.vector.tensor_sub(out=var, in0=m2, in1=musq)
                sd = wk.tile([1, 512], FP32, tag="sd", name="sd")
                nc.scalar.activation(out=sd, in_=var, func=AF.Sqrt, bias=eps_t)
                r_bf = wk.tile([1, 512], BF16, tag="rbf", name="rbf")
                nc.vector.reciprocal(out=r_bf, in_=sd)
                mur = wk.tile([1, 512], BF16, tag="mur", name="mur")
                nc.vector.tensor_mul(out=mur, in0=mu, in1=r_bf)
                for po in range(kdo):
                    gsl = rowpack[:, g_off + 128 * po:g_off + 128 * (po + 1)]
                    pRg = pb.tile([128, 512], FP32, tag="rg", name="rg")
                    nc.tensor.matmul(pRg, gsl, r_bf, start=True, stop=True)
                    pMg = pb.tile([128, 512], FP32, tag="mg", name="mg")
                    nc.tensor.matmul(pMg, gsl, mur, start=True, stop=True)
                    t1 = ap.tile([128, 512], BF16, tag="t1", name="t1")
                    nc.vector.tensor_mul(out=t1, in0=zs[po], in1=pRg)
                    be_sl = ppack[:, be_off + po:be_off + po + 1]
                    if gelu:
                        t2 = ap.tile([128, 512], BF16, tag="t2", name="t2")
                        nc.vector.scalar_tensor_tensor(
                            out=t2, in0=t1, scalar=be_sl, in1=pMg,
                            op0=ALU.add, op1=ALU.subtract)
                        nc.scalar.activation(
                            out=outT[:, po, cs], in_=t2,
                            func=(AF.Copy if GELU_AS_COPY else AF.Gelu))
                    else:
                        nc.vector.scalar_tensor_tensor(
                            out=outT[:, po, cs], in0=t1, scalar=be_sl, in1=pMg,
                            op0=ALU.add, op1=ALU.subtract)

    h1T = acts.tile([128, KD1, T], BF16, tag="QT")
    mlp_layer(0, embT, KH, w1T, KD1, OF_B1, OF_G1, PP_BE1, WB1, 0, True, h1T)
    if _stop("mlp1"):
        fin()
        return
    h2T = acts.tile([128, KD1, T], BF16, tag="KT")   # planes 0:KD2 used
    mlp_layer(1, h1T, KD1, w2T, KD2, OF_B2, OF_G2, PP_BE2, WB2, 1, True, h2T)
    if _stop("mlp2"):
        fin()
        return
    featT = acts.tile([128, KH, T], BF16, tag="XT")  # planes 0:KD3 used
    mlp_layer(2, h2T, KD2, w3T, KD3, OF_B3, OF_G3, PP_BE3, WB3, 2, False, featT)

    if _stop("mlp3"):
        fin()
        return

    # ---- P9: CE in [9, T] layout ----
    with tc.tile_pool(name="pce", bufs=2, space="PSUM") as pce, \
         tc.tile_pool(name="pse", bufs=2, space="PSUM") as pse, \
         tc.tile_pool(name="ppk", bufs=2, space="PSUM") as ppk, \
         tc.tile_pool(name="wce", bufs=2) as wce:
        for c in range(2):
            cs = slice(512 * c, 512 * (c + 1))
            pl = pce.tile([NUM_LABELS, 512], FP32, tag="pl", name="pl")
            for d3 in range(KD3):
                nc.tensor.matmul(pl, wcT[:, d3, :], featT[:, d3, cs],
                                 start=(d3 == 0), stop=False)
            for f in range(KH):
                nc.tensor.matmul(pl, wrT[:, f, :], embT[:, f, cs],
                                 start=False, stop=False)
            nc.tensor.matmul(pl, rowpack[:, OF_BCLS:OF_BCLS + NUM_LABELS],
                             ones_row[:, 0:512], start=False, stop=True)
            e9 = wce.tile([NUM_LABELS, 512], BF16, tag="e9", name="e9")
            nc.scalar.activation(out=e9, in_=pl, func=AF.Exp)
            ps_se = pse.tile([1, 512], FP32, tag="se", name="se")
            nc.tensor.matmul(ps_se, ones9, e9, start=True, stop=True)
            lns = wce.tile([1, 512], FP32, tag="lns", name="lns")
            nc.scalar.activation(out=lns, in_=ps_se, func=AF.Ln)
            nc.vector.reduce_sum(out=partials[0:1, 3 + c:4 + c], in_=lns,
                                 axis=AX.X)
            po9 = wce.tile([NUM_LABELS, 512], FP32, tag="po9", name="po9")
            nc.vector.tensor_mul(out=po9, in0=onehot[:, cs], in1=pl)
            pr9 = wce.tile([NUM_LABELS, 1], BF16, tag="pr9", name="pr9")
            nc.vector.reduce_sum(out=pr9, in_=po9, axis=AX.X)
            ps_pk = ppk.tile([1, 1], FP32, tag="pk", name="pk")
            nc.tensor.matmul(ps_pk, pr9, ones9, start=True, stop=True)
            nc.vector.tensor_copy(out=partials[0:1, 5 + c:6 + c], in_=ps_pk)

    if _stop("ce"):
        fin()
        return

    # ---- P7: quadruplet loss (emb -> token-major, host-built selectors) ----
    emb_tok = acts.tile([128, NT, H], BF16, tag="XTOK")
    with tc.tile_pool(name="ptp2", bufs=4, space="PSUM") as ptp2:
        i = 0
        for t in range(NT):
            for f in range(KH):
                ps = ptp2.tile([128, 128], BF16, tag="tr2", name="tr2")
                nc.tensor.transpose(ps, embT[:, f, 128 * t:128 * (t + 1)],
                                    ident)
                if i % 2 == 0:
                    nc.scalar.activation(out=emb_tok[:, t, 128 * f:128 * (f + 1)],
                                         in_=ps, func=AF.Copy)
                else:
                    nc.vector.tensor_copy(out=emb_tok[:, t, 128 * f:128 * (f + 1)],
                                          in_=ps)
                i += 1
    with tc.tile_pool(name="pqd", bufs=1, space="PSUM") as pqd_pool, \
         tc.tile_pool(name="wqd", bufs=1) as wqd:
        pq1 = pqd_pool.tile([6, 512], FP32, tag="pq1", name="pq1")
        pq2 = pqd_pool.tile([6, H - 512], FP32, tag="pq2", name="pq2")
        for t in range(NT):
            nc.tensor.matmul(pq1, sel[:, t, :], emb_tok[:, t, 0:512],
                             start=(t == 0), stop=(t == NT - 1))
        for t in range(NT):
            nc.tensor.matmul(pq2, sel[:, t, :], emb_tok[:, t, 512:H],
                             start=(t == 0), stop=(t == NT - 1))
        dq1 = wqd.tile([6, 512], FP32, tag="dq1", name="dq1")
        nc.scalar.activation(out=dq1, in_=pq1, func=AF.Square)
        dq2 = wqd.tile([6, H - 512], FP32, tag="dq2", name="dq2")
        nc.scalar.activation(out=dq2, in_=pq2, func=AF.Square)
        d1c = wqd.tile([6, 1], FP32, tag="d1c", name="d1c")
        nc.vector.reduce_sum(out=d1c, in_=dq1, axis=AX.X)
        d2c = wqd.tile([6, 1], FP32, tag="d2c", name="d2c")
        nc.vector.reduce_sum(out=d2c, in_=dq2, axis=AX.X)
        dist = wqd.tile([6, 1], BF16, tag="dist", name="dist")
        nc.vector.tensor_add(out=dist, in0=d1c, in1=d2c)
        pqd = pqd_pool.tile([4, 1], FP32, tag="pqd", name="pqd")
        nc.tensor.matmul(pqd, combo, dist, start=True, stop=True)
        qrelu = wqd.tile([4, 1], BF16, tag="qrelu", name="qrelu")
        nc.scalar.activation(out=qrelu, in_=pqd, func=AF.Relu,
                             bias=ppack[0:4, PP_MARG:PP_MARG + 1])
        psq = pqd_pool.tile([1, 1], FP32, tag="psq", name="psq")
        nc.tensor.matmul(psq, qrelu, onescol[0:4, :], start=True, stop=True)
        nc.vector.tensor_copy(out=partials[0:1, 0:1], in_=psq)

    fin()


def _get_nc():
    global _CACHED
    if _CACHED is None:
        _CACHED = _build()
    return _CACHED


def _shard(inputs):
    f32 = lambda a: np.asarray(a, np.float32)
    bf = lambda a: np.ascontiguousarray(np.asarray(a, np.float32)).astype(BFD)

    seq = f32(inputs["sequence_output"])
    labels = np.asarray(inputs["labels"]).astype(np.int64)
    a_p = np.asarray(inputs["anchor_positions"]).astype(np.int64)
    p_p = np.asarray(inputs["positive_positions"]).astype(np.int64)
    n1_p = np.asarray(inputs["negative1_positions"]).astype(np.int64)
    n2_p = np.asarray(inputs["negative2_positions"]).astype(np.int64)

    w_qkv, b_qkv = f32(inputs["w_qkv"]), f32(inputs["b_qkv"])
    w_o, b_o = f32(inputs["w_o"]), f32(inputs["b_o"])
    w1, b1 = f32(inputs["w1"]), f32(inputs["b1"])
    g1, be1 = f32(inputs["g1"]), f32(inputs["be1"])
    w2, b2 = f32(inputs["w2"]), f32(inputs["b2"])
    g2, be2 = f32(inputs["g2"]), f32(inputs["be2"])
    w3, b3 = f32(inputs["w3"]), f32(inputs["b3"])
    g3, be3 = f32(inputs["g3"]), f32(inputs["be3"])
    wc, bc = f32(inputs["wc"]), f32(inputs["bc"])
    wr, br = f32(inputs["wr"]), f32(inputs["br"])

    wq, wk, wv = w_qkv[0:H], w_qkv[H:2 * H], w_qkv[2 * H:3 * H]
    bq, bk, bv = b_qkv[0:H], b_qkv[H:2 * H], b_qkv[2 * H:3 * H]

    wqk = np.concatenate([wq.T, wk.T], axis=1)               # [H, 2H]
    wv97 = np.zeros((H, NH, 97), np.float32)
    wv97[:, :, :96] = wv.T.reshape(H, NH, HD)
    wv97 = wv97.reshape(H, 776)
    wopk = np.ascontiguousarray(
        w_o.T.reshape(NH, HD, H).transpose(1, 0, 2)).reshape(HD * NH, H)

    rowpack = np.zeros((1, ROWN), np.float32)
    rowpack[0, OF_BQK:OF_BQK + H] = bq
    rowpack[0, OF_BQK + H:OF_BQK + 2 * H] = bk
    bv97 = np.zeros((NH, 97), np.float32)
    bv97[:, :96] = bv.reshape(NH, HD)
    bv97[:, 96] = 1.0
    rowpack[0, OF_BV:OF_BV + 776] = bv97.reshape(776)
    rowpack[0, OF_BO:OF_BO + H] = b_o
    rowpack[0, OF_B1:OF_B1 + D1] = b1
    rowpack[0, OF_B2:OF_B2 + D2] = b2
    rowpack[0, OF_B3:OF_B3 + D3] = b3
    rowpack[0, OF_BCLS:OF_BCLS + NUM_LABELS] = bc + ALPHA * br
    rowpack[0, OF_S1B + 0] = b1.sum()
    rowpack[0, OF_S1B + 1] = b2.sum()
    rowpack[0, OF_S1B + 2] = b3.sum()
    rowpack[0, OF_G1:OF_G1 + D1] = g1
    rowpack[0, OF_G2:OF_G2 + D2] = g2
    rowpack[0, OF_G3:OF_G3 + D3] = g3

    ppack = np.zeros((128, PPC), np.float32)
    ppack[:, PP_BE1:PP_BE1 + KD1] = be1.reshape(KD1, 128).T
    ppack[:, PP_BE2:PP_BE2 + KD2] = be2.reshape(KD2, 128).T
    ppack[:, PP_BE3:PP_BE3 + KD3] = be3.reshape(KD3, 128).T
    ppack[0:4, PP_MARG] = [MARGIN1, MARGIN2, MARGIN1, MARGIN2]

    wbcol = np.zeros((128, WBC), np.float32)
    wbcol[:, WB1:WB1 + KH] = w1.sum(axis=0).reshape(KH, 128).T
    wbcol[:, WB2:WB2 + KD1] = w2.sum(axis=0).reshape(KD1, 128).T
    wbcol[:, WB3:WB3 + KD2] = w3.sum(axis=0).reshape(KD2, 128).T

    combo = np.zeros((6, 4), np.float32)
    for k, (ipd, ind) in enumerate([(0, 1), (0, 2), (3, 4), (3, 5)]):
        combo[ipd, k] = 1.0
        combo[ind, k] = -1.0

    base = {
        "wqk": bf(wqk), "wv97": bf(wv97), "wopk": bf(wopk),
        "w1t": bf(w1.T), "w2t": bf(w2.T), "w3t": bf(w3.T),
        "wct": bf(wc.T), "wrt": bf(ALPHA * wr.T),
        "rowpack": bf(rowpack), "ppack": np.ascontiguousarray(ppack),
        "wbcol": bf(wbcol), "combo": bf(combo),
    }

    in_maps = []
    for c in range(NCORES):
        sl = slice(BL * c, BL * (c + 1))
        lab = labels[sl]                       # [BL, S]
        oh = np.zeros((NUM_LABELS, T), np.float32)
        flat = lab.reshape(T)
        oh[flat, np.arange(T)] = 1.0
        m = (lab[:, :-1] != 0) & (lab[:, :-1] == lab[:, 1:])   # [BL, S-1]
        cm = np.zeros((1, T - 1), np.float32)
        for s in range(BL):
            cm[0, S * s:S * s + S - 1] = m[s]
        selm = np.zeros((T, 6), np.float32)
        for s in range(BL):
            b = BL * c + s
            av = int(a_p[b]) + S * s
            for j, pos in enumerate((p_p[b], n1_p[b], n2_p[b])):
                col = 3 * s + j
                selm[av, col] += 1.0
                selm[int(pos) + S * s, col] -= 1.0
        mm = dict(base)
        mm["x"] = bf(seq[sl].reshape(T, H))
        mm["onehot"] = bf(oh)
        mm["ctxmask"] = bf(cm)
        mm["sel"] = bf(selm)
        in_maps.append(mm)
    return in_maps


def kernel(**inputs):
    nc = _get_nc()
    in_maps = _shard(inputs)
    res = run_bass_kernel_spmd(nc, in_maps, core_ids=list(range(NCORES)))
    ce = quad = ctx = 0.0
    for c in range(NCORES):
        o = np.asarray(res.results[c]["out"], np.float64).reshape(8)
        quad += float(o[0])
        ctx += float(o[1] + o[2])
        ce += float(o[3] + o[4] - o[5] - o[6])
    total = ce / (B * S) + ALPHA * (quad / B) + BETA * (ctx / (B * S))
    return np.float32(total)


# revision 3
# speedup vs baseline: 1.0363x; 1.0363x over previous
"""Trainium2 Bass kernel for nn_LLMCC_74414603370526 (loss_fn) — v2.

Data-parallel over batch: 16 sequences -> 8 cores x 2 sequences each; host
combines the scalar partial losses (the sanctioned all-reduce).

v2 design (instruction-count-minimized vs v1):
  - activations flow feature-major end-to-end; the only PE transposes are
    x (48) and the emb token-major copy needed by the quadruplet gather (48)
  - weights are pre-transposed on the HOST into DMA-friendly layouts; no
    on-device weight transposes
  - biases/residual/softmax-rowsum fold into matmuls (K=1 outer products,
    identity lhsT accumulate, ones column appended to v)
  - LayerNorm runs feature-major: per-token stats via ones/w-bar matmul
    column reductions; scale/shift applied via PE rank-1 outer products
    (g x rstd) and two vector ops per tile
  - CE in [9, T] label-major layout with host-built one-hot labels;
    context-loss label mask is host-built
  - SBUF slots reused via byte-equal tag chains:
    XTOK: x_tok -> emb_tok | XT: xT -> featT | QT: qT -> h1T
    KT: kT -> h2T | WQKT: wqkT -> w1T | WOT: woT -> w2T
"""

import numpy as np
import ml_dtypes

import concourse.bass as bass
import concourse.mybir as mybir
import concourse.tile as tile
from concourse import bacc
from concourse.bass_utils import run_bass_kernel_spmd
from concourse.masks import make_identity

FP32 = mybir.dt.float32
BF16 = mybir.dt.bfloat16
AF = mybir.ActivationFunctionType
ALU = mybir.AluOpType
AX = mybir.AxisListType

B, S, H = 16, 512, 768
NH, HD = 8, 96
NUM_LABELS = 9
MARGIN1, MARGIN2 = 1.0, 0.5
ALPHA, BETA = 0.2, 0.1
EPS = 1e-5

NCORES = 8
BL = B // NCORES          # 2 sequences per core
T = BL * S                # 1024 tokens per core
NT = T // 128             # 8 token tiles
KH = H // 128             # 6 feature tiles
D1, D2, D3 = 1024, 512, 256
KD1, KD2, KD3 = D1 // 128, D2 // 128, D3 // 128
ISQ = 1.0 / float(np.sqrt(HD))
BFD = ml_dtypes.bfloat16

# rowpack offsets (bf16 row: biases / row-vectors)
OF_BQK = 0                       # 1536
OF_BV = OF_BQK + 2 * H           # 776
OF_BO = OF_BV + 776              # 768
OF_B1 = OF_BO + H                # 1024
OF_B2 = OF_B1 + D1               # 512
OF_B3 = OF_B2 + D2               # 256
OF_BCLS = OF_B3 + D3             # 9
OF_S1B = OF_BCLS + NUM_LABELS    # 3
OF_G1 = OF_S1B + 3               # 1024
OF_G2 = OF_G1 + D1               # 512
OF_G3 = OF_G2 + D2               # 256
ROWN = OF_G3 + D3

# ppack (fp32 per-partition pack)
PP_BE1 = 0          # 8 cols
PP_BE2 = 8          # 4
PP_BE3 = 12         # 2
PP_MARG = 14        # margins in rows 0:4 of col 14
PPC = 15

# wbcol (bf16 per-partition colsum-weight pack)
WB1, WB2, WB3 = 0, 6, 14
WBC = 18

GELU_AS_COPY = False   # sim-debug only (CoreSim executor lacks Gelu numerics)

_CACHED = None


def _build(stop_after=None):
    nc = bacc.Bacc(None, target_bir_lowering=False)
    dd = {}

    def di(name, shape, dt=BF16):
        dd[name] = nc.dram_tensor(name, shape, dt, kind="ExternalInput")

    di("x", [T, H])
    di("wqk", [H, 2 * H])
    di("wv97", [H, 776])
    di("wopk", [HD * NH, H])       # rows = (d-within-head, head), cols f
    di("w1t", [H, D1])
    di("w2t", [D1, D2])
    di("w3t", [D2, D3])
    di("wct", [D3, NUM_LABELS])
    di("wrt", [H, NUM_LABELS])     # pre-scaled by 0.2 on host
    di("rowpack", [1, ROWN])
    di("ppack", [128, PPC], FP32)
    di("wbcol", [128, WBC])
    di("onehot", [NUM_LABELS, T])
    di("ctxmask", [1, T - 1])
    di("sel", [T, 6])
    di("combo", [6, 4], FP32)
    out_d = nc.dram_tensor("out", [1, 8], FP32, kind="ExternalOutput")

    with tile.TileContext(nc) as tc:
        with nc.allow_low_precision(reason="bf16 PE-transpose PSUM tiles"):
            _body(nc, tc, dd, out_d, stop_after)
    nc.finalize()
    return nc


def _body(nc, tc, dd, out_d, stop_after=None):
    const = tc.alloc_tile_pool(name="const", bufs=1)
    acts = tc.alloc_tile_pool(name="acts", bufs=1)

    def _stop(phase):
        return stop_after == phase

    def fin():
        nc.sync.dma_start(out=out_d[:, :], in_=partials)
        acts.release()
        const.release()


    # ---- constants / inputs ----
    ident = const.tile([128, 128], BF16)
    make_identity(nc, ident)
    ones_row = const.tile([1, 1024], BF16)
    nc.vector.memset(ones_row, 1.0)
    onescol = const.tile([128, 1], BF16)
    nc.vector.memset(onescol, 1.0)
    ones9 = const.tile([NUM_LABELS, 1], BF16)
    nc.vector.memset(ones9, 1.0)
    ones4f = const.tile([4, 1], FP32)
    nc.vector.memset(ones4f, 1.0)
    eps_t = const.tile([1, 1], FP32)
    nc.vector.memset(eps_t, EPS)
    partials = const.tile([1, 8], FP32)
    nc.vector.memset(partials, 0.0)

    x_tok = acts.tile([128, NT, H], BF16, tag="XTOK")
    nc.sync.dma_start(out=x_tok, in_=dd["x"].rearrange("(n p) h -> p n h", p=128))
    wqkT = acts.tile([128, KH, 1536], BF16, tag="WQKT")
    nc.sync.dma_start(out=wqkT, in_=dd["wqk"].rearrange("(f p) c -> p f c", p=128))
    rowpack = const.tile([1, ROWN], BF16)
    nc.sync.dma_start(out=rowpack, in_=dd["rowpack"][:, :])
    wv97T = const.tile([128, KH, 776], BF16)
    nc.gpsimd.dma_start(out=wv97T, in_=dd["wv97"].rearrange("(f p) c -> p f c", p=128))
    woT = acts.tile([HD, NH, H], BF16, tag="WOT")
    nc.gpsimd.dma_start(out=woT, in_=dd["wopk"].rearrange("(p h) f -> p h f", p=HD))
    w3T = const.tile([128, KD2, D3], BF16)
    nc.gpsimd.dma_start(out=w3T, in_=dd["w3t"].rearrange("(f p) c -> p f c", p=128))
    wcT = const.tile([128, KD3, NUM_LABELS], BF16)
    nc.gpsimd.dma_start(out=wcT, in_=dd["wct"].rearrange("(f p) c -> p f c", p=128))
    wrT = const.tile([128, KH, NUM_LABELS], BF16)
    nc.gpsimd.dma_start(out=wrT, in_=dd["wrt"].rearrange("(f p) c -> p f c", p=128))
    ppack = const.tile([128, PPC], FP32)
    nc.gpsimd.dma_start(out=ppack, in_=dd["ppack"][:, :])
    wbcol = const.tile([128, WBC], BF16)
    nc.gpsimd.dma_start(out=wbcol, in_=dd["wbcol"][:, :])
    onehot = const.tile([NUM_LABELS, T], BF16)
    nc.gpsimd.dma_start(out=onehot, in_=dd["onehot"][:, :])
    ctxmask = const.tile([1, T - 1], BF16)
    nc.gpsimd.dma_start(out=ctxmask, in_=dd["ctxmask"][:, :])
    sel = const.tile([128, NT, 6], BF16)
    nc.gpsimd.dma_start(out=sel, in_=dd["sel"].rearrange("(n p) c -> p n c", p=128))
    combo = const.tile([6, 4], FP32)
    nc.gpsimd.dma_start(out=combo, in_=dd["combo"][:, :])

    if _stop("load"):
        fin()
        return

    # ---- P1: x -> xT (feature-major) ----
    xT = acts.tile([128, KH, T], BF16, tag="XT")
    with tc.tile_pool(name="ptp", bufs=6, space="PSUM") as ptp:
        i = 0
        for t in range(NT):
            for f in range(KH):
                ps = ptp.tile([128, 128], BF16, tag="tr", name="tr")
                nc.tensor.transpose(ps, x_tok[:, t, 128 * f:128 * (f + 1)], ident)
                if i % 2 == 0:
                    nc.scalar.activation(out=xT[:, f, 128 * t:128 * (t + 1)],
                                         in_=ps, func=AF.Copy)
                else:
                    nc.vector.tensor_copy(out=xT[:, f, 128 * t:128 * (t + 1)],
                                          in_=ps)
                i += 1

    if _stop("p1"):
        fin()
        return

    # ---- P2+P3: q, k, v ----
    qT = acts.tile([HD, NH, T], BF16, tag="QT")
    kT = acts.tile([HD, NH, T], BF16, tag="KT")
    v97 = acts.tile([128, NT, 776], BF16, tag="V97")
    with tc.tile_pool(name="pq", bufs=4, space="PSUM") as pq, \
         tc.tile_pool(name="pv", bufs=2, space="PSUM") as pv:
        i = 0
        for h in range(NH):
            for w in range(2):      # 0=q, 1=k
                dst = qT if w == 0 else kT
                off = H * w + HD * h
                for c in range(2):
                    ps = pq.tile([HD, 512], FP32, tag="pqk", name="pqk")
                    for f in range(KH):
                        nc.tensor.matmul(ps, wqkT[:, f, off:off + HD],
                                         xT[:, f, 512 * c:512 * (c + 1)],
                                         start=(f == 0), stop=False)
                    nc.tensor.matmul(ps,
                                     rowpack[:, OF_BQK + off:OF_BQK + off + HD],
                                     ones_row[:, 0:512], start=False, stop=True)
                    if i % 2 == 0:
                        nc.scalar.activation(out=dst[:, h, 512 * c:512 * (c + 1)],
                                             in_=ps, func=AF.Copy)
                    else:
                        nc.vector.tensor_copy(out=dst[:, h, 512 * c:512 * (c + 1)],
                                              in_=ps)
                    i += 1
        for t in range(NT):
            for g in range(2):
                ps = pv.tile([128, 388], FP32, tag="pv", name="pv")
                for f in range(KH):
                    nc.tensor.matmul(ps, xT[:, f, 128 * t:128 * (t + 1)],
                                     wv97T[:, f, 388 * g:388 * (g + 1)],
                                     start=(f == 0), stop=False)
                nc.tensor.matmul(ps, ones_row[0:1, 0:128],
                                 rowpack[:, OF_BV + 388 * g:OF_BV + 388 * (g + 1)],
                                 start=False, stop=True)
                if t % 2 == 0:
                    nc.scalar.activation(out=v97[:, t, 388 * g:388 * (g + 1)],
                                         in_=ps, func=AF.Copy)
                else:
                    nc.vector.tensor_copy(out=v97[:, t, 388 * g:388 * (g + 1)],
                                          in_=ps)

    if _stop("qkv"):
        fin()
        return

    # ---- P4: attention (exp without max-shift; rowsum via v ones column) ----
    aoT = acts.tile([HD, BL, NH, S], BF16, tag="AOT")
    with tc.tile_pool(name="pa", bufs=4, space="PSUM") as pa, \
         tc.tile_pool(name="pa2", bufs=2, space="PSUM") as pa2, \
         tc.tile_pool(name="pa3", bufs=2, space="PSUM") as pa3, \
         tc.tile_pool(name="wet", bufs=6) as wet, \
         tc.tile_pool(name="wra", bufs=2) as wra:
        def tail(sh):
            s, h, pao = sh
            rec = wra.tile([1, S], BF16, tag="rec", name="rec")
            nc.vector.reciprocal(out=rec, in_=pao[HD:HD + 1, :])
            prec = pa3.tile([HD, S], FP32, tag="prec", name="prec")
            nc.tensor.matmul(prec, ones_row[0:1, 0:HD], rec, start=True,
                             stop=True)
            aou = wra.tile([HD, S], BF16, tag="aou", name="aou")
            nc.scalar.activation(out=aou, in_=pao[0:HD, :], func=AF.Copy)
            nc.vector.tensor_mul(out=aoT[:, s, h, :], in0=aou, in1=prec)

        pending = None
        for s in range(BL):
            for h in range(NH):
                et = []
                for kt in range(4):
                    psc = pa.tile([128, S], FP32, tag="psc", name="psc")
                    nc.tensor.matmul(
                        psc, kT[:, h, S * s + 128 * kt:S * s + 128 * (kt + 1)],
                        qT[:, h, S * s:S * (s + 1)], start=True, stop=True)
                    e = wet.tile([128, S], BF16, tag="et", name="et")
                    nc.scalar.activation(out=e, in_=psc, func=AF.Exp, scale=ISQ)
                    et.append(e)
                pao = pa2.tile([HD + 1, S], FP32, tag="pao", name="pao")
                g, hh = h // 4, h % 4
                voff = 388 * g + 97 * hh
                for kt in range(4):
                    nc.tensor.matmul(pao, v97[:, 4 * s + kt, voff:voff + 97],
                                     et[kt], start=(kt == 0), stop=(kt == 3))
                if pending is not None:
                    tail(pending)
                pending = (s, h, pao)
        tail(pending)

    if _stop("attn"):
        fin()
        return

    # ---- P5: w_o + bias + residual -> embT ----
    embT = acts.tile([128, KH, T], BF16, tag="EMBT")
    with tc.tile_pool(name="pw", bufs=3, space="PSUM") as pw:
        for f in range(KH):
            for s in range(BL):
                ps = pw.tile([128, S], FP32, tag="pwo", name="pwo")
                for h in range(NH):
                    nc.tensor.matmul(ps, woT[:, h, 128 * f:128 * (f + 1)],
                                     aoT[:, s, h, :], start=(h == 0), stop=False)
                nc.tensor.matmul(ps,
                                 rowpack[:, OF_BO + 128 * f:OF_BO + 128 * (f + 1)],
                                 ones_row[:, 0:S], start=False, stop=False)
                nc.tensor.matmul(ps, ident, xT[:, f, S * s:S * (s + 1)],
                                 start=False, stop=True)
                if (2 * f + s) % 2 == 0:
                    nc.scalar.activation(out=embT[:, f, S * s:S * (s + 1)],
                                         in_=ps, func=AF.Copy)
                else:
                    nc.vector.tensor_copy(out=embT[:, f, S * s:S * (s + 1)],
                                          in_=ps)

    # late weight loads into freed slots (scalar DMA queue, off the hot path)
    w1T = acts.tile([128, KH, 1536], BF16, tag="WQKT")
    nc.scalar.dma_start(out=w1T[:, :, 0:D1],
                        in_=dd["w1t"].rearrange("(f p) c -> p f c", p=128))
    w2T = acts.tile([128, NH, H], BF16, tag="WOT")
    nc.scalar.dma_start(out=w2T[:, :, 0:D2],
                        in_=dd["w2t"].rearrange("(f p) c -> p f c", p=128))

    if _stop("wo"):
        fin()
        return

    # ---- P6: context loss (chunked) ----
    with tc.tile_pool(name="pctx", bufs=1, space="PSUM") as pcx, \
         tc.tile_pool(name="wctx", bufs=2) as wctx:
        chunks = [(0, 512), (512, T - 1)]
        for ci, (lo, hi) in enumerate(chunks):
            n = hi - lo
            pc = pcx.tile([1, 512], FP32, tag=f"pc{ci}", name="pc")
            for f in range(KH):
                d_ = wctx.tile([128, 512], BF16, tag="ctxd", name="ctxd")
                nc.vector.tensor_sub(out=d_[:, 0:n], in0=embT[:, f, lo:hi],
                                     in1=embT[:, f, lo + 1:hi + 1])
                dsq = wctx.tile([128, 512], BF16, tag="ctxsq", name="ctxsq")
                nc.scalar.activation(out=dsq[:, 0:n], in_=d_[:, 0:n],
                                     func=AF.Square)
                nc.tensor.matmul(pc[:, 0:n], onescol, dsq[:, 0:n],
                                 start=(f == 0), stop=(f == KH - 1))
            nrm = wctx.tile([1, 512], FP32, tag="nrm", name="nrm")
            nc.scalar.activation(out=nrm[:, 0:n], in_=pc[:, 0:n], func=AF.Sqrt)
            msk = wctx.tile([1, 512], FP32, tag="msk", name="msk")
            nc.vector.tensor_mul(out=msk[:, 0:n], in0=nrm[:, 0:n],
                                 in1=ctxmask[:, lo:hi])
            nc.vector.reduce_sum(out=partials[0:1, 1 + ci:2 + ci],
                                 in_=msk[:, 0:n], axis=AX.X)

    if _stop("ctx"):
        fin()
        return

    # ---- P8: MLP layers (feature-major LayerNorm) ----
    def mlp_layer(li, inT, kd, wT, kdo, b_off, g_off, be_off, wb_off, s1_i,
                  gelu, outT):
        dout = 128 * kdo
        inv_d = 1.0 / float(dout)
        with tc.tile_pool(name=f"pm{li}", bufs=2, space="PSUM") as pm, \
             tc.tile_pool(name=f"pp1_{li}", bufs=1, space="PSUM") as pp1, \
             tc.tile_pool(name=f"pp2_{li}", bufs=1, space="PSUM") as pp2, \
             tc.tile_pool(name=f"pb{li}", bufs=2, space="PSUM") as pb, \
             tc.tile_pool(name=f"wk{li}", bufs=1) as wk, \
             tc.tile_pool(name=f"ap{li}", bufs=2) as ap, \
             tc.tile_pool(name=f"sq{li}", bufs=3) as sqp, \
             tc.tile_pool(name=f"zb{li}", bufs=kdo + 1) as zbp:
            for c in range(2):
                cs = slice(512 * c, 512 * (c + 1))
                ps1 = pp1.tile([1, 512], FP32, tag="s1", name="s1")
                for f in range(kd):
                    nc.tensor.matmul(ps1, wbcol[:, wb_off + f:wb_off + f + 1],
                                     inT[:, f, cs], start=(f == 0), stop=False)
                nc.tensor.matmul(ps1, rowpack[:, OF_S1B + s1_i:OF_S1B + s1_i + 1],
                                 ones_row[:, 0:512], start=False, stop=True)
                ps2 = pp2.tile([1, 512], FP32, tag="s2", name="s2")
                zs, sqs = [], []
                for po in range(kdo):
                    ps = pm.tile([128, 512], FP32, tag="z", name="z")
                    for f in range(kd):
                        nc.tensor.matmul(ps, wT[:, f, 128 * po:128 * (po + 1)],
                                         inT[:, f, cs], start=(f == 0),
                                         stop=False)
                    nc.tensor.matmul(
                        ps, rowpack[:, b_off + 128 * po:b_off + 128 * (po + 1)],
                        ones_row[:, 0:512], start=False, stop=True)
                    zb = zbp.tile([128, 512], BF16, tag="zb", name="zb")
                    nc.vector.tensor_copy(out=zb, in_=ps)
                    sq = sqp.tile([128, 512], BF16, tag="sq", name="sq")
                    nc.scalar.activation(out=sq, in_=ps, func=AF.Square)
                    zs.append(zb)
                    sqs.append(sq)
                    if po >= 1:
                        nc.tensor.matmul(ps2, onescol, sqs[po - 1],
                                         start=(po == 1), stop=False)
                nc.tensor.matmul(ps2, onescol, sqs[kdo - 1],
                                 start=(kdo == 1), stop=True)
                mu = wk.tile([1, 512], FP32, tag="mu", name="mu")
                nc.vector.tensor_scalar(out=mu, in0=ps1, scalar1=inv_d,
                                        scalar2=None, op0=ALU.mult)
                m2 = wk.tile([1, 512], FP32, tag="m2", name="m2")
                nc.vector.tensor_scalar(out=m2, in0=ps2, scalar1=inv_d,
                                        scalar2=None, op0=ALU.mult)
                musq = wk.tile([1, 512], FP32, tag="musq", name="musq")
                nc.vector.tensor_mul(out=musq, in0=mu, in1=mu)
                var = wk.tile([1, 512], FP32, tag="var", name="var")
                nc.vector.tensor_sub(out=var, in0=m2, in1=musq)
                sd = wk.tile([1, 512], FP32, tag="sd", name="sd")
                nc.scalar.activation(out=sd, in_=var, func=AF.Sqrt, bias=eps_t)
                r_bf = wk.tile([1, 512], BF16, tag="rbf", name="rbf")
                nc.vector.reciprocal(out=r_bf, in_=sd)
                mur = wk.tile([1, 512], BF16, tag="mur", name="mur")
                nc.vector.tensor_mul(out=mur, in0=mu, in1=r_bf)
                for po in range(kdo):
                    gsl = rowpack[:, g_off + 128 * po:g_off + 128 * (po + 1)]
                    pRg = pb.tile([128, 512], FP32, tag="rg", name="rg")
                    nc.tensor.matmul(pRg, gsl, r_bf, start=True, stop=True)
                    pMg = pb.tile([128, 512], FP32, tag="mg", name="mg")
                    nc.tensor.matmul(pMg, gsl, mur, start=True, stop=True)
                    t1 = ap.tile([128, 512], BF16, tag="t1", name="t1")
                    nc.vector.tensor_mul(out=t1, in0=zs[po], in1=pRg)
                    be_sl = ppack[:, be_off + po:be_off + po + 1]
                    if gelu:
                        t2 = ap.tile([128, 512], BF16, tag="t2", name="t2")
                        nc.vector.scalar_tensor_tensor(
                            out=t2, in0=t1, scalar=be_sl, in1=pMg,
                            op0=ALU.add, op1=ALU.subtract)
                        nc.scalar.activation(
                            out=outT[:, po, cs], in_=t2,
                            func=(AF.Copy if GELU_AS_COPY else AF.Gelu))
                    else:
                        nc.vector.scalar_tensor_tensor(
                            out=outT[:, po, cs], in0=t1, scalar=be_sl, in1=pMg,
                            op0=ALU.add, op1=ALU.subtract)

    h1T = acts.tile([128, KD1, T], BF16, tag="QT")
    mlp_layer(0, embT, KH, w1T, KD1, OF_B1, OF_G1, PP_BE1, WB1, 0, True, h1T)
    if _stop("mlp1"):
        fin()
        return
    h2T = acts.tile([128, KD1, T], BF16, tag="KT")   # planes 0:KD2 used
    mlp_layer(1, h1T, KD1, w2T, KD2, OF_B2, OF_G2, PP_BE2, WB2, 1, True, h2T)
    if _stop("mlp2"):
        fin()
        return
    featT = acts.tile([128, KH, T], BF16, tag="XT")  # planes 0:KD3 used
    mlp_layer(2, h2T, KD2, w3T, KD3, OF_B3, OF_G3, PP_BE3, WB3, 2, False, featT)

    if _stop("mlp3"):
        fin()
        return

    # ---- P9: CE in [9, T] layout ----
    with tc.tile_pool(name="pce", bufs=2, space="PSUM") as pce, \
         tc.tile_pool(name="pse", bufs=2, space="PSUM") as pse, \
         tc.tile_pool(name="ppk", bufs=2, space="PSUM") as ppk, \
         tc.tile_pool(name="wce", bufs=2) as wce:
        for c in range(2):
            cs = slice(512 * c, 512 * (c + 1))
            pl = pce.tile([NUM_LABELS, 512], FP32, tag="pl", name="pl")
            for d3 in range(KD3):
                nc.tensor.matmul(pl, wcT[:, d3, :], featT[:, d3, cs],
                                 start=(d3 == 0), stop=False)
            for f in range(KH):
                nc.tensor.matmul(pl, wrT[:, f, :], embT[:, f, cs],
                                 start=False, stop=False)
            nc.tensor.matmul(pl, rowpack[:, OF_BCLS:OF_BCLS + NUM_LABELS],
                             ones_row[:, 0:512], start=False, stop=True)
            e9 = wce.tile([NUM_LABELS, 512], BF16, tag="e9", name="e9")
            nc.scalar.activation(out=e9, in_=pl, func=AF.Exp)
            ps_se = pse.tile([1, 512], FP32, tag="se", name="se")
            nc.tensor.matmul(ps_se, ones9, e9, start=True, stop=True)
            lns = wce.tile([1, 512], FP32, tag="lns", name="lns")
            nc.scalar.activation(out=lns, in_=ps_se, func=AF.Ln)
            nc.vector.reduce_sum(out=partials[0:1, 3 + c:4 + c], in_=lns,
                                 axis=AX.X)
            po9 = wce.tile([NUM_LABELS, 512], FP32, tag="po9", name="po9")
            nc.vector.tensor_mul(out=po9, in0=onehot[:, cs], in1=pl)
            pr9 = wce.tile([NUM_LABELS, 1], BF16, tag="pr9", name="pr9")
            nc.vector.reduce_sum(out=pr9, in_=po9, axis=AX.X)
            ps_pk = ppk.tile([1, 1], FP32, tag="pk", name="pk")
            nc.tensor.matmul(ps_pk, pr9, ones9, start=True, stop=True)
            nc.vector.tensor_copy(out=partials[0:1, 5 + c:6 + c], in_=ps_pk)

    if _stop("ce"):
        fin()
        return

    # ---- P7: quadruplet loss (emb -> token-major, host-built selectors) ----
    emb_tok = acts.tile([128, NT, H], BF16, tag="XTOK")
    with tc.tile_pool(name="ptp2", bufs=4, space="PSUM") as ptp2:
        i = 0
        for t in range(NT):
            for f in range(KH):
                ps = ptp2.tile([128, 128], BF16, tag="tr2", name="tr2")
                nc.tensor.transpose(ps, embT[:, f, 128 * t:128 * (t + 1)],
                                    ident)
                if i % 2 == 0:
                    nc.scalar.activation(out=emb_tok[:, t, 128 * f:128 * (f + 1)],
                                         in_=ps, func=AF.Copy)
                else:
                    nc.vector.tensor_copy(out=emb_tok[:, t, 128 * f:128 * (f + 1)],
                                          in_=ps)
                i += 1
    with tc.tile_pool(name="pqd", bufs=1, space="PSUM") as pqd_pool, \
         tc.tile_pool(name="wqd", bufs=1) as wqd:
        pq1 = pqd_pool.tile([6, 512], FP32, tag="pq1", name="pq1")
        pq2 = pqd_pool.tile([6, H - 512], FP32, tag="pq2", name="pq2")
        for t in range(NT):
            nc.tensor.matmul(pq1, sel[:, t, :], emb_tok[:, t, 0:512],
                             start=(t == 0), stop=(t == NT - 1))
        for t in range(NT):
            nc.tensor.matmul(pq2, sel[:, t, :], emb_tok[:, t, 512:H],
                             start=(t == 0), stop=(t == NT - 1))
        dq1 = wqd.tile([6, 512], FP32, tag="dq1", name="dq1")
        nc.scalar.activation(out=dq1, in_=pq1, func=AF.Square)
        dq2 = wqd.tile([6, H - 512], FP32, tag="dq2", name="dq2")
        nc.scalar.activation(out=dq2, in_=pq2, func=AF.Square)
        d1c = wqd.tile([6, 1], FP32, tag="d1c", name="d1c")
        nc.vector.reduce_sum(out=d1c, in_=dq1, axis=AX.X)
        d2c = wqd.tile([6, 1], FP32, tag="d2c", name="d2c")
        nc.vector.reduce_sum(out=d2c, in_=dq2, axis=AX.X)
        dist = wqd.tile([6, 1], FP32, tag="dist", name="dist")
        nc.vector.tensor_add(out=dist, in0=d1c, in1=d2c)
        pqd = pqd_pool.tile([4, 1], FP32, tag="pqd", name="pqd")
        nc.tensor.matmul(pqd, combo, dist, start=True, stop=True)
        qrelu = wqd.tile([4, 1], FP32, tag="qrelu", name="qrelu")
        nc.scalar.activation(out=qrelu, in_=pqd, func=AF.Relu,
                             bias=ppack[0:4, PP_MARG:PP_MARG + 1])
        psq = pqd_pool.tile([1, 1], FP32, tag="psq", name="psq")
        nc.tensor.matmul(psq, qrelu, ones4f, start=True, stop=True)
        nc.vector.tensor_copy(out=partials[0:1, 0:1], in_=psq)

    fin()


def _get_nc():
    global _CACHED
    if _CACHED is None:
        _CACHED = _build()
    return _CACHED


def _shard(inputs):
    f32 = lambda a: np.asarray(a, np.float32)
    bf = lambda a: np.ascontiguousarray(np.asarray(a, np.float32)).astype(BFD)

    seq = f32(inputs["sequence_output"])
    labels = np.asarray(inputs["labels"]).astype(np.int64)
    a_p = np.asarray(inputs["anchor_positions"]).astype(np.int64)
    p_p = np.asarray(inputs["positive_positions"]).astype(np.int64)
    n1_p = np.asarray(inputs["negative1_positions"]).astype(np.int64)
    n2_p = np.asarray(inputs["negative2_positions"]).astype(np.int64)

    w_qkv, b_qkv = f32(inputs["w_qkv"]), f32(inputs["b_qkv"])
    w_o, b_o = f32(inputs["w_o"]), f32(inputs["b_o"])
    w1, b1 = f32(inputs["w1"]), f32(inputs["b1"])
    g1, be1 = f32(inputs["g1"]), f32(inputs["be1"])
    w2, b2 = f32(inputs["w2"]), f32(inputs["b2"])
    g2, be2 = f32(inputs["g2"]), f32(inputs["be2"])
    w3, b3 = f32(inputs["w3"]), f32(inputs["b3"])
    g3, be3 = f32(inputs["g3"]), f32(inputs["be3"])
    wc, bc = f32(inputs["wc"]), f32(inputs["bc"])
    wr, br = f32(inputs["wr"]), f32(inputs["br"])

    wq, wk, wv = w_qkv[0:H], w_qkv[H:2 * H], w_qkv[2 * H:3 * H]
    bq, bk, bv = b_qkv[0:H], b_qkv[H:2 * H], b_qkv[2 * H:3 * H]

    wqk = np.concatenate([wq.T, wk.T], axis=1)               # [H, 2H]
    wv97 = np.zeros((H, NH, 97), np.float32)
    wv97[:, :, :96] = wv.T.reshape(H, NH, HD)
    wv97 = wv97.reshape(H, 776)
    wopk = np.ascontiguousarray(
        w_o.T.reshape(NH, HD, H).transpose(1, 0, 2)).reshape(HD * NH, H)

    rowpack = np.zeros((1, ROWN), np.float32)
    rowpack[0, OF_BQK:OF_BQK + H] = bq
    rowpack[0, OF_BQK + H:OF_BQK + 2 * H] = bk
    bv97 = np.zeros((NH, 97), np.float32)
    bv97[:, :96] = bv.reshape(NH, HD)
    bv97[:, 96] = 1.0
    rowpack[0, OF_BV:OF_BV + 776] = bv97.reshape(776)
    rowpack[0, OF_BO:OF_BO + H] = b_o
    rowpack[0, OF_B1:OF_B1 + D1] = b1
    rowpack[0, OF_B2:OF_B2 + D2] = b2
    rowpack[0, OF_B3:OF_B3 + D3] = b3
    rowpack[0, OF_BCLS:OF_BCLS + NUM_LABELS] = bc + ALPHA * br
    rowpack[0, OF_S1B + 0] = b1.sum()
    rowpack[0, OF_S1B + 1] = b2.sum()
    rowpack[0, OF_S1B + 2] = b3.sum()
    rowpack[0, OF_G1:OF_G1 + D1] = g1
    rowpack[0, OF_G2:OF_G2 + D2] = g2
    rowpack[0, OF_G3:OF_G3 + D3] = g3

    ppack = np.zeros((128, PPC), np.float32)
    ppack[:, PP_BE1:PP_BE1 + KD1] = be1.reshape(KD1, 128).T
    ppack[:, PP_BE2:PP_BE2 + KD2] = be2.reshape(KD2, 128).T
    ppack[:, PP_BE3:PP_BE3 + KD3] = be3.reshape(KD3, 128).T
    ppack[0:4, PP_MARG] = [MARGIN1, MARGIN2, MARGIN1, MARGIN2]

    wbcol = np.zeros((128, WBC), np.float32)
    wbcol[:, WB1:WB1 + KH] = w1.sum(axis=0).reshape(KH, 128).T
    wbcol[:, WB2:WB2 + KD1] = w2.sum(axis=0).reshape(KD1, 128).T
    wbcol[:, WB3:WB3 + KD2] = w3.sum(axis=0).reshape(KD2, 128).T

    combo = np.zeros((6, 4), np.float32)
    for k, (ipd, ind) in enumerate([(0, 1), (0, 2), (3, 4), (3, 5)]):
        combo[ipd, k] = 1.0
        combo[ind, k] = -1.0

    base = {
        "wqk": bf(wqk), "wv97": bf(wv97), "wopk": bf(wopk),
        "w1t": bf(w1.T), "w2t": bf(w2.T), "w3t": bf(w3.T),
        "wct": bf(wc.T), "wrt": bf(ALPHA * wr.T),
        "rowpack": bf(rowpack), "ppack": np.ascontiguousarray(ppack),
        "wbcol": bf(wbcol), "combo": np.ascontiguousarray(combo),
    }

    in_maps = []
    for c in range(NCORES):
        sl = slice(BL * c, BL * (c + 1))
        lab = labels[sl]                       # [BL, S]
        oh = np.zeros((NUM_LABELS, T), np.float32)
        flat = lab.reshape(T)
        oh[flat, np.arange(T)] = 1.0
        m = (lab[:, :-1] != 0) & (lab[:, :-1] == lab[:, 1:])   # [BL, S-1]
        cm = np.zeros((1, T - 1), np.float32)
        for s in range(BL):
            cm[0, S * s:S * s + S - 1] = m[s]
        selm = np.zeros((T, 6), np.float32)
        for s in range(BL):
            b = BL * c + s
            av = int(a_p[b]) + S * s
            for j, pos in enumerate((p_p[b], n1_p[b], n2_p[b])):
                col = 3 * s + j
                selm[av, col] += 1.0
                selm[int(pos) + S * s, col] -= 1.0
        mm = dict(base)
        mm["x"] = bf(seq[sl].reshape(T, H))
        mm["onehot"] = bf(oh)
        mm["ctxmask"] = bf(cm)
        mm["sel"] = bf(selm)
        in_maps.append(mm)
    return in_maps


def kernel(**inputs):
    nc = _get_nc()
    in_maps = _shard(inputs)
    res = run_bass_kernel_spmd(nc, in_maps, core_ids=list(range(NCORES)))
    ce = quad = ctx = 0.0
    for c in range(NCORES):
        o = np.asarray(res.results[c]["out"], np.float64).reshape(8)
        quad += float(o[0])
        ctx += float(o[1] + o[2])
        ce += float(o[3] + o[4] - o[5] - o[6])
    total = ce / (B * S) + ALPHA * (quad / B) + BETA * (ctx / (B * S))
    return np.float32(total)


# revision 5
# speedup vs baseline: 1.0960x; 1.0576x over previous
"""Trainium2 Bass kernel for nn_LLMCC_74414603370526 (loss_fn) — v2.

Data-parallel over batch: 16 sequences -> 8 cores x 2 sequences each; host
combines the scalar partial losses (the sanctioned all-reduce).

v2 design (instruction-count-minimized vs v1):
  - activations flow feature-major end-to-end; the only PE transposes are
    x (48) and the emb token-major copy needed by the quadruplet gather (48)
  - weights are pre-transposed on the HOST into DMA-friendly layouts; no
    on-device weight transposes
  - biases/residual/softmax-rowsum fold into matmuls (K=1 outer products,
    identity lhsT accumulate, ones column appended to v)
  - LayerNorm runs feature-major: per-token stats via ones/w-bar matmul
    column reductions; scale/shift applied via PE rank-1 outer products
    (g x rstd) and two vector ops per tile
  - CE in [9, T] label-major layout with host-built one-hot labels;
    context-loss label mask is host-built
  - SBUF slots reused via byte-equal tag chains:
    XTOK: x_tok -> emb_tok | XT: xT -> featT | QT: qT -> h1T
    KT: kT -> h2T | WQKT: wqkT -> w1T | WOT: woT -> w2T
"""

import numpy as np
import ml_dtypes

import concourse.bass as bass
import concourse.mybir as mybir
import concourse.tile as tile
from concourse import bacc
from concourse.bass_utils import run_bass_kernel_spmd
from concourse.masks import make_identity

FP32 = mybir.dt.float32
BF16 = mybir.dt.bfloat16
AF = mybir.ActivationFunctionType
ALU = mybir.AluOpType
AX = mybir.AxisListType

B, S, H = 16, 512, 768
NH, HD = 8, 96
NUM_LABELS = 9
MARGIN1, MARGIN2 = 1.0, 0.5
ALPHA, BETA = 0.2, 0.1
EPS = 1e-5

NCORES = 8
BL = B // NCORES          # 2 sequences per core
T = BL * S                # 1024 tokens per core
NT = T // 128             # 8 token tiles
KH = H // 128             # 6 feature tiles
D1, D2, D3 = 1024, 512, 256
KD1, KD2, KD3 = D1 // 128, D2 // 128, D3 // 128
ISQ = 1.0 / float(np.sqrt(HD))
BFD = ml_dtypes.bfloat16

# rowpack offsets (bf16 row: biases / row-vectors)
OF_BQK = 0                       # 1536
OF_BV = OF_BQK + 2 * H           # 776
OF_BO = OF_BV + 776              # 768
OF_B1 = OF_BO + H                # 1024
OF_B2 = OF_B1 + D1               # 512
OF_B3 = OF_B2 + D2               # 256
OF_BCLS = OF_B3 + D3             # 9
OF_S1B = OF_BCLS + NUM_LABELS    # 3
OF_G1 = OF_S1B + 3               # 1024
OF_G2 = OF_G1 + D1               # 512
OF_G3 = OF_G2 + D2               # 256
ROWN = OF_G3 + D3

# ppack (fp32 per-partition pack)
PP_BE1 = 0          # 8 cols
PP_BE2 = 8          # 4
PP_BE3 = 12         # 2
PP_MARG = 14        # margins in rows 0:4 of col 14
PPC = 15

# wbcol (bf16 per-partition colsum-weight pack)
WB1, WB2, WB3 = 0, 6, 14
WBC = 18

GELU_AS_COPY = False   # sim-debug only (CoreSim executor lacks Gelu numerics)

_CACHED = None


def _build(stop_after=None):
    nc = bacc.Bacc(None, target_bir_lowering=False)
    dd = {}

    def di(name, shape, dt=BF16):
        dd[name] = nc.dram_tensor(name, shape, dt, kind="ExternalInput")

    di("x", [T, H])
    di("wqk", [H, 2 * H])
    di("wv97", [H, 776])
    di("wopk", [HD * NH, H])       # rows = (d-within-head, head), cols f
    di("w1t", [H, D1])
    di("w2t", [D1, D2])
    di("w3t", [D2, D3])
    di("wct", [D3, NUM_LABELS])
    di("wrt", [H, NUM_LABELS])     # pre-scaled by 0.2 on host
    di("rowpack", [1, ROWN])
    di("ppack", [128, PPC], FP32)
    di("wbcol", [128, WBC])
    di("onehot", [NUM_LABELS, T])
    di("ctxmask", [1, T - 1])
    di("sel", [T, 6])
    di("combo", [6, 4], FP32)
    out_d = nc.dram_tensor("out", [1, 8], FP32, kind="ExternalOutput")

    with tile.TileContext(nc) as tc:
        with nc.allow_low_precision(reason="bf16 PE-transpose PSUM tiles"):
            _body(nc, tc, dd, out_d, stop_after)
    nc.finalize()
    return nc


def _body(nc, tc, dd, out_d, stop_after=None):
    const = tc.alloc_tile_pool(name="const", bufs=1)
    acts = tc.alloc_tile_pool(name="acts", bufs=1)

    def _stop(phase):
        return stop_after == phase

    def fin():
        nc.sync.dma_start(out=out_d[:, :], in_=partials)
        acts.release()
        const.release()


    # ---- constants / inputs ----
    ident = const.tile([128, 128], BF16)
    make_identity(nc, ident)
    ones_row = const.tile([1, 1024], BF16)
    nc.vector.memset(ones_row, 1.0)
    onescol = const.tile([128, 1], BF16)
    nc.vector.memset(onescol, 1.0)
    ones9 = const.tile([NUM_LABELS, 1], BF16)
    nc.vector.memset(ones9, 1.0)
    ones4f = const.tile([4, 1], FP32)
    nc.vector.memset(ones4f, 1.0)
    eps_t = const.tile([1, 1], FP32)
    nc.vector.memset(eps_t, EPS)
    partials = const.tile([1, 8], FP32)
    nc.vector.memset(partials, 0.0)

    x_tok = acts.tile([128, NT, H], BF16, tag="XTOK")
    nc.sync.dma_start(out=x_tok, in_=dd["x"].rearrange("(n p) h -> p n h", p=128))
    wqkT = acts.tile([128, KH, 1536], BF16, tag="WQKT")
    nc.sync.dma_start(out=wqkT, in_=dd["wqk"].rearrange("(f p) c -> p f c", p=128))
    rowpack = const.tile([1, ROWN], BF16)
    nc.sync.dma_start(out=rowpack, in_=dd["rowpack"][:, :])
    wv97T = const.tile([128, KH, 776], BF16)
    nc.gpsimd.dma_start(out=wv97T, in_=dd["wv97"].rearrange("(f p) c -> p f c", p=128))
    woT = acts.tile([HD, NH, H], BF16, tag="WOT")
    nc.gpsimd.dma_start(out=woT, in_=dd["wopk"].rearrange("(p h) f -> p h f", p=HD))
    w3T = const.tile([128, KD2, D3], BF16)
    nc.gpsimd.dma_start(out=w3T, in_=dd["w3t"].rearrange("(f p) c -> p f c", p=128))
    wcT = const.tile([128, KD3, NUM_LABELS], BF16)
    nc.gpsimd.dma_start(out=wcT, in_=dd["wct"].rearrange("(f p) c -> p f c", p=128))
    wrT = const.tile([128, KH, NUM_LABELS], BF16)
    nc.gpsimd.dma_start(out=wrT, in_=dd["wrt"].rearrange("(f p) c -> p f c", p=128))
    ppack = const.tile([128, PPC], FP32)
    nc.gpsimd.dma_start(out=ppack, in_=dd["ppack"][:, :])
    wbcol = const.tile([128, WBC], BF16)
    nc.gpsimd.dma_start(out=wbcol, in_=dd["wbcol"][:, :])
    onehot = const.tile([NUM_LABELS, T], BF16)
    nc.gpsimd.dma_start(out=onehot, in_=dd["onehot"][:, :])
    ctxmask = const.tile([1, T - 1], BF16)
    nc.gpsimd.dma_start(out=ctxmask, in_=dd["ctxmask"][:, :])
    sel = const.tile([128, NT, 6], BF16)
    nc.gpsimd.dma_start(out=sel, in_=dd["sel"].rearrange("(n p) c -> p n c", p=128))
    combo = const.tile([6, 4], FP32)
    nc.gpsimd.dma_start(out=combo, in_=dd["combo"][:, :])

    if _stop("load"):
        fin()
        return

    # ---- P1: x -> xT (feature-major; 4 transposes share one PSUM bank) ----
    xT = acts.tile([128, KH, T], BF16, tag="XT")
    with tc.tile_pool(name="ptp", bufs=4, space="PSUM") as ptp:
        i = 0
        for f in range(KH):
            for tq in range(0, NT, 4):
                ps = ptp.tile([128, 512], BF16, tag="tr", name="tr")
                for j in range(4):
                    nc.tensor.transpose(
                        ps[:, 128 * j:128 * (j + 1)],
                        x_tok[:, tq + j, 128 * f:128 * (f + 1)], ident)
                if i % 2 == 0:
                    nc.scalar.activation(
                        out=xT[:, f, 128 * tq:128 * (tq + 4)], in_=ps,
                        func=AF.Copy)
                else:
                    nc.vector.tensor_copy(
                        out=xT[:, f, 128 * tq:128 * (tq + 4)], in_=ps)
                i += 1

    if _stop("p1"):
        fin()
        return

    # ---- P2+P3: q, k, v ----
    qT = acts.tile([HD, NH, T], BF16, tag="QT")
    kT = acts.tile([HD, NH, T], BF16, tag="KT")
    v97 = acts.tile([128, NT, 776], BF16, tag="V97")
    with tc.tile_pool(name="pq", bufs=4, space="PSUM") as pq, \
         tc.tile_pool(name="pv", bufs=2, space="PSUM") as pv:
        i = 0
        for h in range(NH):
            for w in range(2):      # 0=q, 1=k
                dst = qT if w == 0 else kT
                off = H * w + HD * h
                for c in range(2):
                    ps = pq.tile([HD, 512], FP32, tag="pqk", name="pqk")
                    for f in range(KH):
                        nc.tensor.matmul(ps, wqkT[:, f, off:off + HD],
                                         xT[:, f, 512 * c:512 * (c + 1)],
                                         start=(f == 0), stop=False)
                    nc.tensor.matmul(ps,
                                     rowpack[:, OF_BQK + off:OF_BQK + off + HD],
                                     ones_row[:, 0:512], start=False, stop=True)
                    if i % 2 == 0:
                        nc.scalar.activation(out=dst[:, h, 512 * c:512 * (c + 1)],
                                             in_=ps, func=AF.Copy)
                    else:
                        nc.vector.tensor_copy(out=dst[:, h, 512 * c:512 * (c + 1)],
                                              in_=ps)
                    i += 1
        for t in range(NT):
            for g in range(2):
                ps = pv.tile([128, 388], FP32, tag="pv", name="pv")
                for f in range(KH):
                    nc.tensor.matmul(ps, xT[:, f, 128 * t:128 * (t + 1)],
                                     wv97T[:, f, 388 * g:388 * (g + 1)],
                                     start=(f == 0), stop=False)
                nc.tensor.matmul(ps, ones_row[0:1, 0:128],
                                 rowpack[:, OF_BV + 388 * g:OF_BV + 388 * (g + 1)],
                                 start=False, stop=True)
                if t % 2 == 0:
                    nc.scalar.activation(out=v97[:, t, 388 * g:388 * (g + 1)],
                                         in_=ps, func=AF.Copy)
                else:
                    nc.vector.tensor_copy(out=v97[:, t, 388 * g:388 * (g + 1)],
                                          in_=ps)

    if _stop("qkv"):
        fin()
        return

    # ---- P4: attention (exp without max-shift; rowsum via v ones column) ----
    aoT = acts.tile([HD, BL, NH, S], BF16, tag="AOT")
    with tc.tile_pool(name="pa", bufs=4, space="PSUM") as pa, \
         tc.tile_pool(name="pa2", bufs=2, space="PSUM") as pa2, \
         tc.tile_pool(name="pa3", bufs=2, space="PSUM") as pa3, \
         tc.tile_pool(name="wet", bufs=6) as wet, \
         tc.tile_pool(name="wra", bufs=2) as wra:
        def tail(sh):
            s, h, pao = sh
            rec = wra.tile([1, S], BF16, tag="rec", name="rec")
            nc.vector.reciprocal(out=rec, in_=pao[HD:HD + 1, :])
            prec = pa3.tile([HD, S], FP32, tag="prec", name="prec")
            nc.tensor.matmul(prec, ones_row[0:1, 0:HD], rec, start=True,
                             stop=True)
            aou = wra.tile([HD, S], BF16, tag="aou", name="aou")
            nc.scalar.activation(out=aou, in_=pao[0:HD, :], func=AF.Copy)
            nc.vector.tensor_mul(out=aoT[:, s, h, :], in0=aou, in1=prec)

        pending = None
        for s in range(BL):
            for h in range(NH):
                et = []
                for kt in range(4):
                    psc = pa.tile([128, S], FP32, tag="psc", name="psc")
                    nc.tensor.matmul(
                        psc, kT[:, h, S * s + 128 * kt:S * s + 128 * (kt + 1)],
                        qT[:, h, S * s:S * (s + 1)], start=True, stop=True)
                    e = wet.tile([128, S], BF16, tag="et", name="et")
                    nc.scalar.activation(out=e, in_=psc, func=AF.Exp, scale=ISQ)
                    et.append(e)
                pao = pa2.tile([HD + 1, S], FP32, tag="pao", name="pao")
                g, hh = h // 4, h % 4
                voff = 388 * g + 97 * hh
                for kt in range(4):
                    nc.tensor.matmul(pao, v97[:, 4 * s + kt, voff:voff + 97],
                                     et[kt], start=(kt == 0), stop=(kt == 3))
                if pending is not None:
                    tail(pending)
                pending = (s, h, pao)
        tail(pending)

    if _stop("attn"):
        fin()
        return

    # ---- P5: w_o + bias + residual -> embT ----
    embT = acts.tile([128, KH, T], BF16, tag="EMBT")
    with tc.tile_pool(name="pw", bufs=3, space="PSUM") as pw:
        for f in range(KH):
            for s in range(BL):
                ps = pw.tile([128, S], FP32, tag="pwo", name="pwo")
                for h in range(NH):
                    nc.tensor.matmul(ps, woT[:, h, 128 * f:128 * (f + 1)],
                                     aoT[:, s, h, :], start=(h == 0), stop=False)
                nc.tensor.matmul(ps,
                                 rowpack[:, OF_BO + 128 * f:OF_BO + 128 * (f + 1)],
                                 ones_row[:, 0:S], start=False, stop=False)
                nc.tensor.matmul(ps, ident, xT[:, f, S * s:S * (s + 1)],
                                 start=False, stop=True)
                if (2 * f + s) % 2 == 0:
                    nc.scalar.activation(out=embT[:, f, S * s:S * (s + 1)],
                                         in_=ps, func=AF.Copy)
                else:
                    nc.vector.tensor_copy(out=embT[:, f, S * s:S * (s + 1)],
                                          in_=ps)

    # late weight loads into freed slots (scalar DMA queue, off the hot path)
    w1T = acts.tile([128, KH, 1536], BF16, tag="WQKT")
    nc.scalar.dma_start(out=w1T[:, :, 0:D1],
                        in_=dd["w1t"].rearrange("(f p) c -> p f c", p=128))
    w2T = acts.tile([128, NH, H], BF16, tag="WOT")
    nc.scalar.dma_start(out=w2T[:, :, 0:D2],
                        in_=dd["w2t"].rearrange("(f p) c -> p f c", p=128))

    if _stop("wo"):
        fin()
        return

    # ---- P6: context loss (chunked) ----
    with tc.tile_pool(name="pctx", bufs=1, space="PSUM") as pcx, \
         tc.tile_pool(name="wctx", bufs=2) as wctx:
        chunks = [(0, 512), (512, T - 1)]
        for ci, (lo, hi) in enumerate(chunks):
            n = hi - lo
            pc = pcx.tile([1, 512], FP32, tag=f"pc{ci}", name="pc")
            for f in range(KH):
                d_ = wctx.tile([128, 512], BF16, tag="ctxd", name="ctxd")
                nc.vector.tensor_sub(out=d_[:, 0:n], in0=embT[:, f, lo:hi],
                                     in1=embT[:, f, lo + 1:hi + 1])
                dsq = wctx.tile([128, 512], BF16, tag="ctxsq", name="ctxsq")
                nc.scalar.activation(out=dsq[:, 0:n], in_=d_[:, 0:n],
                                     func=AF.Square)
                nc.tensor.matmul(pc[:, 0:n], onescol, dsq[:, 0:n],
                                 start=(f == 0), stop=(f == KH - 1))
            nrm = wctx.tile([1, 512], FP32, tag="nrm", name="nrm")
            nc.scalar.activation(out=nrm[:, 0:n], in_=pc[:, 0:n], func=AF.Sqrt)
            msk = wctx.tile([1, 512], FP32, tag="msk", name="msk")
            nc.vector.tensor_mul(out=msk[:, 0:n], in0=nrm[:, 0:n],
                                 in1=ctxmask[:, lo:hi])
            nc.vector.reduce_sum(out=partials[0:1, 1 + ci:2 + ci],
                                 in_=msk[:, 0:n], axis=AX.X)

    if _stop("ctx"):
        fin()
        return

    # ---- P8: MLP layers (feature-major LayerNorm) ----
    def mlp_layer(li, inT, kd, wT, kdo, b_off, g_off, be_off, wb_off, s1_i,
                  gelu, outT):
        dout = 128 * kdo
        inv_d = 1.0 / float(dout)
        with tc.tile_pool(name=f"pm{li}", bufs=2, space="PSUM") as pm, \
             tc.tile_pool(name=f"pp1_{li}", bufs=1, space="PSUM") as pp1, \
             tc.tile_pool(name=f"pp2_{li}", bufs=1, space="PSUM") as pp2, \
             tc.tile_pool(name=f"pb{li}", bufs=2, space="PSUM") as pb, \
             tc.tile_pool(name=f"wk{li}", bufs=1) as wk, \
             tc.tile_pool(name=f"ap{li}", bufs=2) as ap, \
             tc.tile_pool(name=f"sq{li}", bufs=3) as sqp, \
             tc.tile_pool(name=f"zb{li}", bufs=kdo + 1) as zbp:
            for c in range(2):
                cs = slice(512 * c, 512 * (c + 1))
                ps1 = pp1.tile([1, 512], FP32, tag="s1", name="s1")
                for f in range(kd):
                    nc.tensor.matmul(ps1, wbcol[:, wb_off + f:wb_off + f + 1],
                                     inT[:, f, cs], start=(f == 0), stop=False)
                nc.tensor.matmul(ps1, rowpack[:, OF_S1B + s1_i:OF_S1B + s1_i + 1],
                                 ones_row[:, 0:512], start=False, stop=True)
                ps2 = pp2.tile([1, 512], FP32, tag="s2", name="s2")
                zs, sqs = [], []
                for po in range(kdo):
                    ps = pm.tile([128, 512], FP32, tag="z", name="z")
                    for f in range(kd):
                        nc.tensor.matmul(ps, wT[:, f, 128 * po:128 * (po + 1)],
                                         inT[:, f, cs], start=(f == 0),
                                         stop=False)
                    nc.tensor.matmul(
                        ps, rowpack[:, b_off + 128 * po:b_off + 128 * (po + 1)],
                        ones_row[:, 0:512], start=False, stop=True)
                    zb = zbp.tile([128, 512], BF16, tag="zb", name="zb")
                    nc.vector.tensor_copy(out=zb, in_=ps)
                    sq = sqp.tile([128, 512], BF16, tag="sq", name="sq")
                    nc.scalar.activation(out=sq, in_=ps, func=AF.Square)
                    zs.append(zb)
                    sqs.append(sq)
                    if po >= 1:
                        nc.tensor.matmul(ps2, onescol, sqs[po - 1],
                                         start=(po == 1), stop=False)
                nc.tensor.matmul(ps2, onescol, sqs[kdo - 1],
                                 start=(kdo == 1), stop=True)
                mu = wk.tile([1, 512], FP32, tag="mu", name="mu")
                nc.vector.tensor_scalar(out=mu, in0=ps1, scalar1=inv_d,
                                        scalar2=None, op0=ALU.mult)
                m2 = wk.tile([1, 512], FP32, tag="m2", name="m2")
                nc.vector.tensor_scalar(out=m2, in0=ps2, scalar1=inv_d,
                                        scalar2=None, op0=ALU.mult)
                musq = wk.tile([1, 512], FP32, tag="musq", name="musq")
                nc.vector.tensor_mul(out=musq, in0=mu, in1=mu)
                var = wk.tile([1, 512], FP32, tag="var", name="var")
                nc.vector.tensor_sub(out=var, in0=m2, in1=musq)
                sd = wk.tile([1, 512], FP32, tag="sd", name="sd")
                nc.scalar.activation(out=sd, in_=var, func=AF.Sqrt, bias=eps_t)
                r_bf = wk.tile([1, 512], BF16, tag="rbf", name="rbf")
                nc.vector.reciprocal(out=r_bf, in_=sd)
                mur = wk.tile([1, 512], BF16, tag="mur", name="mur")
                nc.vector.tensor_mul(out=mur, in0=mu, in1=r_bf)
                for po in range(kdo):
                    gsl = rowpack[:, g_off + 128 * po:g_off + 128 * (po + 1)]
                    pRg = pb.tile([128, 512], FP32, tag="rg", name="rg")
                    nc.tensor.matmul(pRg, gsl, r_bf, start=True, stop=True)
                    pMg = pb.tile([128, 512], FP32, tag="mg", name="mg")
                    nc.tensor.matmul(pMg, gsl, mur, start=True, stop=True)
                    t1 = ap.tile([128, 512], BF16, tag="t1", name="t1")
                    nc.vector.tensor_mul(out=t1, in0=zs[po], in1=pRg)
                    be_sl = ppack[:, be_off + po:be_off + po + 1]
                    if gelu:
                        t2 = ap.tile([128, 512], BF16, tag="t2", name="t2")
                        nc.vector.scalar_tensor_tensor(
                            out=t2, in0=t1, scalar=be_sl, in1=pMg,
                            op0=ALU.add, op1=ALU.subtract)
                        nc.scalar.activation(
                            out=outT[:, po, cs], in_=t2,
                            func=(AF.Copy if GELU_AS_COPY else AF.Gelu))
                    else:
                        nc.vector.scalar_tensor_tensor(
                            out=outT[:, po, cs], in0=t1, scalar=be_sl, in1=pMg,
                            op0=ALU.add, op1=ALU.subtract)

    h1T = acts.tile([128, KD1, T], BF16, tag="QT")
    mlp_layer(0, embT, KH, w1T, KD1, OF_B1, OF_G1, PP_BE1, WB1, 0, True, h1T)
    if _stop("mlp1"):
        fin()
        return
    h2T = acts.tile([128, KD1, T], BF16, tag="KT")   # planes 0:KD2 used
    mlp_layer(1, h1T, KD1, w2T, KD2, OF_B2, OF_G2, PP_BE2, WB2, 1, True, h2T)
    if _stop("mlp2"):
        fin()
        return
    featT = acts.tile([128, KH, T], BF16, tag="XT")  # planes 0:KD3 used
    mlp_layer(2, h2T, KD2, w3T, KD3, OF_B3, OF_G3, PP_BE3, WB3, 2, False, featT)

    if _stop("mlp3"):
        fin()
        return

    # ---- P9: CE in [9, T] layout ----
    with tc.tile_pool(name="pce", bufs=2, space="PSUM") as pce, \
         tc.tile_pool(name="pse", bufs=2, space="PSUM") as pse, \
         tc.tile_pool(name="ppk", bufs=2, space="PSUM") as ppk, \
         tc.tile_pool(name="wce", bufs=2) as wce:
        for c in range(2):
            cs = slice(512 * c, 512 * (c + 1))
            pl = pce.tile([NUM_LABELS, 512], FP32, tag="pl", name="pl")
            for d3 in range(KD3):
                nc.tensor.matmul(pl, wcT[:, d3, :], featT[:, d3, cs],
                                 start=(d3 == 0), stop=False)
            for f in range(KH):
                nc.tensor.matmul(pl, wrT[:, f, :], embT[:, f, cs],
                                 start=False, stop=False)
            nc.tensor.matmul(pl, rowpack[:, OF_BCLS:OF_BCLS + NUM_LABELS],
                             ones_row[:, 0:512], start=False, stop=True)
            e9 = wce.tile([NUM_LABELS, 512], BF16, tag="e9", name="e9")
            nc.scalar.activation(out=e9, in_=pl, func=AF.Exp)
            ps_se = pse.tile([1, 512], FP32, tag="se", name="se")
            nc.tensor.matmul(ps_se, ones9, e9, start=True, stop=True)
            lns = wce.tile([1, 512], FP32, tag="lns", name="lns")
            nc.scalar.activation(out=lns, in_=ps_se, func=AF.Ln)
            nc.vector.reduce_sum(out=partials[0:1, 3 + c:4 + c], in_=lns,
                                 axis=AX.X)
            po9 = wce.tile([NUM_LABELS, 512], FP32, tag="po9", name="po9")
            nc.vector.tensor_mul(out=po9, in0=onehot[:, cs], in1=pl)
            pr9 = wce.tile([NUM_LABELS, 1], BF16, tag="pr9", name="pr9")
            nc.vector.reduce_sum(out=pr9, in_=po9, axis=AX.X)
            ps_pk = ppk.tile([1, 1], FP32, tag="pk", name="pk")
            nc.tensor.matmul(ps_pk, pr9, ones9, start=True, stop=True)
            nc.vector.tensor_copy(out=partials[0:1, 5 + c:6 + c], in_=ps_pk)

    if _stop("ce"):
        fin()
        return

    # ---- P7: quadruplet loss (emb -> token-major, host-built selectors) ----
    emb_tok = acts.tile([128, NT, H], BF16, tag="XTOK")
    with tc.tile_pool(name="ptp2", bufs=4, space="PSUM") as ptp2:
        i = 0
        for t in range(NT):
            for fq, fw in ((0, 4), (4, 2)):
                ps = ptp2.tile([128, 512], BF16, tag="tr2", name="tr2")
                for j in range(fw):
                    nc.tensor.transpose(
                        ps[:, 128 * j:128 * (j + 1)],
                        embT[:, fq + j, 128 * t:128 * (t + 1)], ident)
                w = 128 * fw
                if i % 2 == 0:
                    nc.scalar.activation(
                        out=emb_tok[:, t, 128 * fq:128 * fq + w],
                        in_=ps[:, 0:w], func=AF.Copy)
                else:
                    nc.vector.tensor_copy(
                        out=emb_tok[:, t, 128 * fq:128 * fq + w],
                        in_=ps[:, 0:w])
                i += 1
    with tc.tile_pool(name="pqd", bufs=1, space="PSUM") as pqd_pool, \
         tc.tile_pool(name="wqd", bufs=1) as wqd:
        pq1 = pqd_pool.tile([6, 512], FP32, tag="pq1", name="pq1")
        pq2 = pqd_pool.tile([6, H - 512], FP32, tag="pq2", name="pq2")
        for t in range(NT):
            nc.tensor.matmul(pq1, sel[:, t, :], emb_tok[:, t, 0:512],
                             start=(t == 0), stop=(t == NT - 1))
        for t in range(NT):
            nc.tensor.matmul(pq2, sel[:, t, :], emb_tok[:, t, 512:H],
                             start=(t == 0), stop=(t == NT - 1))
        dq1 = wqd.tile([6, 512], FP32, tag="dq1", name="dq1")
        nc.scalar.activation(out=dq1, in_=pq1, func=AF.Square)
        dq2 = wqd.tile([6, H - 512], FP32, tag="dq2", name="dq2")
        nc.scalar.activation(out=dq2, in_=pq2, func=AF.Square)
        d1c = wqd.tile([6, 1], FP32, tag="d1c", name="d1c")
        nc.vector.reduce_sum(out=d1c, in_=dq1, axis=AX.X)
        d2c = wqd.tile([6, 1], FP32, tag="d2c", name="d2c")
        nc.vector.reduce_sum(out=d2c, in_=dq2, axis=AX.X)
        dist = wqd.tile([6, 1], FP32, tag="dist", name="dist")
        nc.vector.tensor_add(out=dist, in0=d1c, in1=d2c)
        pqd = pqd_pool.tile([4, 1], FP32, tag="pqd", name="pqd")
        nc.tensor.matmul(pqd, combo, dist, start=True, stop=True)
        qrelu = wqd.tile([4, 1], FP32, tag="qrelu", name="qrelu")
        nc.scalar.activation(out=qrelu, in_=pqd, func=AF.Relu,
                             bias=ppack[0:4, PP_MARG:PP_MARG + 1])
        psq = pqd_pool.tile([1, 1], FP32, tag="psq", name="psq")
        nc.tensor.matmul(psq, qrelu, ones4f, start=True, stop=True)
        nc.vector.tensor_copy(out=partials[0:1, 0:1], in_=psq)

    fin()


def _get_nc():
    global _CACHED
    if _CACHED is None:
        _CACHED = _build()
    return _CACHED


def _shard(inputs):
    f32 = lambda a: np.asarray(a, np.float32)
    bf = lambda a: np.ascontiguousarray(np.asarray(a, np.float32)).astype(BFD)

    seq = f32(inputs["sequence_output"])
    labels = np.asarray(inputs["labels"]).astype(np.int64)
    a_p = np.asarray(inputs["anchor_positions"]).astype(np.int64)
    p_p = np.asarray(inputs["positive_positions"]).astype(np.int64)
    n1_p = np.asarray(inputs["negative1_positions"]).astype(np.int64)
    n2_p = np.asarray(inputs["negative2_positions"]).astype(np.int64)

    w_qkv, b_qkv = f32(inputs["w_qkv"]), f32(inputs["b_qkv"])
    w_o, b_o = f32(inputs["w_o"]), f32(inputs["b_o"])
    w1, b1 = f32(inputs["w1"]), f32(inputs["b1"])
    g1, be1 = f32(inputs["g1"]), f32(inputs["be1"])
    w2, b2 = f32(inputs["w2"]), f32(inputs["b2"])
    g2, be2 = f32(inputs["g2"]), f32(inputs["be2"])
    w3, b3 = f32(inputs["w3"]), f32(inputs["b3"])
    g3, be3 = f32(inputs["g3"]), f32(inputs["be3"])
    wc, bc = f32(inputs["wc"]), f32(inputs["bc"])
    wr, br = f32(inputs["wr"]), f32(inputs["br"])

    wq, wk, wv = w_qkv[0:H], w_qkv[H:2 * H], w_qkv[2 * H:3 * H]
    bq, bk, bv = b_qkv[0:H], b_qkv[H:2 * H], b_qkv[2 * H:3 * H]

    wqk = np.concatenate([wq.T, wk.T], axis=1)               # [H, 2H]
    wv97 = np.zeros((H, NH, 97), np.float32)
    wv97[:, :, :96] = wv.T.reshape(H, NH, HD)
    wv97 = wv97.reshape(H, 776)
    wopk = np.ascontiguousarray(
        w_o.T.reshape(NH, HD, H).transpose(1, 0, 2)).reshape(HD * NH, H)

    rowpack = np.zeros((1, ROWN), np.float32)
    rowpack[0, OF_BQK:OF_BQK + H] = bq
    rowpack[0, OF_BQK + H:OF_BQK + 2 * H] = bk
    bv97 = np.zeros((NH, 97), np.float32)
    bv97[:, :96] = bv.reshape(NH, HD)
    bv97[:, 96] = 1.0
    rowpack[0, OF_BV:OF_BV + 776] = bv97.reshape(776)
    rowpack[0, OF_BO:OF_BO + H] = b_o
    rowpack[0, OF_B1:OF_B1 + D1] = b1
    rowpack[0, OF_B2:OF_B2 + D2] = b2
    rowpack[0, OF_B3:OF_B3 + D3] = b3
    rowpack[0, OF_BCLS:OF_BCLS + NUM_LABELS] = bc + ALPHA * br
    rowpack[0, OF_S1B + 0] = b1.sum()
    rowpack[0, OF_S1B + 1] = b2.sum()
    rowpack[0, OF_S1B + 2] = b3.sum()
    rowpack[0, OF_G1:OF_G1 + D1] = g1
    rowpack[0, OF_G2:OF_G2 + D2] = g2
    rowpack[0, OF_G3:OF_G3 + D3] = g3

    ppack = np.zeros((128, PPC), np.float32)
    ppack[:, PP_BE1:PP_BE1 + KD1] = be1.reshape(KD1, 128).T
    ppack[:, PP_BE2:PP_BE2 + KD2] = be2.reshape(KD2, 128).T
    ppack[:, PP_BE3:PP_BE3 + KD3] = be3.reshape(KD3, 128).T
    ppack[0:4, PP_MARG] = [MARGIN1, MARGIN2, MARGIN1, MARGIN2]

    wbcol = np.zeros((128, WBC), np.float32)
    wbcol[:, WB1:WB1 + KH] = w1.sum(axis=0).reshape(KH, 128).T
    wbcol[:, WB2:WB2 + KD1] = w2.sum(axis=0).reshape(KD1, 128).T
    wbcol[:, WB3:WB3 + KD2] = w3.sum(axis=0).reshape(KD2, 128).T

    combo = np.zeros((6, 4), np.float32)
    for k, (ipd, ind) in enumerate([(0, 1), (0, 2), (3, 4), (3, 5)]):
        combo[ipd, k] = 1.0
        combo[ind, k] = -1.0

    base = {
        "wqk": bf(wqk), "wv97": bf(wv97), "wopk": bf(wopk),
        "w1t": bf(w1.T), "w2t": bf(w2.T), "w3t": bf(w3.T),
        "wct": bf(wc.T), "wrt": bf(ALPHA * wr.T),
        "rowpack": bf(rowpack), "ppack": np.ascontiguousarray(ppack),
        "wbcol": bf(wbcol), "combo": np.ascontiguousarray(combo),
    }

    in_maps = []
    for c in range(NCORES):
        sl = slice(BL * c, BL * (c + 1))
        lab = labels[sl]                       # [BL, S]
        oh = np.zeros((NUM_LABELS, T), np.float32)
        flat = lab.reshape(T)
        oh[flat, np.arange(T)] = 1.0
        m = (lab[:, :-1] != 0) & (lab[:, :-1] == lab[:, 1:])   # [BL, S-1]
        cm = np.zeros((1, T - 1), np.float32)
        for s in range(BL):
            cm[0, S * s:S * s + S - 1] = m[s]
        selm = np.zeros((T, 6), np.float32)
        for s in range(BL):
            b = BL * c + s
            av = int(a_p[b]) + S * s
            for j, pos in enumerate((p_p[b], n1_p[b], n2_p[b])):
                col = 3 * s + j
                selm[av, col] += 1.0
                selm[int(pos) + S * s, col] -= 1.0
        mm = dict(base)
        mm["x"] = bf(seq[sl].reshape(T, H))
        mm["onehot"] = bf(oh)
        mm["ctxmask"] = bf(cm)
        mm["sel"] = bf(selm)
        in_maps.append(mm)
    return in_maps


def kernel(**inputs):
    nc = _get_nc()
    in_maps = _shard(inputs)
    res = run_bass_kernel_spmd(nc, in_maps, core_ids=list(range(NCORES)))
    ce = quad = ctx = 0.0
    for c in range(NCORES):
        o = np.asarray(res.results[c]["out"], np.float64).reshape(8)
        quad += float(o[0])
        ctx += float(o[1] + o[2])
        ce += float(o[3] + o[4] - o[5] - o[6])
    total = ce / (B * S) + ALPHA * (quad / B) + BETA * (ctx / (B * S))
    return np.float32(total)


# revision 7
# speedup vs baseline: 1.1119x; 1.0146x over previous
"""Trainium2 Bass kernel for nn_LLMCC_74414603370526 (loss_fn) — v2.

Data-parallel over batch: 16 sequences -> 8 cores x 2 sequences each; host
combines the scalar partial losses (the sanctioned all-reduce).

v2 design (instruction-count-minimized vs v1):
  - activations flow feature-major end-to-end; the only PE transposes are
    x (48) and the emb token-major copy needed by the quadruplet gather (48)
  - weights are pre-transposed on the HOST into DMA-friendly layouts; no
    on-device weight transposes
  - biases/residual/softmax-rowsum fold into matmuls (K=1 outer products,
    identity lhsT accumulate, ones column appended to v)
  - LayerNorm runs feature-major: per-token stats via ones/w-bar matmul
    column reductions; scale/shift applied via PE rank-1 outer products
    (g x rstd) and two vector ops per tile
  - CE in [9, T] label-major layout with host-built one-hot labels;
    context-loss label mask is host-built
  - SBUF slots reused via byte-equal tag chains:
    XTOK: x_tok -> emb_tok | XT: xT -> featT | QT: qT -> h1T
    KT: kT -> h2T | WQKT: wqkT -> w1T | WOT: woT -> w2T
"""

import numpy as np
import ml_dtypes

import concourse.bass as bass
import concourse.mybir as mybir
import concourse.tile as tile
from concourse import bacc
from concourse.bass_utils import run_bass_kernel_spmd
from concourse.masks import make_identity

FP32 = mybir.dt.float32
BF16 = mybir.dt.bfloat16
AF = mybir.ActivationFunctionType
ALU = mybir.AluOpType
AX = mybir.AxisListType

B, S, H = 16, 512, 768
NH, HD = 8, 96
NUM_LABELS = 9
MARGIN1, MARGIN2 = 1.0, 0.5
ALPHA, BETA = 0.2, 0.1
EPS = 1e-5

NCORES = 8
BL = B // NCORES          # 2 sequences per core
T = BL * S                # 1024 tokens per core
NT = T // 128             # 8 token tiles
KH = H // 128             # 6 feature tiles
D1, D2, D3 = 1024, 512, 256
KD1, KD2, KD3 = D1 // 128, D2 // 128, D3 // 128
ISQ = 1.0 / float(np.sqrt(HD))
BFD = ml_dtypes.bfloat16

# rowpack offsets (bf16 row: biases / row-vectors)
OF_BQK = 0                       # 1536
OF_BV = OF_BQK + 2 * H           # 776
OF_BO = OF_BV + 776              # 768
OF_B1 = OF_BO + H                # 1024
OF_B2 = OF_B1 + D1               # 512
OF_B3 = OF_B2 + D2               # 256
OF_BCLS = OF_B3 + D3             # 9
OF_S1B = OF_BCLS + NUM_LABELS    # 3
OF_G1 = OF_S1B + 3               # 1024
OF_G2 = OF_G1 + D1               # 512
OF_G3 = OF_G2 + D2               # 256
ROWN = OF_G3 + D3

# ppack (fp32 per-partition pack)
PP_BE1 = 0          # 8 cols
PP_BE2 = 8          # 4
PP_BE3 = 12         # 2
PP_MARG = 14        # margins in rows 0:4 of col 14
PPC = 15

# wbcol (bf16 per-partition colsum-weight pack)
WB1, WB2, WB3 = 0, 6, 14
WBC = 18

GELU_AS_COPY = False   # sim-debug only (CoreSim executor lacks Gelu numerics)

_CACHED = None


def _build(stop_after=None):
    nc = bacc.Bacc(None, target_bir_lowering=False)
    dd = {}

    def di(name, shape, dt=BF16):
        dd[name] = nc.dram_tensor(name, shape, dt, kind="ExternalInput")

    di("x", [T, H])
    di("wqk", [H, 2 * H])
    di("wv97", [H, 776])
    di("wopk", [HD * NH, H])       # rows = (d-within-head, head), cols f
    di("w1t", [H, D1])
    di("w2t", [D1, D2])
    di("w3t", [D2, D3])
    di("wct", [D3, NUM_LABELS])
    di("wrt", [H, NUM_LABELS])     # pre-scaled by 0.2 on host
    di("rowpack", [1, ROWN])
    di("ppack", [128, PPC], FP32)
    di("wbcol", [128, WBC])
    di("onehot", [NUM_LABELS, T])
    di("ctxmask", [1, T - 1])
    di("sel", [T, 6])
    di("combo", [6, 4], FP32)
    out_d = nc.dram_tensor("out", [1, 8], FP32, kind="ExternalOutput")

    with tile.TileContext(nc) as tc:
        with nc.allow_low_precision(reason="bf16 PE-transpose PSUM tiles"):
            _body(nc, tc, dd, out_d, stop_after)
    nc.finalize()
    return nc


def _body(nc, tc, dd, out_d, stop_after=None):
    const = tc.alloc_tile_pool(name="const", bufs=1)
    acts = tc.alloc_tile_pool(name="acts", bufs=1)

    def _stop(phase):
        return stop_after == phase

    def fin():
        nc.sync.dma_start(out=out_d[:, :], in_=partials)
        acts.release()
        const.release()


    # ---- constants / inputs ----
    ident = const.tile([128, 128], BF16)
    make_identity(nc, ident)
    ones_row = const.tile([1, 1024], BF16)
    nc.vector.memset(ones_row, 1.0)
    onescol = const.tile([128, 1], BF16)
    nc.vector.memset(onescol, 1.0)
    ones9 = const.tile([NUM_LABELS, 1], BF16)
    nc.vector.memset(ones9, 1.0)
    ones4f = const.tile([4, 1], FP32)
    nc.vector.memset(ones4f, 1.0)
    eps_t = const.tile([1, 1], FP32)
    nc.vector.memset(eps_t, EPS)
    partials = const.tile([1, 8], FP32)
    nc.vector.memset(partials, 0.0)

    x_tok = acts.tile([128, NT, H], BF16, tag="XTOK")
    nc.sync.dma_start(out=x_tok, in_=dd["x"].rearrange("(n p) h -> p n h", p=128))
    wqkT = acts.tile([128, KH, 1536], BF16, tag="WQKT")
    nc.sync.dma_start(out=wqkT, in_=dd["wqk"].rearrange("(f p) c -> p f c", p=128))
    rowpack = const.tile([1, ROWN], BF16)
    nc.sync.dma_start(out=rowpack, in_=dd["rowpack"][:, :])
    wv97T = const.tile([128, KH, 776], BF16)
    nc.gpsimd.dma_start(out=wv97T, in_=dd["wv97"].rearrange("(f p) c -> p f c", p=128))
    woT = acts.tile([HD, NH, H], BF16, tag="WOT")
    nc.gpsimd.dma_start(out=woT, in_=dd["wopk"].rearrange("(p h) f -> p h f", p=HD))
    w3T = const.tile([128, KD2, D3], BF16)
    nc.gpsimd.dma_start(out=w3T, in_=dd["w3t"].rearrange("(f p) c -> p f c", p=128))
    wcT = const.tile([128, KD3, NUM_LABELS], BF16)
    nc.gpsimd.dma_start(out=wcT, in_=dd["wct"].rearrange("(f p) c -> p f c", p=128))
    wrT = const.tile([128, KH, NUM_LABELS], BF16)
    nc.gpsimd.dma_start(out=wrT, in_=dd["wrt"].rearrange("(f p) c -> p f c", p=128))
    ppack = const.tile([128, PPC], FP32)
    nc.gpsimd.dma_start(out=ppack, in_=dd["ppack"][:, :])
    wbcol = const.tile([128, WBC], BF16)
    nc.gpsimd.dma_start(out=wbcol, in_=dd["wbcol"][:, :])
    onehot = const.tile([NUM_LABELS, T], BF16)
    nc.gpsimd.dma_start(out=onehot, in_=dd["onehot"][:, :])
    ctxmask = const.tile([1, T - 1], BF16)
    nc.gpsimd.dma_start(out=ctxmask, in_=dd["ctxmask"][:, :])
    sel = const.tile([128, NT, 6], BF16)
    nc.gpsimd.dma_start(out=sel, in_=dd["sel"].rearrange("(n p) c -> p n c", p=128))
    combo = const.tile([6, 4], FP32)
    nc.gpsimd.dma_start(out=combo, in_=dd["combo"][:, :])

    if _stop("load"):
        fin()
        return

    # ---- P1: x -> xT (feature-major; 4 transposes share one PSUM bank) ----
    xT = acts.tile([128, KH, T], BF16, tag="XT")
    with tc.tile_pool(name="ptp", bufs=4, space="PSUM") as ptp:
        i = 0
        for f in range(KH):
            for tq in range(0, NT, 4):
                ps = ptp.tile([128, 512], BF16, tag="tr", name="tr")
                for j in range(4):
                    nc.tensor.transpose(
                        ps[:, 128 * j:128 * (j + 1)],
                        x_tok[:, tq + j, 128 * f:128 * (f + 1)], ident)
                if i % 2 == 0:
                    nc.scalar.activation(
                        out=xT[:, f, 128 * tq:128 * (tq + 4)], in_=ps,
                        func=AF.Copy)
                else:
                    nc.vector.tensor_copy(
                        out=xT[:, f, 128 * tq:128 * (tq + 4)], in_=ps)
                i += 1

    if _stop("p1"):
        fin()
        return

    # ---- P2+P3: q, k, v ----
    qT = acts.tile([HD, NH, T], BF16, tag="QT")
    kT = acts.tile([HD, NH, T], BF16, tag="KT")
    v97 = acts.tile([128, NT, 776], BF16, tag="V97")
    with tc.tile_pool(name="pq", bufs=4, space="PSUM") as pq, \
         tc.tile_pool(name="pv", bufs=2, space="PSUM") as pv:
        i = 0
        for h in range(NH):
            for w in range(2):      # 0=q, 1=k
                dst = qT if w == 0 else kT
                off = H * w + HD * h
                for c in range(2):
                    ps = pq.tile([HD, 512], FP32, tag="pqk", name="pqk")
                    for f in range(KH):
                        nc.tensor.matmul(ps, wqkT[:, f, off:off + HD],
                                         xT[:, f, 512 * c:512 * (c + 1)],
                                         start=(f == 0), stop=False)
                    nc.tensor.matmul(ps,
                                     rowpack[:, OF_BQK + off:OF_BQK + off + HD],
                                     ones_row[:, 0:512], start=False, stop=True)
                    if i % 2 == 0:
                        nc.scalar.activation(out=dst[:, h, 512 * c:512 * (c + 1)],
                                             in_=ps, func=AF.Copy)
                    else:
                        nc.vector.tensor_copy(out=dst[:, h, 512 * c:512 * (c + 1)],
                                              in_=ps)
                    i += 1
        for t in range(NT):
            for g in range(2):
                ps = pv.tile([128, 388], FP32, tag="pv", name="pv")
                for f in range(KH):
                    nc.tensor.matmul(ps, xT[:, f, 128 * t:128 * (t + 1)],
                                     wv97T[:, f, 388 * g:388 * (g + 1)],
                                     start=(f == 0), stop=False)
                nc.tensor.matmul(ps, ones_row[0:1, 0:128],
                                 rowpack[:, OF_BV + 388 * g:OF_BV + 388 * (g + 1)],
                                 start=False, stop=True)
                if t % 2 == 0:
                    nc.scalar.activation(out=v97[:, t, 388 * g:388 * (g + 1)],
                                         in_=ps, func=AF.Copy)
                else:
                    nc.vector.tensor_copy(out=v97[:, t, 388 * g:388 * (g + 1)],
                                          in_=ps)

    if _stop("qkv"):
        fin()
        return

    # ---- P4: attention (exp without max-shift; rowsum via v ones column) ----
    aoT = acts.tile([HD, BL, NH, S], BF16, tag="AOT")
    with tc.tile_pool(name="pa", bufs=4, space="PSUM") as pa, \
         tc.tile_pool(name="pa2", bufs=2, space="PSUM") as pa2, \
         tc.tile_pool(name="pa3", bufs=2, space="PSUM") as pa3, \
         tc.tile_pool(name="wet", bufs=6) as wet, \
         tc.tile_pool(name="wra", bufs=2) as wra:
        def tail(sh):
            s, h, pao = sh
            rec = wra.tile([1, S], BF16, tag="rec", name="rec")
            nc.vector.reciprocal(out=rec, in_=pao[HD:HD + 1, :])
            prec = pa3.tile([HD, S], FP32, tag="prec", name="prec")
            nc.tensor.matmul(prec, ones_row[0:1, 0:HD], rec, start=True,
                             stop=True)
            aou = wra.tile([HD, S], BF16, tag="aou", name="aou")
            nc.scalar.activation(out=aou, in_=pao[0:HD, :], func=AF.Copy)
            nc.vector.tensor_mul(out=aoT[:, s, h, :], in0=aou, in1=prec)

        pending = None
        for s in range(BL):
            for h in range(NH):
                et = []
                for kt in range(4):
                    psc = pa.tile([128, S], FP32, tag="psc", name="psc")
                    nc.tensor.matmul(
                        psc, kT[:, h, S * s + 128 * kt:S * s + 128 * (kt + 1)],
                        qT[:, h, S * s:S * (s + 1)], start=True, stop=True)
                    e = wet.tile([128, S], BF16, tag="et", name="et")
                    nc.scalar.activation(out=e, in_=psc, func=AF.Exp, scale=ISQ)
                    et.append(e)
                pao = pa2.tile([HD + 1, S], FP32, tag="pao", name="pao")
                g, hh = h // 4, h % 4
                voff = 388 * g + 97 * hh
                for kt in range(4):
                    nc.tensor.matmul(pao, v97[:, 4 * s + kt, voff:voff + 97],
                                     et[kt], start=(kt == 0), stop=(kt == 3))
                if pending is not None:
                    tail(pending)
                pending = (s, h, pao)
        tail(pending)

    if _stop("attn"):
        fin()
        return

    # ---- P5: w_o + bias + residual -> embT ----
    embT = acts.tile([128, KH, T], BF16, tag="EMBT")
    with tc.tile_pool(name="pw", bufs=3, space="PSUM") as pw:
        for f in range(KH):
            for s in range(BL):
                ps = pw.tile([128, S], FP32, tag="pwo", name="pwo")
                for h in range(NH):
                    nc.tensor.matmul(ps, woT[:, h, 128 * f:128 * (f + 1)],
                                     aoT[:, s, h, :], start=(h == 0), stop=False)
                nc.tensor.matmul(ps,
                                 rowpack[:, OF_BO + 128 * f:OF_BO + 128 * (f + 1)],
                                 ones_row[:, 0:S], start=False, stop=False)
                nc.tensor.matmul(ps, ident, xT[:, f, S * s:S * (s + 1)],
                                 start=False, stop=True)
                if (2 * f + s) % 2 == 0:
                    nc.scalar.activation(out=embT[:, f, S * s:S * (s + 1)],
                                         in_=ps, func=AF.Copy)
                else:
                    nc.vector.tensor_copy(out=embT[:, f, S * s:S * (s + 1)],
                                          in_=ps)

    # late weight loads into freed slots (scalar DMA queue, off the hot path)
    w1T = acts.tile([128, KH, 1536], BF16, tag="WQKT")
    nc.scalar.dma_start(out=w1T[:, :, 0:D1],
                        in_=dd["w1t"].rearrange("(f p) c -> p f c", p=128))
    w2T = acts.tile([128, NH, H], BF16, tag="WOT")
    nc.scalar.dma_start(out=w2T[:, :, 0:D2],
                        in_=dd["w2t"].rearrange("(f p) c -> p f c", p=128))

    if _stop("wo"):
        fin()
        return

    # ---- P6: context loss (chunked) ----
    with tc.tile_pool(name="pctx", bufs=1, space="PSUM") as pcx, \
         tc.tile_pool(name="wctx", bufs=2) as wctx:
        chunks = [(0, 512), (512, T - 1)]
        for ci, (lo, hi) in enumerate(chunks):
            n = hi - lo
            pc = pcx.tile([1, 512], FP32, tag=f"pc{ci}", name="pc")
            for f in range(KH):
                d_ = wctx.tile([128, 512], BF16, tag="ctxd", name="ctxd")
                nc.vector.tensor_sub(out=d_[:, 0:n], in0=embT[:, f, lo:hi],
                                     in1=embT[:, f, lo + 1:hi + 1])
                dsq = wctx.tile([128, 512], BF16, tag="ctxsq", name="ctxsq")
                nc.scalar.activation(out=dsq[:, 0:n], in_=d_[:, 0:n],
                                     func=AF.Square)
                nc.tensor.matmul(pc[:, 0:n], onescol, dsq[:, 0:n],
                                 start=(f == 0), stop=(f == KH - 1))
            nrm = wctx.tile([1, 512], FP32, tag="nrm", name="nrm")
            nc.scalar.activation(out=nrm[:, 0:n], in_=pc[:, 0:n], func=AF.Sqrt)
            msk = wctx.tile([1, 512], FP32, tag="msk", name="msk")
            nc.vector.tensor_mul(out=msk[:, 0:n], in0=nrm[:, 0:n],
                                 in1=ctxmask[:, lo:hi])
            nc.vector.reduce_sum(out=partials[0:1, 1 + ci:2 + ci],
                                 in_=msk[:, 0:n], axis=AX.X)

    if _stop("ctx"):
        fin()
        return

    # ---- P7: quadruplet loss (emb -> token-major, host-built selectors) ----
    emb_tok = acts.tile([128, NT, H], BF16, tag="XTOK")
    with tc.tile_pool(name="ptp2", bufs=4, space="PSUM") as ptp2:
        i = 0
        for t in range(NT):
            for fq, fw in ((0, 4), (4, 2)):
                ps = ptp2.tile([128, 512], BF16, tag="tr2", name="tr2")
                for j in range(fw):
                    nc.tensor.transpose(
                        ps[:, 128 * j:128 * (j + 1)],
                        embT[:, fq + j, 128 * t:128 * (t + 1)], ident)
                w = 128 * fw
                if i % 2 == 0:
                    nc.scalar.activation(
                        out=emb_tok[:, t, 128 * fq:128 * fq + w],
                        in_=ps[:, 0:w], func=AF.Copy)
                else:
                    nc.vector.tensor_copy(
                        out=emb_tok[:, t, 128 * fq:128 * fq + w],
                        in_=ps[:, 0:w])
                i += 1
    with tc.tile_pool(name="pqd", bufs=1, space="PSUM") as pqd_pool, \
         tc.tile_pool(name="wqd", bufs=1) as wqd:
        pq1 = pqd_pool.tile([6, 512], FP32, tag="pq1", name="pq1")
        pq2 = pqd_pool.tile([6, H - 512], FP32, tag="pq2", name="pq2")
        for t in range(NT):
            nc.tensor.matmul(pq1, sel[:, t, :], emb_tok[:, t, 0:512],
                             start=(t == 0), stop=(t == NT - 1))
        for t in range(NT):
            nc.tensor.matmul(pq2, sel[:, t, :], emb_tok[:, t, 512:H],
                             start=(t == 0), stop=(t == NT - 1))
        dq1 = wqd.tile([6, 512], FP32, tag="dq1", name="dq1")
        nc.scalar.activation(out=dq1, in_=pq1, func=AF.Square)
        dq2 = wqd.tile([6, H - 512], FP32, tag="dq2", name="dq2")
        nc.scalar.activation(out=dq2, in_=pq2, func=AF.Square)
        d1c = wqd.tile([6, 1], FP32, tag="d1c", name="d1c")
        nc.vector.reduce_sum(out=d1c, in_=dq1, axis=AX.X)
        d2c = wqd.tile([6, 1], FP32, tag="d2c", name="d2c")
        nc.vector.reduce_sum(out=d2c, in_=dq2, axis=AX.X)
        dist = wqd.tile([6, 1], FP32, tag="dist", name="dist")
        nc.vector.tensor_add(out=dist, in0=d1c, in1=d2c)
        pqd = pqd_pool.tile([4, 1], FP32, tag="pqd", name="pqd")
        nc.tensor.matmul(pqd, combo, dist, start=True, stop=True)
        qrelu = wqd.tile([4, 1], FP32, tag="qrelu", name="qrelu")
        nc.scalar.activation(out=qrelu, in_=pqd, func=AF.Relu,
                             bias=ppack[0:4, PP_MARG:PP_MARG + 1])
        psq = pqd_pool.tile([1, 1], FP32, tag="psq", name="psq")
        nc.tensor.matmul(psq, qrelu, ones4f, start=True, stop=True)
        nc.vector.tensor_copy(out=partials[0:1, 0:1], in_=psq)


    # ---- P8: MLP layers (feature-major LayerNorm) ----
    def mlp_layer(li, inT, kd, wT, kdo, b_off, g_off, be_off, wb_off, s1_i,
                  gelu, outT):
        dout = 128 * kdo
        inv_d = 1.0 / float(dout)
        with tc.tile_pool(name=f"pm{li}", bufs=2, space="PSUM") as pm, \
             tc.tile_pool(name=f"pp1_{li}", bufs=1, space="PSUM") as pp1, \
             tc.tile_pool(name=f"pp2_{li}", bufs=1, space="PSUM") as pp2, \
             tc.tile_pool(name=f"pb{li}", bufs=2, space="PSUM") as pb, \
             tc.tile_pool(name=f"wk{li}", bufs=1) as wk, \
             tc.tile_pool(name=f"ap{li}", bufs=2) as ap, \
             tc.tile_pool(name=f"sq{li}", bufs=3) as sqp, \
             tc.tile_pool(name=f"zb{li}", bufs=kdo + 1) as zbp:
            for c in range(2):
                cs = slice(512 * c, 512 * (c + 1))
                ps1 = pp1.tile([1, 512], FP32, tag="s1", name="s1")
                for f in range(kd):
                    nc.tensor.matmul(ps1, wbcol[:, wb_off + f:wb_off + f + 1],
                                     inT[:, f, cs], start=(f == 0), stop=False)
                nc.tensor.matmul(ps1, rowpack[:, OF_S1B + s1_i:OF_S1B + s1_i + 1],
                                 ones_row[:, 0:512], start=False, stop=True)
                ps2 = pp2.tile([1, 512], FP32, tag="s2", name="s2")
                zs, sqs = [], []
                for po in range(kdo):
                    ps = pm.tile([128, 512], FP32, tag="z", name="z")
                    for f in range(kd):
                        nc.tensor.matmul(ps, wT[:, f, 128 * po:128 * (po + 1)],
                                         inT[:, f, cs], start=(f == 0),
                                         stop=False)
                    nc.tensor.matmul(
                        ps, rowpack[:, b_off + 128 * po:b_off + 128 * (po + 1)],
                        ones_row[:, 0:512], start=False, stop=True)
                    zb = zbp.tile([128, 512], BF16, tag="zb", name="zb")
                    nc.vector.tensor_copy(out=zb, in_=ps)
                    sq = sqp.tile([128, 512], BF16, tag="sq", name="sq")
                    nc.scalar.activation(out=sq, in_=ps, func=AF.Square)
                    zs.append(zb)
                    sqs.append(sq)
                    if po >= 1:
                        nc.tensor.matmul(ps2, onescol, sqs[po - 1],
                                         start=(po == 1), stop=False)
                nc.tensor.matmul(ps2, onescol, sqs[kdo - 1],
                                 start=(kdo == 1), stop=True)
                mu = wk.tile([1, 512], FP32, tag="mu", name="mu")
                nc.vector.tensor_scalar(out=mu, in0=ps1, scalar1=inv_d,
                                        scalar2=None, op0=ALU.mult)
                m2 = wk.tile([1, 512], FP32, tag="m2", name="m2")
                nc.vector.tensor_scalar(out=m2, in0=ps2, scalar1=inv_d,
                                        scalar2=None, op0=ALU.mult)
                musq = wk.tile([1, 512], FP32, tag="musq", name="musq")
                nc.vector.tensor_mul(out=musq, in0=mu, in1=mu)
                var = wk.tile([1, 512], FP32, tag="var", name="var")
                nc.vector.tensor_sub(out=var, in0=m2, in1=musq)
                sd = wk.tile([1, 512], FP32, tag="sd", name="sd")
                nc.scalar.activation(out=sd, in_=var, func=AF.Sqrt, bias=eps_t)
                r_bf = wk.tile([1, 512], BF16, tag="rbf", name="rbf")
                nc.vector.reciprocal(out=r_bf, in_=sd)
                mur = wk.tile([1, 512], BF16, tag="mur", name="mur")
                nc.vector.tensor_mul(out=mur, in0=mu, in1=r_bf)
                for po in range(kdo):
                    gsl = rowpack[:, g_off + 128 * po:g_off + 128 * (po + 1)]
                    pRg = pb.tile([128, 512], FP32, tag="rg", name="rg")
                    nc.tensor.matmul(pRg, gsl, r_bf, start=True, stop=True)
                    pMg = pb.tile([128, 512], FP32, tag="mg", name="mg")
                    nc.tensor.matmul(pMg, gsl, mur, start=True, stop=True)
                    t1 = ap.tile([128, 512], BF16, tag="t1", name="t1")
                    nc.vector.tensor_mul(out=t1, in0=zs[po], in1=pRg)
                    be_sl = ppack[:, be_off + po:be_off + po + 1]
                    if gelu:
                        t2 = ap.tile([128, 512], BF16, tag="t2", name="t2")
                        nc.vector.scalar_tensor_tensor(
                            out=t2, in0=t1, scalar=be_sl, in1=pMg,
                            op0=ALU.add, op1=ALU.subtract)
                        nc.scalar.activation(
                            out=outT[:, po, cs], in_=t2,
                            func=(AF.Copy if GELU_AS_COPY else AF.Gelu))
                    else:
                        nc.vector.scalar_tensor_tensor(
                            out=outT[:, po, cs], in0=t1, scalar=be_sl, in1=pMg,
                            op0=ALU.add, op1=ALU.subtract)

    h1T = acts.tile([128, KD1, T], BF16, tag="QT")
    mlp_layer(0, embT, KH, w1T, KD1, OF_B1, OF_G1, PP_BE1, WB1, 0, True, h1T)
    if _stop("mlp1"):
        fin()
        return
    h2T = acts.tile([128, KD1, T], BF16, tag="KT")   # planes 0:KD2 used
    mlp_layer(1, h1T, KD1, w2T, KD2, OF_B2, OF_G2, PP_BE2, WB2, 1, True, h2T)
    if _stop("mlp2"):
        fin()
        return
    featT = acts.tile([128, KH, T], BF16, tag="XT")  # planes 0:KD3 used
    mlp_layer(2, h2T, KD2, w3T, KD3, OF_B3, OF_G3, PP_BE3, WB3, 2, False, featT)

    if _stop("mlp3"):
        fin()
        return

    # ---- P9: CE in [9, T] layout ----
    with tc.tile_pool(name="pce", bufs=2, space="PSUM") as pce, \
         tc.tile_pool(name="pse", bufs=2, space="PSUM") as pse, \
         tc.tile_pool(name="ppk", bufs=2, space="PSUM") as ppk, \
         tc.tile_pool(name="wce", bufs=2) as wce:
        for c in range(2):
            cs = slice(512 * c, 512 * (c + 1))
            pl = pce.tile([NUM_LABELS, 512], FP32, tag="pl", name="pl")
            for d3 in range(KD3):
                nc.tensor.matmul(pl, wcT[:, d3, :], featT[:, d3, cs],
                                 start=(d3 == 0), stop=False)
            for f in range(KH):
                nc.tensor.matmul(pl, wrT[:, f, :], embT[:, f, cs],
                                 start=False, stop=False)
            nc.tensor.matmul(pl, rowpack[:, OF_BCLS:OF_BCLS + NUM_LABELS],
                             ones_row[:, 0:512], start=False, stop=True)
            e9 = wce.tile([NUM_LABELS, 512], BF16, tag="e9", name="e9")
            nc.scalar.activation(out=e9, in_=pl, func=AF.Exp)
            ps_se = pse.tile([1, 512], FP32, tag="se", name="se")
            nc.tensor.matmul(ps_se, ones9, e9, start=True, stop=True)
            lns = wce.tile([1, 512], FP32, tag="lns", name="lns")
            nc.scalar.activation(out=lns, in_=ps_se, func=AF.Ln)
            nc.vector.reduce_sum(out=partials[0:1, 3 + c:4 + c], in_=lns,
                                 axis=AX.X)
            po9 = wce.tile([NUM_LABELS, 512], FP32, tag="po9", name="po9")
            nc.vector.tensor_mul(out=po9, in0=onehot[:, cs], in1=pl)
            pr9 = wce.tile([NUM_LABELS, 1], BF16, tag="pr9", name="pr9")
            nc.vector.reduce_sum(out=pr9, in_=po9, axis=AX.X)
            ps_pk = ppk.tile([1, 1], FP32, tag="pk", name="pk")
            nc.tensor.matmul(ps_pk, pr9, ones9, start=True, stop=True)
            nc.vector.tensor_copy(out=partials[0:1, 5 + c:6 + c], in_=ps_pk)

    if _stop("ce"):
        fin()
        return

    fin()


def _get_nc():
    global _CACHED
    if _CACHED is None:
        _CACHED = _build()
    return _CACHED


def _shard(inputs):
    f32 = lambda a: np.asarray(a, np.float32)
    bf = lambda a: np.ascontiguousarray(np.asarray(a, np.float32)).astype(BFD)

    seq = f32(inputs["sequence_output"])
    labels = np.asarray(inputs["labels"]).astype(np.int64)
    a_p = np.asarray(inputs["anchor_positions"]).astype(np.int64)
    p_p = np.asarray(inputs["positive_positions"]).astype(np.int64)
    n1_p = np.asarray(inputs["negative1_positions"]).astype(np.int64)
    n2_p = np.asarray(inputs["negative2_positions"]).astype(np.int64)

    w_qkv, b_qkv = f32(inputs["w_qkv"]), f32(inputs["b_qkv"])
    w_o, b_o = f32(inputs["w_o"]), f32(inputs["b_o"])
    w1, b1 = f32(inputs["w1"]), f32(inputs["b1"])
    g1, be1 = f32(inputs["g1"]), f32(inputs["be1"])
    w2, b2 = f32(inputs["w2"]), f32(inputs["b2"])
    g2, be2 = f32(inputs["g2"]), f32(inputs["be2"])
    w3, b3 = f32(inputs["w3"]), f32(inputs["b3"])
    g3, be3 = f32(inputs["g3"]), f32(inputs["be3"])
    wc, bc = f32(inputs["wc"]), f32(inputs["bc"])
    wr, br = f32(inputs["wr"]), f32(inputs["br"])

    wq, wk, wv = w_qkv[0:H], w_qkv[H:2 * H], w_qkv[2 * H:3 * H]
    bq, bk, bv = b_qkv[0:H], b_qkv[H:2 * H], b_qkv[2 * H:3 * H]

    wqk = np.concatenate([wq.T, wk.T], axis=1)               # [H, 2H]
    wv97 = np.zeros((H, NH, 97), np.float32)
    wv97[:, :, :96] = wv.T.reshape(H, NH, HD)
    wv97 = wv97.reshape(H, 776)
    wopk = np.ascontiguousarray(
        w_o.T.reshape(NH, HD, H).transpose(1, 0, 2)).reshape(HD * NH, H)

    rowpack = np.zeros((1, ROWN), np.float32)
    rowpack[0, OF_BQK:OF_BQK + H] = bq
    rowpack[0, OF_BQK + H:OF_BQK + 2 * H] = bk
    bv97 = np.zeros((NH, 97), np.float32)
    bv97[:, :96] = bv.reshape(NH, HD)
    bv97[:, 96] = 1.0
    rowpack[0, OF_BV:OF_BV + 776] = bv97.reshape(776)
    rowpack[0, OF_BO:OF_BO + H] = b_o
    rowpack[0, OF_B1:OF_B1 + D1] = b1
    rowpack[0, OF_B2:OF_B2 + D2] = b2
    rowpack[0, OF_B3:OF_B3 + D3] = b3
    rowpack[0, OF_BCLS:OF_BCLS + NUM_LABELS] = bc + ALPHA * br
    rowpack[0, OF_S1B + 0] = b1.sum()
    rowpack[0, OF_S1B + 1] = b2.sum()
    rowpack[0, OF_S1B + 2] = b3.sum()
    rowpack[0, OF_G1:OF_G1 + D1] = g1
    rowpack[0, OF_G2:OF_G2 + D2] = g2
    rowpack[0, OF_G3:OF_G3 + D3] = g3

    ppack = np.zeros((128, PPC), np.float32)
    ppack[:, PP_BE1:PP_BE1 + KD1] = be1.reshape(KD1, 128).T
    ppack[:, PP_BE2:PP_BE2 + KD2] = be2.reshape(KD2, 128).T
    ppack[:, PP_BE3:PP_BE3 + KD3] = be3.reshape(KD3, 128).T
    ppack[0:4, PP_MARG] = [MARGIN1, MARGIN2, MARGIN1, MARGIN2]

    wbcol = np.zeros((128, WBC), np.float32)
    wbcol[:, WB1:WB1 + KH] = w1.sum(axis=0).reshape(KH, 128).T
    wbcol[:, WB2:WB2 + KD1] = w2.sum(axis=0).reshape(KD1, 128).T
    wbcol[:, WB3:WB3 + KD2] = w3.sum(axis=0).reshape(KD2, 128).T

    combo = np.zeros((6, 4), np.float32)
    for k, (ipd, ind) in enumerate([(0, 1), (0, 2), (3, 4), (3, 5)]):
        combo[ipd, k] = 1.0
        combo[ind, k] = -1.0

    base = {
        "wqk": bf(wqk), "wv97": bf(wv97), "wopk": bf(wopk),
        "w1t": bf(w1.T), "w2t": bf(w2.T), "w3t": bf(w3.T),
        "wct": bf(wc.T), "wrt": bf(ALPHA * wr.T),
        "rowpack": bf(rowpack), "ppack": np.ascontiguousarray(ppack),
        "wbcol": bf(wbcol), "combo": np.ascontiguousarray(combo),
    }

    in_maps = []
    for c in range(NCORES):
        sl = slice(BL * c, BL * (c + 1))
        lab = labels[sl]                       # [BL, S]
        oh = np.zeros((NUM_LABELS, T), np.float32)
        flat = lab.reshape(T)
        oh[flat, np.arange(T)] = 1.0
        m = (lab[:, :-1] != 0) & (lab[:, :-1] == lab[:, 1:])   # [BL, S-1]
        cm = np.zeros((1, T - 1), np.float32)
        for s in range(BL):
            cm[0, S * s:S * s + S - 1] = m[s]
        selm = np.zeros((T, 6), np.float32)
        for s in range(BL):
            b = BL * c + s
            av = int(a_p[b]) + S * s
            for j, pos in enumerate((p_p[b], n1_p[b], n2_p[b])):
                col = 3 * s + j
                selm[av, col] += 1.0
                selm[int(pos) + S * s, col] -= 1.0
        mm = dict(base)
        mm["x"] = bf(seq[sl].reshape(T, H))
        mm["onehot"] = bf(oh)
        mm["ctxmask"] = bf(cm)
        mm["sel"] = bf(selm)
        in_maps.append(mm)
    return in_maps


def kernel(**inputs):
    nc = _get_nc()
    in_maps = _shard(inputs)
    res = run_bass_kernel_spmd(nc, in_maps, core_ids=list(range(NCORES)))
    ce = quad = ctx = 0.0
    for c in range(NCORES):
        o = np.asarray(res.results[c]["out"], np.float64).reshape(8)
        quad += float(o[0])
        ctx += float(o[1] + o[2])
        ce += float(o[3] + o[4] - o[5] - o[6])
    total = ce / (B * S) + ALPHA * (quad / B) + BETA * (ctx / (B * S))
    return np.float32(total)


# revision 8
# speedup vs baseline: 1.1161x; 1.0038x over previous
"""Trainium2 Bass kernel for nn_LLMCC_74414603370526 (loss_fn) — v2.

Data-parallel over batch: 16 sequences -> 8 cores x 2 sequences each; host
combines the scalar partial losses (the sanctioned all-reduce).

v2 design (instruction-count-minimized vs v1):
  - activations flow feature-major end-to-end (x arrives host-transposed);
    the only PE transposes are the emb token-major copy for the quadruplet
    gather (48, packed 4-to-a-bank)
  - weights are pre-transposed on the HOST into DMA-friendly layouts; no
    on-device weight transposes
  - biases/residual/softmax-rowsum fold into matmuls (K=1 outer products,
    identity lhsT accumulate, ones column appended to v)
  - LayerNorm runs feature-major: per-token stats via ones/w-bar matmul
    column reductions; scale/shift applied via PE rank-1 outer products
    (g x rstd) and two vector ops per tile
  - CE in [9, T] label-major layout with host-built one-hot labels;
    context-loss label mask is host-built
  - SBUF slots reused via byte-equal tag chains:
    XTOK: x_tok -> emb_tok | XT: xT -> featT | QT: qT -> h1T
    KT: kT -> h2T | WQKT: wqkT -> w1T | WOT: woT -> w2T
"""

import numpy as np
import ml_dtypes

import concourse.bass as bass
import concourse.mybir as mybir
import concourse.tile as tile
from concourse import bacc
from concourse.bass_utils import run_bass_kernel_spmd
from concourse.masks import make_identity

FP32 = mybir.dt.float32
BF16 = mybir.dt.bfloat16
AF = mybir.ActivationFunctionType
ALU = mybir.AluOpType
AX = mybir.AxisListType

B, S, H = 16, 512, 768
NH, HD = 8, 96
NUM_LABELS = 9
MARGIN1, MARGIN2 = 1.0, 0.5
ALPHA, BETA = 0.2, 0.1
EPS = 1e-5

NCORES = 8
BL = B // NCORES          # 2 sequences per core
T = BL * S                # 1024 tokens per core
NT = T // 128             # 8 token tiles
KH = H // 128             # 6 feature tiles
D1, D2, D3 = 1024, 512, 256
KD1, KD2, KD3 = D1 // 128, D2 // 128, D3 // 128
ISQ = 1.0 / float(np.sqrt(HD))
BFD = ml_dtypes.bfloat16

# rowpack offsets (bf16 row: biases / row-vectors)
OF_BQK = 0                       # 1536
OF_BV = OF_BQK + 2 * H           # 776
OF_BO = OF_BV + 776              # 768
OF_B1 = OF_BO + H                # 1024
OF_B2 = OF_B1 + D1               # 512
OF_B3 = OF_B2 + D2               # 256
OF_BCLS = OF_B3 + D3             # 9
OF_S1B = OF_BCLS + NUM_LABELS    # 3
OF_G1 = OF_S1B + 3               # 1024
OF_G2 = OF_G1 + D1               # 512
OF_G3 = OF_G2 + D2               # 256
ROWN = OF_G3 + D3

# ppack (fp32 per-partition pack)
PP_BE1 = 0          # 8 cols
PP_BE2 = 8          # 4
PP_BE3 = 12         # 2
PP_MARG = 14        # margins in rows 0:4 of col 14
PPC = 15

# wbcol (bf16 per-partition colsum-weight pack)
WB1, WB2, WB3 = 0, 6, 14
WBC = 18

GELU_AS_COPY = False   # sim-debug only (CoreSim executor lacks Gelu numerics)

_CACHED = None


def _build(stop_after=None):
    nc = bacc.Bacc(None, target_bir_lowering=False)
    dd = {}

    def di(name, shape, dt=BF16):
        dd[name] = nc.dram_tensor(name, shape, dt, kind="ExternalInput")

    di("xt", [H, T])
    di("wqk", [H, 2 * H])
    di("wv97", [H, 776])
    di("wopk", [HD * NH, H])       # rows = (d-within-head, head), cols f
    di("w1t", [H, D1])
    di("w2t", [D1, D2])
    di("w3t", [D2, D3])
    di("wct", [D3, NUM_LABELS])
    di("wrt", [H, NUM_LABELS])     # pre-scaled by 0.2 on host
    di("rowpack", [1, ROWN])
    di("ppack", [128, PPC], FP32)
    di("wbcol", [128, WBC])
    di("onehot", [NUM_LABELS, T])
    di("ctxmask", [1, T - 1])
    di("sel", [T, 6])
    di("combo", [6, 4], FP32)
    out_d = nc.dram_tensor("out", [1, 8], FP32, kind="ExternalOutput")

    with tile.TileContext(nc) as tc:
        with nc.allow_low_precision(reason="bf16 PE-transpose PSUM tiles"):
            _body(nc, tc, dd, out_d, stop_after)
    nc.finalize()
    return nc


def _body(nc, tc, dd, out_d, stop_after=None):
    const = tc.alloc_tile_pool(name="const", bufs=1)
    acts = tc.alloc_tile_pool(name="acts", bufs=1)

    def _stop(phase):
        return stop_after == phase

    def fin():
        nc.sync.dma_start(out=out_d[:, :], in_=partials)
        acts.release()
        const.release()


    # ---- constants / inputs ----
    ident = const.tile([128, 128], BF16)
    make_identity(nc, ident)
    ones_row = const.tile([1, 1024], BF16)
    nc.vector.memset(ones_row, 1.0)
    onescol = const.tile([128, 1], BF16)
    nc.vector.memset(onescol, 1.0)
    ones9 = const.tile([NUM_LABELS, 1], BF16)
    nc.vector.memset(ones9, 1.0)
    ones4f = const.tile([4, 1], FP32)
    nc.vector.memset(ones4f, 1.0)
    eps_t = const.tile([1, 1], FP32)
    nc.vector.memset(eps_t, EPS)
    partials = const.tile([1, 8], FP32)
    nc.vector.memset(partials, 0.0)

    xT = acts.tile([128, KH, T], BF16, tag="XT")
    nc.sync.dma_start(out=xT, in_=dd["xt"].rearrange("(f p) t -> p f t", p=128))
    wqkT = acts.tile([128, KH, 1536], BF16, tag="WQKT")
    nc.sync.dma_start(out=wqkT, in_=dd["wqk"].rearrange("(f p) c -> p f c", p=128))
    rowpack = const.tile([1, ROWN], BF16)
    nc.sync.dma_start(out=rowpack, in_=dd["rowpack"][:, :])
    wv97T = const.tile([128, KH, 776], BF16)
    nc.gpsimd.dma_start(out=wv97T, in_=dd["wv97"].rearrange("(f p) c -> p f c", p=128))
    woT = acts.tile([HD, NH, H], BF16, tag="WOT")
    nc.gpsimd.dma_start(out=woT, in_=dd["wopk"].rearrange("(p h) f -> p h f", p=HD))
    w3T = const.tile([128, KD2, D3], BF16)
    nc.gpsimd.dma_start(out=w3T, in_=dd["w3t"].rearrange("(f p) c -> p f c", p=128))
    wcT = const.tile([128, KD3, NUM_LABELS], BF16)
    nc.gpsimd.dma_start(out=wcT, in_=dd["wct"].rearrange("(f p) c -> p f c", p=128))
    wrT = const.tile([128, KH, NUM_LABELS], BF16)
    nc.gpsimd.dma_start(out=wrT, in_=dd["wrt"].rearrange("(f p) c -> p f c", p=128))
    ppack = const.tile([128, PPC], FP32)
    nc.gpsimd.dma_start(out=ppack, in_=dd["ppack"][:, :])
    wbcol = const.tile([128, WBC], BF16)
    nc.gpsimd.dma_start(out=wbcol, in_=dd["wbcol"][:, :])
    onehot = const.tile([NUM_LABELS, T], BF16)
    nc.gpsimd.dma_start(out=onehot, in_=dd["onehot"][:, :])
    ctxmask = const.tile([1, T - 1], BF16)
    nc.gpsimd.dma_start(out=ctxmask, in_=dd["ctxmask"][:, :])
    sel = const.tile([128, NT, 6], BF16)
    nc.gpsimd.dma_start(out=sel, in_=dd["sel"].rearrange("(n p) c -> p n c", p=128))
    combo = const.tile([6, 4], FP32)
    nc.gpsimd.dma_start(out=combo, in_=dd["combo"][:, :])

    if _stop("load"):
        fin()
        return

    if _stop("p1"):
        fin()
        return

    # ---- P2+P3: q, k, v ----
    qT = acts.tile([HD, NH, T], BF16, tag="QT")
    kT = acts.tile([HD, NH, T], BF16, tag="KT")
    v97 = acts.tile([128, NT, 776], BF16, tag="V97")
    with tc.tile_pool(name="pq", bufs=4, space="PSUM") as pq, \
         tc.tile_pool(name="pv", bufs=2, space="PSUM") as pv:
        i = 0
        for h in range(NH):
            for w in range(2):      # 0=q, 1=k
                dst = qT if w == 0 else kT
                off = H * w + HD * h
                for c in range(2):
                    ps = pq.tile([HD, 512], FP32, tag="pqk", name="pqk")
                    for f in range(KH):
                        nc.tensor.matmul(ps, wqkT[:, f, off:off + HD],
                                         xT[:, f, 512 * c:512 * (c + 1)],
                                         start=(f == 0), stop=False)
                    nc.tensor.matmul(ps,
                                     rowpack[:, OF_BQK + off:OF_BQK + off + HD],
                                     ones_row[:, 0:512], start=False, stop=True)
                    if i % 2 == 0:
                        nc.scalar.activation(out=dst[:, h, 512 * c:512 * (c + 1)],
                                             in_=ps, func=AF.Copy)
                    else:
                        nc.vector.tensor_copy(out=dst[:, h, 512 * c:512 * (c + 1)],
                                              in_=ps)
                    i += 1
        for t in range(NT):
            for g in range(2):
                ps = pv.tile([128, 388], FP32, tag="pv", name="pv")
                for f in range(KH):
                    nc.tensor.matmul(ps, xT[:, f, 128 * t:128 * (t + 1)],
                                     wv97T[:, f, 388 * g:388 * (g + 1)],
                                     start=(f == 0), stop=False)
                nc.tensor.matmul(ps, ones_row[0:1, 0:128],
                                 rowpack[:, OF_BV + 388 * g:OF_BV + 388 * (g + 1)],
                                 start=False, stop=True)
                if t % 2 == 0:
                    nc.scalar.activation(out=v97[:, t, 388 * g:388 * (g + 1)],
                                         in_=ps, func=AF.Copy)
                else:
                    nc.vector.tensor_copy(out=v97[:, t, 388 * g:388 * (g + 1)],
                                          in_=ps)

    if _stop("qkv"):
        fin()
        return

    # ---- P4: attention (exp without max-shift; rowsum via v ones column) ----
    aoT = acts.tile([HD, BL, NH, S], BF16, tag="AOT")
    with tc.tile_pool(name="pa", bufs=4, space="PSUM") as pa, \
         tc.tile_pool(name="pa2", bufs=2, space="PSUM") as pa2, \
         tc.tile_pool(name="pa3", bufs=2, space="PSUM") as pa3, \
         tc.tile_pool(name="wet", bufs=6) as wet, \
         tc.tile_pool(name="wra", bufs=2) as wra:
        def tail(sh):
            s, h, pao = sh
            rec = wra.tile([1, S], BF16, tag="rec", name="rec")
            nc.vector.reciprocal(out=rec, in_=pao[HD:HD + 1, :])
            prec = pa3.tile([HD, S], FP32, tag="prec", name="prec")
            nc.tensor.matmul(prec, ones_row[0:1, 0:HD], rec, start=True,
                             stop=True)
            aou = wra.tile([HD, S], BF16, tag="aou", name="aou")
            nc.scalar.activation(out=aou, in_=pao[0:HD, :], func=AF.Copy)
            nc.vector.tensor_mul(out=aoT[:, s, h, :], in0=aou, in1=prec)

        pending = None
        for s in range(BL):
            for h in range(NH):
                et = []
                for kt in range(4):
                    psc = pa.tile([128, S], FP32, tag="psc", name="psc")
                    nc.tensor.matmul(
                        psc, kT[:, h, S * s + 128 * kt:S * s + 128 * (kt + 1)],
                        qT[:, h, S * s:S * (s + 1)], start=True, stop=True)
                    e = wet.tile([128, S], BF16, tag="et", name="et")
                    nc.scalar.activation(out=e, in_=psc, func=AF.Exp, scale=ISQ)
                    et.append(e)
                pao = pa2.tile([HD + 1, S], FP32, tag="pao", name="pao")
                g, hh = h // 4, h % 4
                voff = 388 * g + 97 * hh
                for kt in range(4):
                    nc.tensor.matmul(pao, v97[:, 4 * s + kt, voff:voff + 97],
                                     et[kt], start=(kt == 0), stop=(kt == 3))
                if pending is not None:
                    tail(pending)
                pending = (s, h, pao)
        tail(pending)

    if _stop("attn"):
        fin()
        return

    # ---- P5: w_o + bias + residual -> embT ----
    embT = acts.tile([128, KH, T], BF16, tag="EMBT")
    with tc.tile_pool(name="pw", bufs=3, space="PSUM") as pw:
        for f in range(KH):
            for s in range(BL):
                ps = pw.tile([128, S], FP32, tag="pwo", name="pwo")
                for h in range(NH):
                    nc.tensor.matmul(ps, woT[:, h, 128 * f:128 * (f + 1)],
                                     aoT[:, s, h, :], start=(h == 0), stop=False)
                nc.tensor.matmul(ps,
                                 rowpack[:, OF_BO + 128 * f:OF_BO + 128 * (f + 1)],
                                 ones_row[:, 0:S], start=False, stop=False)
                nc.tensor.matmul(ps, ident, xT[:, f, S * s:S * (s + 1)],
                                 start=False, stop=True)
                if (2 * f + s) % 2 == 0:
                    nc.scalar.activation(out=embT[:, f, S * s:S * (s + 1)],
                                         in_=ps, func=AF.Copy)
                else:
                    nc.vector.tensor_copy(out=embT[:, f, S * s:S * (s + 1)],
                                          in_=ps)

    # late weight loads into freed slots (scalar DMA queue, off the hot path)
    w1T = acts.tile([128, KH, 1536], BF16, tag="WQKT")
    nc.scalar.dma_start(out=w1T[:, :, 0:D1],
                        in_=dd["w1t"].rearrange("(f p) c -> p f c", p=128))
    w2T = acts.tile([128, NH, H], BF16, tag="WOT")
    nc.scalar.dma_start(out=w2T[:, :, 0:D2],
                        in_=dd["w2t"].rearrange("(f p) c -> p f c", p=128))

    if _stop("wo"):
        fin()
        return

    # ---- P6: context loss (chunked) ----
    with tc.tile_pool(name="pctx", bufs=1, space="PSUM") as pcx, \
         tc.tile_pool(name="wctx", bufs=2) as wctx:
        chunks = [(0, 512), (512, T - 1)]
        for ci, (lo, hi) in enumerate(chunks):
            n = hi - lo
            pc = pcx.tile([1, 512], FP32, tag=f"pc{ci}", name="pc")
            for f in range(KH):
                d_ = wctx.tile([128, 512], BF16, tag="ctxd", name="ctxd")
                nc.vector.tensor_sub(out=d_[:, 0:n], in0=embT[:, f, lo:hi],
                                     in1=embT[:, f, lo + 1:hi + 1])
                dsq = wctx.tile([128, 512], BF16, tag="ctxsq", name="ctxsq")
                nc.scalar.activation(out=dsq[:, 0:n], in_=d_[:, 0:n],
                                     func=AF.Square)
                nc.tensor.matmul(pc[:, 0:n], onescol, dsq[:, 0:n],
                                 start=(f == 0), stop=(f == KH - 1))
            nrm = wctx.tile([1, 512], FP32, tag="nrm", name="nrm")
            nc.scalar.activation(out=nrm[:, 0:n], in_=pc[:, 0:n], func=AF.Sqrt)
            msk = wctx.tile([1, 512], FP32, tag="msk", name="msk")
            nc.vector.tensor_mul(out=msk[:, 0:n], in0=nrm[:, 0:n],
                                 in1=ctxmask[:, lo:hi])
            nc.vector.reduce_sum(out=partials[0:1, 1 + ci:2 + ci],
                                 in_=msk[:, 0:n], axis=AX.X)

    if _stop("ctx"):
        fin()
        return

    # ---- P7: quadruplet loss (emb -> token-major, host-built selectors) ----
    emb_tok = acts.tile([128, NT, H], BF16, tag="XTOK")
    with tc.tile_pool(name="ptp2", bufs=4, space="PSUM") as ptp2:
        i = 0
        for t in range(NT):
            for fq, fw in ((0, 4), (4, 2)):
                ps = ptp2.tile([128, 512], BF16, tag="tr2", name="tr2")
                for j in range(fw):
                    nc.tensor.transpose(
                        ps[:, 128 * j:128 * (j + 1)],
                        embT[:, fq + j, 128 * t:128 * (t + 1)], ident)
                w = 128 * fw
                if i % 2 == 0:
                    nc.scalar.activation(
                        out=emb_tok[:, t, 128 * fq:128 * fq + w],
                        in_=ps[:, 0:w], func=AF.Copy)
                else:
                    nc.vector.tensor_copy(
                        out=emb_tok[:, t, 128 * fq:128 * fq + w],
                        in_=ps[:, 0:w])
                i += 1
    with tc.tile_pool(name="pqd", bufs=1, space="PSUM") as pqd_pool, \
         tc.tile_pool(name="wqd", bufs=1) as wqd:
        pq1 = pqd_pool.tile([6, 512], FP32, tag="pq1", name="pq1")
        pq2 = pqd_pool.tile([6, H - 512], FP32, tag="pq2", name="pq2")
        for t in range(NT):
            nc.tensor.matmul(pq1, sel[:, t, :], emb_tok[:, t, 0:512],
                             start=(t == 0), stop=(t == NT - 1))
        for t in range(NT):
            nc.tensor.matmul(pq2, sel[:, t, :], emb_tok[:, t, 512:H],
                             start=(t == 0), stop=(t == NT - 1))
        dq1 = wqd.tile([6, 512], FP32, tag="dq1", name="dq1")
        nc.scalar.activation(out=dq1, in_=pq1, func=AF.Square)
        dq2 = wqd.tile([6, H - 512], FP32, tag="dq2", name="dq2")
        nc.scalar.activation(out=dq2, in_=pq2, func=AF.Square)
        d1c = wqd.tile([6, 1], FP32, tag="d1c", name="d1c")
        nc.vector.reduce_sum(out=d1c, in_=dq1, axis=AX.X)
        d2c = wqd.tile([6, 1], FP32, tag="d2c", name="d2c")
        nc.vector.reduce_sum(out=d2c, in_=dq2, axis=AX.X)
        dist = wqd.tile([6, 1], FP32, tag="dist", name="dist")
        nc.vector.tensor_add(out=dist, in0=d1c, in1=d2c)
        pqd = pqd_pool.tile([4, 1], FP32, tag="pqd", name="pqd")
        nc.tensor.matmul(pqd, combo, dist, start=True, stop=True)
        qrelu = wqd.tile([4, 1], FP32, tag="qrelu", name="qrelu")
        nc.scalar.activation(out=qrelu, in_=pqd, func=AF.Relu,
                             bias=ppack[0:4, PP_MARG:PP_MARG + 1])
        psq = pqd_pool.tile([1, 1], FP32, tag="psq", name="psq")
        nc.tensor.matmul(psq, qrelu, ones4f, start=True, stop=True)
        nc.vector.tensor_copy(out=partials[0:1, 0:1], in_=psq)


    # ---- P8: MLP layers (feature-major LayerNorm) ----
    def mlp_layer(li, inT, kd, wT, kdo, b_off, g_off, be_off, wb_off, s1_i,
                  gelu, outT):
        dout = 128 * kdo
        inv_d = 1.0 / float(dout)
        with tc.tile_pool(name=f"pm{li}", bufs=2, space="PSUM") as pm, \
             tc.tile_pool(name=f"pp1_{li}", bufs=1, space="PSUM") as pp1, \
             tc.tile_pool(name=f"pp2_{li}", bufs=1, space="PSUM") as pp2, \
             tc.tile_pool(name=f"pb{li}", bufs=2, space="PSUM") as pb, \
             tc.tile_pool(name=f"wk{li}", bufs=1) as wk, \
             tc.tile_pool(name=f"ap{li}", bufs=2) as ap, \
             tc.tile_pool(name=f"sq{li}", bufs=3) as sqp, \
             tc.tile_pool(name=f"zb{li}", bufs=kdo + 1) as zbp:
            for c in range(2):
                cs = slice(512 * c, 512 * (c + 1))
                ps1 = pp1.tile([1, 512], FP32, tag="s1", name="s1")
                for f in range(kd):
                    nc.tensor.matmul(ps1, wbcol[:, wb_off + f:wb_off + f + 1],
                                     inT[:, f, cs], start=(f == 0), stop=False)
                nc.tensor.matmul(ps1, rowpack[:, OF_S1B + s1_i:OF_S1B + s1_i + 1],
                                 ones_row[:, 0:512], start=False, stop=True)
                ps2 = pp2.tile([1, 512], FP32, tag="s2", name="s2")
                zs, sqs = [], []
                for po in range(kdo):
                    ps = pm.tile([128, 512], FP32, tag="z", name="z")
                    for f in range(kd):
                        nc.tensor.matmul(ps, wT[:, f, 128 * po:128 * (po + 1)],
                                         inT[:, f, cs], start=(f == 0),
                                         stop=False)
                    nc.tensor.matmul(
                        ps, rowpack[:, b_off + 128 * po:b_off + 128 * (po + 1)],
                        ones_row[:, 0:512], start=False, stop=True)
                    zb = zbp.tile([128, 512], BF16, tag="zb", name="zb")
                    nc.vector.tensor_copy(out=zb, in_=ps)
                    sq = sqp.tile([128, 512], BF16, tag="sq", name="sq")
                    nc.scalar.activation(out=sq, in_=ps, func=AF.Square)
                    zs.append(zb)
                    sqs.append(sq)
                    if po >= 1:
                        nc.tensor.matmul(ps2, onescol, sqs[po - 1],
                                         start=(po == 1), stop=False)
                nc.tensor.matmul(ps2, onescol, sqs[kdo - 1],
                                 start=(kdo == 1), stop=True)
                mu = wk.tile([1, 512], FP32, tag="mu", name="mu")
                nc.vector.tensor_scalar(out=mu, in0=ps1, scalar1=inv_d,
                                        scalar2=None, op0=ALU.mult)
                m2 = wk.tile([1, 512], FP32, tag="m2", name="m2")
                nc.vector.tensor_scalar(out=m2, in0=ps2, scalar1=inv_d,
                                        scalar2=None, op0=ALU.mult)
                musq = wk.tile([1, 512], FP32, tag="musq", name="musq")
                nc.vector.tensor_mul(out=musq, in0=mu, in1=mu)
                var = wk.tile([1, 512], FP32, tag="var", name="var")
                nc.vector.tensor_sub(out=var, in0=m2, in1=musq)
                sd = wk.tile([1, 512], FP32, tag="sd", name="sd")
                nc.scalar.activation(out=sd, in_=var, func=AF.Sqrt, bias=eps_t)
                r_bf = wk.tile([1, 512], BF16, tag="rbf", name="rbf")
                nc.vector.reciprocal(out=r_bf, in_=sd)
                mur = wk.tile([1, 512], BF16, tag="mur", name="mur")
                nc.vector.tensor_mul(out=mur, in0=mu, in1=r_bf)
                for po in range(kdo):
                    gsl = rowpack[:, g_off + 128 * po:g_off + 128 * (po + 1)]
                    pRg = pb.tile([128, 512], FP32, tag="rg", name="rg")
                    nc.tensor.matmul(pRg, gsl, r_bf, start=True, stop=True)
                    pMg = pb.tile([128, 512], FP32, tag="mg", name="mg")
                    nc.tensor.matmul(pMg, gsl, mur, start=True, stop=True)
                    t1 = ap.tile([128, 512], BF16, tag="t1", name="t1")
                    nc.vector.tensor_mul(out=t1, in0=zs[po], in1=pRg)
                    be_sl = ppack[:, be_off + po:be_off + po + 1]
                    if gelu:
                        t2 = ap.tile([128, 512], BF16, tag="t2", name="t2")
                        nc.vector.scalar_tensor_tensor(
                            out=t2, in0=t1, scalar=be_sl, in1=pMg,
                            op0=ALU.add, op1=ALU.subtract)
                        nc.scalar.activation(
                            out=outT[:, po, cs], in_=t2,
                            func=(AF.Copy if GELU_AS_COPY else AF.Gelu))
                    else:
                        nc.vector.scalar_tensor_tensor(
                            out=outT[:, po, cs], in0=t1, scalar=be_sl, in1=pMg,
                            op0=ALU.add, op1=ALU.subtract)

    h1T = acts.tile([128, KD1, T], BF16, tag="QT")
    mlp_layer(0, embT, KH, w1T, KD1, OF_B1, OF_G1, PP_BE1, WB1, 0, True, h1T)
    if _stop("mlp1"):
        fin()
        return
    h2T = acts.tile([128, KD1, T], BF16, tag="KT")   # planes 0:KD2 used
    mlp_layer(1, h1T, KD1, w2T, KD2, OF_B2, OF_G2, PP_BE2, WB2, 1, True, h2T)
    if _stop("mlp2"):
        fin()
        return
    featT = acts.tile([128, KH, T], BF16, tag="XT")  # planes 0:KD3 used
    mlp_layer(2, h2T, KD2, w3T, KD3, OF_B3, OF_G3, PP_BE3, WB3, 2, False, featT)

    if _stop("mlp3"):
        fin()
        return

    # ---- P9: CE in [9, T] layout ----
    with tc.tile_pool(name="pce", bufs=2, space="PSUM") as pce, \
         tc.tile_pool(name="pse", bufs=2, space="PSUM") as pse, \
         tc.tile_pool(name="ppk", bufs=2, space="PSUM") as ppk, \
         tc.tile_pool(name="wce", bufs=2) as wce:
        for c in range(2):
            cs = slice(512 * c, 512 * (c + 1))
            pl = pce.tile([NUM_LABELS, 512], FP32, tag="pl", name="pl")
            for d3 in range(KD3):
                nc.tensor.matmul(pl, wcT[:, d3, :], featT[:, d3, cs],
                                 start=(d3 == 0), stop=False)
            for f in range(KH):
                nc.tensor.matmul(pl, wrT[:, f, :], embT[:, f, cs],
                                 start=False, stop=False)
            nc.tensor.matmul(pl, rowpack[:, OF_BCLS:OF_BCLS + NUM_LABELS],
                             ones_row[:, 0:512], start=False, stop=True)
            e9 = wce.tile([NUM_LABELS, 512], BF16, tag="e9", name="e9")
            nc.scalar.activation(out=e9, in_=pl, func=AF.Exp)
            ps_se = pse.tile([1, 512], FP32, tag="se", name="se")
            nc.tensor.matmul(ps_se, ones9, e9, start=True, stop=True)
            lns = wce.tile([1, 512], FP32, tag="lns", name="lns")
            nc.scalar.activation(out=lns, in_=ps_se, func=AF.Ln)
            nc.vector.reduce_sum(out=partials[0:1, 3 + c:4 + c], in_=lns,
                                 axis=AX.X)
            po9 = wce.tile([NUM_LABELS, 512], FP32, tag="po9", name="po9")
            nc.vector.tensor_mul(out=po9, in0=onehot[:, cs], in1=pl)
            pr9 = wce.tile([NUM_LABELS, 1], BF16, tag="pr9", name="pr9")
            nc.vector.reduce_sum(out=pr9, in_=po9, axis=AX.X)
            ps_pk = ppk.tile([1, 1], FP32, tag="pk", name="pk")
            nc.tensor.matmul(ps_pk, pr9, ones9, start=True, stop=True)
            nc.vector.tensor_copy(out=partials[0:1, 5 + c:6 + c], in_=ps_pk)

    if _stop("ce"):
        fin()
        return

    fin()


def _get_nc():
    global _CACHED
    if _CACHED is None:
        _CACHED = _build()
    return _CACHED


def _shard(inputs):
    f32 = lambda a: np.asarray(a, np.float32)
    bf = lambda a: np.ascontiguousarray(np.asarray(a, np.float32)).astype(BFD)

    seq = f32(inputs["sequence_output"])
    labels = np.asarray(inputs["labels"]).astype(np.int64)
    a_p = np.asarray(inputs["anchor_positions"]).astype(np.int64)
    p_p = np.asarray(inputs["positive_positions"]).astype(np.int64)
    n1_p = np.asarray(inputs["negative1_positions"]).astype(np.int64)
    n2_p = np.asarray(inputs["negative2_positions"]).astype(np.int64)

    w_qkv, b_qkv = f32(inputs["w_qkv"]), f32(inputs["b_qkv"])
    w_o, b_o = f32(inputs["w_o"]), f32(inputs["b_o"])
    w1, b1 = f32(inputs["w1"]), f32(inputs["b1"])
    g1, be1 = f32(inputs["g1"]), f32(inputs["be1"])
    w2, b2 = f32(inputs["w2"]), f32(inputs["b2"])
    g2, be2 = f32(inputs["g2"]), f32(inputs["be2"])
    w3, b3 = f32(inputs["w3"]), f32(inputs["b3"])
    g3, be3 = f32(inputs["g3"]), f32(inputs["be3"])
    wc, bc = f32(inputs["wc"]), f32(inputs["bc"])
    wr, br = f32(inputs["wr"]), f32(inputs["br"])

    wq, wk, wv = w_qkv[0:H], w_qkv[H:2 * H], w_qkv[2 * H:3 * H]
    bq, bk, bv = b_qkv[0:H], b_qkv[H:2 * H], b_qkv[2 * H:3 * H]

    wqk = np.concatenate([wq.T, wk.T], axis=1)               # [H, 2H]
    wv97 = np.zeros((H, NH, 97), np.float32)
    wv97[:, :, :96] = wv.T.reshape(H, NH, HD)
    wv97 = wv97.reshape(H, 776)
    wopk = np.ascontiguousarray(
        w_o.T.reshape(NH, HD, H).transpose(1, 0, 2)).reshape(HD * NH, H)

    rowpack = np.zeros((1, ROWN), np.float32)
    rowpack[0, OF_BQK:OF_BQK + H] = bq
    rowpack[0, OF_BQK + H:OF_BQK + 2 * H] = bk
    bv97 = np.zeros((NH, 97), np.float32)
    bv97[:, :96] = bv.reshape(NH, HD)
    bv97[:, 96] = 1.0
    rowpack[0, OF_BV:OF_BV + 776] = bv97.reshape(776)
    rowpack[0, OF_BO:OF_BO + H] = b_o
    rowpack[0, OF_B1:OF_B1 + D1] = b1
    rowpack[0, OF_B2:OF_B2 + D2] = b2
    rowpack[0, OF_B3:OF_B3 + D3] = b3
    rowpack[0, OF_BCLS:OF_BCLS + NUM_LABELS] = bc + ALPHA * br
    rowpack[0, OF_S1B + 0] = b1.sum()
    rowpack[0, OF_S1B + 1] = b2.sum()
    rowpack[0, OF_S1B + 2] = b3.sum()
    rowpack[0, OF_G1:OF_G1 + D1] = g1
    rowpack[0, OF_G2:OF_G2 + D2] = g2
    rowpack[0, OF_G3:OF_G3 + D3] = g3

    ppack = np.zeros((128, PPC), np.float32)
    ppack[:, PP_BE1:PP_BE1 + KD1] = be1.reshape(KD1, 128).T
    ppack[:, PP_BE2:PP_BE2 + KD2] = be2.reshape(KD2, 128).T
    ppack[:, PP_BE3:PP_BE3 + KD3] = be3.reshape(KD3, 128).T
    ppack[0:4, PP_MARG] = [MARGIN1, MARGIN2, MARGIN1, MARGIN2]

    wbcol = np.zeros((128, WBC), np.float32)
    wbcol[:, WB1:WB1 + KH] = w1.sum(axis=0).reshape(KH, 128).T
    wbcol[:, WB2:WB2 + KD1] = w2.sum(axis=0).reshape(KD1, 128).T
    wbcol[:, WB3:WB3 + KD2] = w3.sum(axis=0).reshape(KD2, 128).T

    combo = np.zeros((6, 4), np.float32)
    for k, (ipd, ind) in enumerate([(0, 1), (0, 2), (3, 4), (3, 5)]):
        combo[ipd, k] = 1.0
        combo[ind, k] = -1.0

    base = {
        "wqk": bf(wqk), "wv97": bf(wv97), "wopk": bf(wopk),
        "w1t": bf(w1.T), "w2t": bf(w2.T), "w3t": bf(w3.T),
        "wct": bf(wc.T), "wrt": bf(ALPHA * wr.T),
        "rowpack": bf(rowpack), "ppack": np.ascontiguousarray(ppack),
        "wbcol": bf(wbcol), "combo": np.ascontiguousarray(combo),
    }

    in_maps = []
    for c in range(NCORES):
        sl = slice(BL * c, BL * (c + 1))
        lab = labels[sl]                       # [BL, S]
        oh = np.zeros((NUM_LABELS, T), np.float32)
        flat = lab.reshape(T)
        oh[flat, np.arange(T)] = 1.0
        m = (lab[:, :-1] != 0) & (lab[:, :-1] == lab[:, 1:])   # [BL, S-1]
        cm = np.zeros((1, T - 1), np.float32)
        for s in range(BL):
            cm[0, S * s:S * s + S - 1] = m[s]
        selm = np.zeros((T, 6), np.float32)
        for s in range(BL):
            b = BL * c + s
            av = int(a_p[b]) + S * s
            for j, pos in enumerate((p_p[b], n1_p[b], n2_p[b])):
                col = 3 * s + j
                selm[av, col] += 1.0
                selm[int(pos) + S * s, col] -= 1.0
        mm = dict(base)
        mm["xt"] = bf(seq[sl].reshape(T, H).T)
        mm["onehot"] = bf(oh)
        mm["ctxmask"] = bf(cm)
        mm["sel"] = bf(selm)
        in_maps.append(mm)
    return in_maps


def kernel(**inputs):
    nc = _get_nc()
    in_maps = _shard(inputs)
    res = run_bass_kernel_spmd(nc, in_maps, core_ids=list(range(NCORES)))
    ce = quad = ctx = 0.0
    for c in range(NCORES):
        o = np.asarray(res.results[c]["out"], np.float64).reshape(8)
        quad += float(o[0])
        ctx += float(o[1] + o[2])
        ce += float(o[3] + o[4] - o[5] - o[6])
    total = ce / (B * S) + ALPHA * (quad / B) + BETA * (ctx / (B * S))
    return np.float32(total)


# revision 10
# speedup vs baseline: 1.1459x; 1.0267x over previous
"""Trainium2 Bass kernel for nn_LLMCC_74414603370526 (loss_fn) — v2.

Data-parallel over batch: 16 sequences -> 8 cores x 2 sequences each; host
combines the scalar partial losses (the sanctioned all-reduce).

v2 design (instruction-count-minimized vs v1):
  - activations flow feature-major end-to-end (x arrives host-transposed);
    the only PE transposes are the emb token-major copy for the quadruplet
    gather (48, packed 4-to-a-bank)
  - weights are pre-transposed on the HOST into DMA-friendly layouts; no
    on-device weight transposes
  - biases/residual/softmax-rowsum fold into matmuls (K=1 outer products,
    identity lhsT accumulate, ones column appended to v)
  - LayerNorm runs feature-major: per-token stats via ones/w-bar matmul
    column reductions; scale/shift applied via PE rank-1 outer products
    (g x rstd) and two vector ops per tile
  - CE in [9, T] label-major layout with host-built one-hot labels;
    context-loss label mask is host-built
  - SBUF slots reused via byte-equal tag chains:
    XTOK: x_tok -> emb_tok | XT: xT -> featT | QT: qT -> h1T
    KT: kT -> h2T | WQKT: wqkT -> w1T | WOT: woT -> w2T
"""

import numpy as np
import ml_dtypes

import concourse.bass as bass
import concourse.mybir as mybir
import concourse.tile as tile
from concourse import bacc
from concourse.bass_utils import run_bass_kernel_spmd
from concourse.masks import make_identity

FP32 = mybir.dt.float32
BF16 = mybir.dt.bfloat16
AF = mybir.ActivationFunctionType
ALU = mybir.AluOpType
AX = mybir.AxisListType

B, S, H = 16, 512, 768
NH, HD = 8, 96
NUM_LABELS = 9
MARGIN1, MARGIN2 = 1.0, 0.5
ALPHA, BETA = 0.2, 0.1
EPS = 1e-5

NCORES = 8
BL = B // NCORES          # 2 sequences per core
T = BL * S                # 1024 tokens per core
NT = T // 128             # 8 token tiles
KH = H // 128             # 6 feature tiles
D1, D2, D3 = 1024, 512, 256
KD1, KD2, KD3 = D1 // 128, D2 // 128, D3 // 128
ISQ = 1.0 / float(np.sqrt(HD))
BFD = ml_dtypes.bfloat16

# rowpack offsets (bf16 row: biases / row-vectors)
OF_BQK = 0                       # 1536
OF_BV = OF_BQK + 2 * H           # 776
OF_BO = OF_BV + 776              # 768
OF_B1 = OF_BO + H                # 1024
OF_B2 = OF_B1 + D1               # 512
OF_B3 = OF_B2 + D2               # 256
OF_BCLS = OF_B3 + D3             # 9
OF_S1B = OF_BCLS + NUM_LABELS    # 3
OF_G1 = OF_S1B + 3               # 1024
OF_G2 = OF_G1 + D1               # 512
OF_G3 = OF_G2 + D2               # 256
ROWN = OF_G3 + D3

# ppack (fp32 per-partition pack)
PP_BE1 = 0          # 8 cols
PP_BE2 = 8          # 4
PP_BE3 = 12         # 2
PP_MARG = 14        # margins in rows 0:4 of col 14
PPC = 15

# wbcol (bf16 per-partition colsum-weight pack)
WB1, WB2, WB3 = 0, 6, 14
WBC = 18

GELU_AS_COPY = False   # sim-debug only (CoreSim executor lacks Gelu numerics)

_CACHED = None


def _build(stop_after=None):
    nc = bacc.Bacc(None, target_bir_lowering=False)
    dd = {}

    def di(name, shape, dt=BF16):
        dd[name] = nc.dram_tensor(name, shape, dt, kind="ExternalInput")

    di("xt", [H, T])
    di("wqk", [H, 2 * H])
    di("wv97", [H, 776])
    di("wopk", [HD * NH, H])       # rows = (d-within-head, head), cols f
    di("w1t", [H, D1])
    di("w2t", [D1, D2])
    di("w3t", [D2, D3])
    di("wct", [D3, NUM_LABELS])
    di("wrt", [H, NUM_LABELS])     # pre-scaled by 0.2 on host
    di("rowpack", [1, ROWN])
    di("ppack", [128, PPC], FP32)
    di("wbcol", [128, WBC])
    di("onehot", [NUM_LABELS, T])
    di("ctxmask", [1, T - 1])
    di("sel", [T, 6])
    di("combo", [6, 4], FP32)
    out_d = nc.dram_tensor("out", [1, 8], FP32, kind="ExternalOutput")

    with tile.TileContext(nc) as tc:
        with nc.allow_low_precision(reason="bf16 PE-transpose PSUM tiles"):
            _body(nc, tc, dd, out_d, stop_after)
    nc.finalize()
    return nc


def _body(nc, tc, dd, out_d, stop_after=None):
    const = tc.alloc_tile_pool(name="const", bufs=1)
    acts = tc.alloc_tile_pool(name="acts", bufs=1)

    def _stop(phase):
        return stop_after == phase

    def fin():
        nc.sync.dma_start(out=out_d[:, :], in_=partials)
        acts.release()
        const.release()


    # ---- constants / inputs ----
    ident = const.tile([128, 128], BF16)
    make_identity(nc, ident)
    ones_row = const.tile([1, 1024], BF16)
    nc.vector.memset(ones_row, 1.0)
    onescol = const.tile([128, 1], BF16)
    nc.vector.memset(onescol, 1.0)
    ones9 = const.tile([NUM_LABELS, 1], BF16)
    nc.vector.memset(ones9, 1.0)
    ones4f = const.tile([4, 1], FP32)
    nc.vector.memset(ones4f, 1.0)
    eps_t = const.tile([1, 1], FP32)
    nc.vector.memset(eps_t, EPS)
    partials = const.tile([1, 8], FP32)
    nc.vector.memset(partials, 0.0)

    xT = acts.tile([128, KH, T], BF16, tag="XT")
    nc.sync.dma_start(out=xT, in_=dd["xt"].rearrange("(f p) t -> p f t", p=128))
    wqkT = acts.tile([128, KH, 1536], BF16, tag="WQKT")
    nc.scalar.dma_start(out=wqkT, in_=dd["wqk"].rearrange("(f p) c -> p f c", p=128))
    rowpack = const.tile([1, ROWN], BF16)
    nc.sync.dma_start(out=rowpack, in_=dd["rowpack"][:, :])
    wv97T = const.tile([128, KH, 776], BF16)
    nc.gpsimd.dma_start(out=wv97T, in_=dd["wv97"].rearrange("(f p) c -> p f c", p=128))
    woT = acts.tile([HD, NH, H], BF16, tag="WOT")
    nc.gpsimd.dma_start(out=woT, in_=dd["wopk"].rearrange("(p h) f -> p h f", p=HD))
    w3T = const.tile([128, KD2, D3], BF16)
    nc.gpsimd.dma_start(out=w3T, in_=dd["w3t"].rearrange("(f p) c -> p f c", p=128))
    wcT = const.tile([128, KD3, NUM_LABELS], BF16)
    nc.gpsimd.dma_start(out=wcT, in_=dd["wct"].rearrange("(f p) c -> p f c", p=128))
    wrT = const.tile([128, KH, NUM_LABELS], BF16)
    nc.gpsimd.dma_start(out=wrT, in_=dd["wrt"].rearrange("(f p) c -> p f c", p=128))
    ppack = const.tile([128, PPC], FP32)
    nc.gpsimd.dma_start(out=ppack, in_=dd["ppack"][:, :])
    wbcol = const.tile([128, WBC], BF16)
    nc.gpsimd.dma_start(out=wbcol, in_=dd["wbcol"][:, :])
    onehot = const.tile([NUM_LABELS, T], BF16)
    nc.gpsimd.dma_start(out=onehot, in_=dd["onehot"][:, :])
    ctxmask = const.tile([1, T - 1], BF16)
    nc.gpsimd.dma_start(out=ctxmask, in_=dd["ctxmask"][:, :])
    sel = const.tile([128, NT, 6], BF16)
    nc.gpsimd.dma_start(out=sel, in_=dd["sel"].rearrange("(n p) c -> p n c", p=128))
    combo = const.tile([6, 4], FP32)
    nc.gpsimd.dma_start(out=combo, in_=dd["combo"][:, :])

    if _stop("load"):
        fin()
        return

    if _stop("p1"):
        fin()
        return

    # ---- P2+P3: q, k, v ----
    qT = acts.tile([HD, NH, T], BF16, tag="QT")
    kT = acts.tile([HD, NH, T], BF16, tag="KT")
    v97 = acts.tile([128, NT, 776], BF16, tag="V97")
    with tc.tile_pool(name="pq", bufs=4, space="PSUM") as pq, \
         tc.tile_pool(name="pv", bufs=2, space="PSUM") as pv:
        i = 0
        for h in range(NH):
            for w in range(2):      # 0=q, 1=k
                dst = qT if w == 0 else kT
                off = H * w + HD * h
                for c in range(2):
                    ps = pq.tile([HD, 512], FP32, tag="pqk", name="pqk")
                    for f in range(KH):
                        nc.tensor.matmul(ps, wqkT[:, f, off:off + HD],
                                         xT[:, f, 512 * c:512 * (c + 1)],
                                         start=(f == 0), stop=False)
                    nc.tensor.matmul(ps,
                                     rowpack[:, OF_BQK + off:OF_BQK + off + HD],
                                     ones_row[:, 0:512], start=False, stop=True)
                    if i % 2 == 0:
                        nc.scalar.activation(out=dst[:, h, 512 * c:512 * (c + 1)],
                                             in_=ps, func=AF.Copy)
                    else:
                        nc.vector.tensor_copy(out=dst[:, h, 512 * c:512 * (c + 1)],
                                              in_=ps)
                    i += 1
        for t in range(NT):
            for g in range(2):
                ps = pv.tile([128, 388], FP32, tag="pv", name="pv")
                for f in range(KH):
                    nc.tensor.matmul(ps, xT[:, f, 128 * t:128 * (t + 1)],
                                     wv97T[:, f, 388 * g:388 * (g + 1)],
                                     start=(f == 0), stop=False)
                nc.tensor.matmul(ps, ones_row[0:1, 0:128],
                                 rowpack[:, OF_BV + 388 * g:OF_BV + 388 * (g + 1)],
                                 start=False, stop=True)
                if t % 2 == 0:
                    nc.scalar.activation(out=v97[:, t, 388 * g:388 * (g + 1)],
                                         in_=ps, func=AF.Copy)
                else:
                    nc.vector.tensor_copy(out=v97[:, t, 388 * g:388 * (g + 1)],
                                          in_=ps)

    if _stop("qkv"):
        fin()
        return

    # ---- P4: attention (exp without max-shift; rowsum via v ones column) ----
    aoT = acts.tile([HD, BL, NH, S], BF16, tag="AOT")
    with tc.tile_pool(name="pa", bufs=4, space="PSUM") as pa, \
         tc.tile_pool(name="pa2", bufs=2, space="PSUM") as pa2, \
         tc.tile_pool(name="pa3", bufs=2, space="PSUM") as pa3, \
         tc.tile_pool(name="wet", bufs=6) as wet, \
         tc.tile_pool(name="wra", bufs=2) as wra:
        def tail(sh):
            s, h, pao = sh
            rec = wra.tile([1, S], BF16, tag="rec", name="rec")
            nc.vector.reciprocal(out=rec, in_=pao[HD:HD + 1, :])
            prec = pa3.tile([HD, S], FP32, tag="prec", name="prec")
            nc.tensor.matmul(prec, ones_row[0:1, 0:HD], rec, start=True,
                             stop=True)
            aou = wra.tile([HD, S], BF16, tag="aou", name="aou")
            nc.scalar.activation(out=aou, in_=pao[0:HD, :], func=AF.Copy)
            nc.vector.tensor_mul(out=aoT[:, s, h, :], in0=aou, in1=prec)

        pending = None
        for s in range(BL):
            for h in range(NH):
                et = []
                for kt in range(4):
                    psc = pa.tile([128, S], FP32, tag="psc", name="psc")
                    nc.tensor.matmul(
                        psc, kT[:, h, S * s + 128 * kt:S * s + 128 * (kt + 1)],
                        qT[:, h, S * s:S * (s + 1)], start=True, stop=True)
                    e = wet.tile([128, S], BF16, tag="et", name="et")
                    nc.scalar.activation(out=e, in_=psc, func=AF.Exp, scale=ISQ)
                    et.append(e)
                pao = pa2.tile([HD + 1, S], FP32, tag="pao", name="pao")
                g, hh = h // 4, h % 4
                voff = 388 * g + 97 * hh
                for kt in range(4):
                    nc.tensor.matmul(pao, v97[:, 4 * s + kt, voff:voff + 97],
                                     et[kt], start=(kt == 0), stop=(kt == 3))
                if pending is not None:
                    tail(pending)
                pending = (s, h, pao)
        tail(pending)

    if _stop("attn"):
        fin()
        return

    # ---- P5: w_o + bias + residual -> embT ----
    embT = acts.tile([128, KH, T], BF16, tag="EMBT")
    with tc.tile_pool(name="pw", bufs=3, space="PSUM") as pw:
        for f in range(KH):
            for s in range(BL):
                ps = pw.tile([128, S], FP32, tag="pwo", name="pwo")
                for h in range(NH):
                    nc.tensor.matmul(ps, woT[:, h, 128 * f:128 * (f + 1)],
                                     aoT[:, s, h, :], start=(h == 0), stop=False)
                nc.tensor.matmul(ps,
                                 rowpack[:, OF_BO + 128 * f:OF_BO + 128 * (f + 1)],
                                 ones_row[:, 0:S], start=False, stop=False)
                nc.tensor.matmul(ps, ident, xT[:, f, S * s:S * (s + 1)],
                                 start=False, stop=True)
                if (2 * f + s) % 2 == 0:
                    nc.scalar.activation(out=embT[:, f, S * s:S * (s + 1)],
                                         in_=ps, func=AF.Copy)
                else:
                    nc.vector.tensor_copy(out=embT[:, f, S * s:S * (s + 1)],
                                          in_=ps)

    # late weight loads into freed slots (scalar DMA queue, off the hot path)
    w1T = acts.tile([128, KH, 1536], BF16, tag="WQKT")
    nc.scalar.dma_start(out=w1T[:, :, 0:D1],
                        in_=dd["w1t"].rearrange("(f p) c -> p f c", p=128))
    w2T = acts.tile([128, NH, H], BF16, tag="WOT")
    nc.scalar.dma_start(out=w2T[:, :, 0:D2],
                        in_=dd["w2t"].rearrange("(f p) c -> p f c", p=128))

    if _stop("wo"):
        fin()
        return

    # ---- P6: context loss (chunked) ----
    with tc.tile_pool(name="pctx", bufs=1, space="PSUM") as pcx, \
         tc.tile_pool(name="wctx", bufs=2) as wctx:
        chunks = [(0, 512), (512, T - 1)]
        for ci, (lo, hi) in enumerate(chunks):
            n = hi - lo
            pc = pcx.tile([1, 512], FP32, tag=f"pc{ci}", name="pc")
            for f in range(KH):
                d_ = wctx.tile([128, 512], BF16, tag="ctxd", name="ctxd")
                nc.vector.tensor_sub(out=d_[:, 0:n], in0=embT[:, f, lo:hi],
                                     in1=embT[:, f, lo + 1:hi + 1])
                dsq = wctx.tile([128, 512], BF16, tag="ctxsq", name="ctxsq")
                nc.scalar.activation(out=dsq[:, 0:n], in_=d_[:, 0:n],
                                     func=AF.Square)
                nc.tensor.matmul(pc[:, 0:n], onescol, dsq[:, 0:n],
                                 start=(f == 0), stop=(f == KH - 1))
            nrm = wctx.tile([1, 512], FP32, tag="nrm", name="nrm")
            nc.scalar.activation(out=nrm[:, 0:n], in_=pc[:, 0:n], func=AF.Sqrt)
            msk = wctx.tile([1, 512], FP32, tag="msk", name="msk")
            nc.vector.tensor_mul(out=msk[:, 0:n], in0=nrm[:, 0:n],
                                 in1=ctxmask[:, lo:hi])
            nc.vector.reduce_sum(out=partials[0:1, 1 + ci:2 + ci],
                                 in_=msk[:, 0:n], axis=AX.X)

    if _stop("ctx"):
        fin()
        return

    # ---- P7: quadruplet loss (emb -> token-major, host-built selectors) ----
    emb_tok = acts.tile([128, NT, H], BF16, tag="XTOK")
    with tc.tile_pool(name="ptp2", bufs=4, space="PSUM") as ptp2:
        i = 0
        for t in range(NT):
            for fq, fw in ((0, 4), (4, 2)):
                ps = ptp2.tile([128, 512], BF16, tag="tr2", name="tr2")
                for j in range(fw):
                    nc.tensor.transpose(
                        ps[:, 128 * j:128 * (j + 1)],
                        embT[:, fq + j, 128 * t:128 * (t + 1)], ident)
                w = 128 * fw
                if i % 2 == 0:
                    nc.scalar.activation(
                        out=emb_tok[:, t, 128 * fq:128 * fq + w],
                        in_=ps[:, 0:w], func=AF.Copy)
                else:
                    nc.vector.tensor_copy(
                        out=emb_tok[:, t, 128 * fq:128 * fq + w],
                        in_=ps[:, 0:w])
                i += 1
    with tc.tile_pool(name="pqd", bufs=1, space="PSUM") as pqd_pool, \
         tc.tile_pool(name="wqd", bufs=1) as wqd:
        pq1 = pqd_pool.tile([6, 512], FP32, tag="pq1", name="pq1")
        pq2 = pqd_pool.tile([6, H - 512], FP32, tag="pq2", name="pq2")
        for t in range(NT):
            nc.tensor.matmul(pq1, sel[:, t, :], emb_tok[:, t, 0:512],
                             start=(t == 0), stop=(t == NT - 1))
        for t in range(NT):
            nc.tensor.matmul(pq2, sel[:, t, :], emb_tok[:, t, 512:H],
                             start=(t == 0), stop=(t == NT - 1))
        dq1 = wqd.tile([6, 512], FP32, tag="dq1", name="dq1")
        nc.scalar.activation(out=dq1, in_=pq1, func=AF.Square)
        dq2 = wqd.tile([6, H - 512], FP32, tag="dq2", name="dq2")
        nc.scalar.activation(out=dq2, in_=pq2, func=AF.Square)
        d1c = wqd.tile([6, 1], FP32, tag="d1c", name="d1c")
        nc.vector.reduce_sum(out=d1c, in_=dq1, axis=AX.X)
        d2c = wqd.tile([6, 1], FP32, tag="d2c", name="d2c")
        nc.vector.reduce_sum(out=d2c, in_=dq2, axis=AX.X)
        dist = wqd.tile([6, 1], FP32, tag="dist", name="dist")
        nc.vector.tensor_add(out=dist, in0=d1c, in1=d2c)
        pqd = pqd_pool.tile([4, 1], FP32, tag="pqd", name="pqd")
        nc.tensor.matmul(pqd, combo, dist, start=True, stop=True)
        qrelu = wqd.tile([4, 1], FP32, tag="qrelu", name="qrelu")
        nc.scalar.activation(out=qrelu, in_=pqd, func=AF.Relu,
                             bias=ppack[0:4, PP_MARG:PP_MARG + 1])
        psq = pqd_pool.tile([1, 1], FP32, tag="psq", name="psq")
        nc.tensor.matmul(psq, qrelu, ones4f, start=True, stop=True)
        nc.vector.tensor_copy(out=partials[0:1, 0:1], in_=psq)


    # ---- P8: MLP layers (feature-major LayerNorm) ----
    def mlp_layer(li, inT, kd, wT, kdo, b_off, g_off, be_off, wb_off, s1_i,
                  gelu, outT):
        dout = 128 * kdo
        inv_d = 1.0 / float(dout)
        with tc.tile_pool(name=f"pm{li}", bufs=2, space="PSUM") as pm, \
             tc.tile_pool(name=f"pp1_{li}", bufs=2, space="PSUM") as pp1, \
             tc.tile_pool(name=f"pp2_{li}", bufs=2, space="PSUM") as pp2, \
             tc.tile_pool(name=f"pb{li}", bufs=1, space="PSUM") as pb, \
             tc.tile_pool(name=f"wka{li}", bufs=1) as wka, \
             tc.tile_pool(name=f"wkb{li}", bufs=2) as wkb, \
             tc.tile_pool(name=f"ap{li}", bufs=2) as ap, \
             tc.tile_pool(name=f"sq{li}", bufs=3) as sqp, \
             tc.tile_pool(name=f"zb{li}", bufs=2 * kdo + 1) as zbp:
            stage1 = []
            for c in range(2):
                cs = slice(512 * c, 512 * (c + 1))
                ps1 = pp1.tile([1, 512], FP32, tag="s1", name="s1")
                for f in range(kd):
                    nc.tensor.matmul(ps1, wbcol[:, wb_off + f:wb_off + f + 1],
                                     inT[:, f, cs], start=(f == 0), stop=False)
                nc.tensor.matmul(ps1, rowpack[:, OF_S1B + s1_i:OF_S1B + s1_i + 1],
                                 ones_row[:, 0:512], start=False, stop=True)
                ps2 = pp2.tile([1, 512], FP32, tag="s2", name="s2")
                zs, sqs = [], []
                for po in range(kdo):
                    ps = pm.tile([128, 512], FP32, tag="z", name="z")
                    for f in range(kd):
                        nc.tensor.matmul(ps, wT[:, f, 128 * po:128 * (po + 1)],
                                         inT[:, f, cs], start=(f == 0),
                                         stop=False)
                    nc.tensor.matmul(
                        ps, rowpack[:, b_off + 128 * po:b_off + 128 * (po + 1)],
                        ones_row[:, 0:512], start=False, stop=True)
                    zb = zbp.tile([128, 512], BF16, tag="zb", name="zb")
                    nc.vector.tensor_copy(out=zb, in_=ps)
                    sq = sqp.tile([128, 512], BF16, tag="sq", name="sq")
                    nc.scalar.activation(out=sq, in_=ps, func=AF.Square)
                    zs.append(zb)
                    sqs.append(sq)
                    if po >= 1:
                        nc.tensor.matmul(ps2, onescol, sqs[po - 1],
                                         start=(po == 1), stop=False)
                nc.tensor.matmul(ps2, onescol, sqs[kdo - 1],
                                 start=(kdo == 1), stop=True)
                stage1.append((cs, ps1, ps2, zs))
            for (cs, ps1, ps2, zs) in stage1:
                mu = wka.tile([1, 512], FP32, tag="mu", name="mu")
                nc.vector.tensor_scalar(out=mu, in0=ps1, scalar1=inv_d,
                                        scalar2=None, op0=ALU.mult)
                m2 = wka.tile([1, 512], FP32, tag="m2", name="m2")
                nc.vector.tensor_scalar(out=m2, in0=ps2, scalar1=inv_d,
                                        scalar2=None, op0=ALU.mult)
                musq = wka.tile([1, 512], FP32, tag="musq", name="musq")
                nc.vector.tensor_mul(out=musq, in0=mu, in1=mu)
                var = wka.tile([1, 512], FP32, tag="var", name="var")
                nc.vector.tensor_sub(out=var, in0=m2, in1=musq)
                sd = wka.tile([1, 512], FP32, tag="sd", name="sd")
                nc.scalar.activation(out=sd, in_=var, func=AF.Sqrt, bias=eps_t)
                r_bf = wkb.tile([1, 512], BF16, tag="rbf", name="rbf")
                nc.vector.reciprocal(out=r_bf, in_=sd)
                mur = wkb.tile([1, 512], BF16, tag="mur", name="mur")
                nc.vector.tensor_mul(out=mur, in0=mu, in1=r_bf)
                for po in range(kdo):
                    gsl = rowpack[:, g_off + 128 * po:g_off + 128 * (po + 1)]
                    pRg = pb.tile([128, 512], FP32, tag="rg", name="rg")
                    nc.tensor.matmul(pRg, gsl, r_bf, start=True, stop=True)
                    pMg = pb.tile([128, 512], FP32, tag="mg", name="mg")
                    nc.tensor.matmul(pMg, gsl, mur, start=True, stop=True)
                    t1 = ap.tile([128, 512], BF16, tag="t1", name="t1")
                    nc.vector.tensor_mul(out=t1, in0=zs[po], in1=pRg)
                    be_sl = ppack[:, be_off + po:be_off + po + 1]
                    if gelu:
                        t2 = ap.tile([128, 512], BF16, tag="t2", name="t2")
                        nc.vector.scalar_tensor_tensor(
                            out=t2, in0=t1, scalar=be_sl, in1=pMg,
                            op0=ALU.add, op1=ALU.subtract)
                        nc.scalar.activation(
                            out=outT[:, po, cs], in_=t2,
                            func=(AF.Copy if GELU_AS_COPY else AF.Gelu))
                    else:
                        nc.vector.scalar_tensor_tensor(
                            out=outT[:, po, cs], in0=t1, scalar=be_sl, in1=pMg,
                            op0=ALU.add, op1=ALU.subtract)

    h1T = acts.tile([128, KD1, T], BF16, tag="QT")
    mlp_layer(0, embT, KH, w1T, KD1, OF_B1, OF_G1, PP_BE1, WB1, 0, True, h1T)
    if _stop("mlp1"):
        fin()
        return
    h2T = acts.tile([128, KD1, T], BF16, tag="KT")   # planes 0:KD2 used
    mlp_layer(1, h1T, KD1, w2T, KD2, OF_B2, OF_G2, PP_BE2, WB2, 1, True, h2T)
    if _stop("mlp2"):
        fin()
        return
    featT = acts.tile([128, KH, T], BF16, tag="XT")  # planes 0:KD3 used
    mlp_layer(2, h2T, KD2, w3T, KD3, OF_B3, OF_G3, PP_BE3, WB3, 2, False, featT)

    if _stop("mlp3"):
        fin()
        return

    # ---- P9: CE in [9, T] layout ----
    with tc.tile_pool(name="pce", bufs=2, space="PSUM") as pce, \
         tc.tile_pool(name="pse", bufs=2, space="PSUM") as pse, \
         tc.tile_pool(name="ppk", bufs=2, space="PSUM") as ppk, \
         tc.tile_pool(name="wce", bufs=2) as wce:
        for c in range(2):
            cs = slice(512 * c, 512 * (c + 1))
            pl = pce.tile([NUM_LABELS, 512], FP32, tag="pl", name="pl")
            for d3 in range(KD3):
                nc.tensor.matmul(pl, wcT[:, d3, :], featT[:, d3, cs],
                                 start=(d3 == 0), stop=False)
            for f in range(KH):
                nc.tensor.matmul(pl, wrT[:, f, :], embT[:, f, cs],
                                 start=False, stop=False)
            nc.tensor.matmul(pl, rowpack[:, OF_BCLS:OF_BCLS + NUM_LABELS],
                             ones_row[:, 0:512], start=False, stop=True)
            e9 = wce.tile([NUM_LABELS, 512], BF16, tag="e9", name="e9")
            nc.scalar.activation(out=e9, in_=pl, func=AF.Exp)
            ps_se = pse.tile([1, 512], FP32, tag="se", name="se")
            nc.tensor.matmul(ps_se, ones9, e9, start=True, stop=True)
            lns = wce.tile([1, 512], FP32, tag="lns", name="lns")
            nc.scalar.activation(out=lns, in_=ps_se, func=AF.Ln)
            nc.vector.reduce_sum(out=partials[0:1, 3 + c:4 + c], in_=lns,
                                 axis=AX.X)
            po9 = wce.tile([NUM_LABELS, 512], FP32, tag="po9", name="po9")
            nc.vector.tensor_mul(out=po9, in0=onehot[:, cs], in1=pl)
            pr9 = wce.tile([NUM_LABELS, 1], BF16, tag="pr9", name="pr9")
            nc.vector.reduce_sum(out=pr9, in_=po9, axis=AX.X)
            ps_pk = ppk.tile([1, 1], FP32, tag="pk", name="pk")
            nc.tensor.matmul(ps_pk, pr9, ones9, start=True, stop=True)
            nc.vector.tensor_copy(out=partials[0:1, 5 + c:6 + c], in_=ps_pk)

    if _stop("ce"):
        fin()
        return

    fin()


def _get_nc():
    global _CACHED
    if _CACHED is None:
        _CACHED = _build()
    return _CACHED


def _shard(inputs):
    f32 = lambda a: np.asarray(a, np.float32)
    bf = lambda a: np.ascontiguousarray(np.asarray(a, np.float32)).astype(BFD)

    seq = f32(inputs["sequence_output"])
    labels = np.asarray(inputs["labels"]).astype(np.int64)
    a_p = np.asarray(inputs["anchor_positions"]).astype(np.int64)
    p_p = np.asarray(inputs["positive_positions"]).astype(np.int64)
    n1_p = np.asarray(inputs["negative1_positions"]).astype(np.int64)
    n2_p = np.asarray(inputs["negative2_positions"]).astype(np.int64)

    w_qkv, b_qkv = f32(inputs["w_qkv"]), f32(inputs["b_qkv"])
    w_o, b_o = f32(inputs["w_o"]), f32(inputs["b_o"])
    w1, b1 = f32(inputs["w1"]), f32(inputs["b1"])
    g1, be1 = f32(inputs["g1"]), f32(inputs["be1"])
    w2, b2 = f32(inputs["w2"]), f32(inputs["b2"])
    g2, be2 = f32(inputs["g2"]), f32(inputs["be2"])
    w3, b3 = f32(inputs["w3"]), f32(inputs["b3"])
    g3, be3 = f32(inputs["g3"]), f32(inputs["be3"])
    wc, bc = f32(inputs["wc"]), f32(inputs["bc"])
    wr, br = f32(inputs["wr"]), f32(inputs["br"])

    wq, wk, wv = w_qkv[0:H], w_qkv[H:2 * H], w_qkv[2 * H:3 * H]
    bq, bk, bv = b_qkv[0:H], b_qkv[H:2 * H], b_qkv[2 * H:3 * H]

    wqk = np.concatenate([wq.T, wk.T], axis=1)               # [H, 2H]
    wv97 = np.zeros((H, NH, 97), np.float32)
    wv97[:, :, :96] = wv.T.reshape(H, NH, HD)
    wv97 = wv97.reshape(H, 776)
    wopk = np.ascontiguousarray(
        w_o.T.reshape(NH, HD, H).transpose(1, 0, 2)).reshape(HD * NH, H)

    rowpack = np.zeros((1, ROWN), np.float32)
    rowpack[0, OF_BQK:OF_BQK + H] = bq
    rowpack[0, OF_BQK + H:OF_BQK + 2 * H] = bk
    bv97 = np.zeros((NH, 97), np.float32)
    bv97[:, :96] = bv.reshape(NH, HD)
    bv97[:, 96] = 1.0
    rowpack[0, OF_BV:OF_BV + 776] = bv97.reshape(776)
    rowpack[0, OF_BO:OF_BO + H] = b_o
    rowpack[0, OF_B1:OF_B1 + D1] = b1
    rowpack[0, OF_B2:OF_B2 + D2] = b2
    rowpack[0, OF_B3:OF_B3 + D3] = b3
    rowpack[0, OF_BCLS:OF_BCLS + NUM_LABELS] = bc + ALPHA * br
    rowpack[0, OF_S1B + 0] = b1.sum()
    rowpack[0, OF_S1B + 1] = b2.sum()
    rowpack[0, OF_S1B + 2] = b3.sum()
    rowpack[0, OF_G1:OF_G1 + D1] = g1
    rowpack[0, OF_G2:OF_G2 + D2] = g2
    rowpack[0, OF_G3:OF_G3 + D3] = g3

    ppack = np.zeros((128, PPC), np.float32)
    ppack[:, PP_BE1:PP_BE1 + KD1] = be1.reshape(KD1, 128).T
    ppack[:, PP_BE2:PP_BE2 + KD2] = be2.reshape(KD2, 128).T
    ppack[:, PP_BE3:PP_BE3 + KD3] = be3.reshape(KD3, 128).T
    ppack[0:4, PP_MARG] = [MARGIN1, MARGIN2, MARGIN1, MARGIN2]

    wbcol = np.zeros((128, WBC), np.float32)
    wbcol[:, WB1:WB1 + KH] = w1.sum(axis=0).reshape(KH, 128).T
    wbcol[:, WB2:WB2 + KD1] = w2.sum(axis=0).reshape(KD1, 128).T
    wbcol[:, WB3:WB3 + KD2] = w3.sum(axis=0).reshape(KD2, 128).T

    combo = np.zeros((6, 4), np.float32)
    for k, (ipd, ind) in enumerate([(0, 1), (0, 2), (3, 4), (3, 5)]):
        combo[ipd, k] = 1.0
        combo[ind, k] = -1.0

    base = {
        "wqk": bf(wqk), "wv97": bf(wv97), "wopk": bf(wopk),
        "w1t": bf(w1.T), "w2t": bf(w2.T), "w3t": bf(w3.T),
        "wct": bf(wc.T), "wrt": bf(ALPHA * wr.T),
        "rowpack": bf(rowpack), "ppack": np.ascontiguousarray(ppack),
        "wbcol": bf(wbcol), "combo": np.ascontiguousarray(combo),
    }

    in_maps = []
    for c in range(NCORES):
        sl = slice(BL * c, BL * (c + 1))
        lab = labels[sl]                       # [BL, S]
        oh = np.zeros((NUM_LABELS, T), np.float32)
        flat = lab.reshape(T)
        oh[flat, np.arange(T)] = 1.0
        m = (lab[:, :-1] != 0) & (lab[:, :-1] == lab[:, 1:])   # [BL, S-1]
        cm = np.zeros((1, T - 1), np.float32)
        for s in range(BL):
            cm[0, S * s:S * s + S - 1] = m[s]
        selm = np.zeros((T, 6), np.float32)
        for s in range(BL):
            b = BL * c + s
            av = int(a_p[b]) + S * s
            for j, pos in enumerate((p_p[b], n1_p[b], n2_p[b])):
                col = 3 * s + j
                selm[av, col] += 1.0
                selm[int(pos) + S * s, col] -= 1.0
        mm = dict(base)
        mm["xt"] = bf(seq[sl].reshape(T, H).T)
        mm["onehot"] = bf(oh)
        mm["ctxmask"] = bf(cm)
        mm["sel"] = bf(selm)
        in_maps.append(mm)
    return in_maps


def kernel(**inputs):
    nc = _get_nc()
    in_maps = _shard(inputs)
    res = run_bass_kernel_spmd(nc, in_maps, core_ids=list(range(NCORES)))
    ce = quad = ctx = 0.0
    for c in range(NCORES):
        o = np.asarray(res.results[c]["out"], np.float64).reshape(8)
        quad += float(o[0])
        ctx += float(o[1] + o[2])
        ce += float(o[3] + o[4] - o[5] - o[6])
    total = ce / (B * S) + ALPHA * (quad / B) + BETA * (ctx / (B * S))
    return np.float32(total)
